# revision 14
# baseline (speedup 1.0000x reference)
"""Trainium2 Bass kernel for prefix-LM CausalSelfAttention (v2).

Problem: B=2, T=2048, C=2048, H=16 heads (hd=128), prefix-LM mask
(bidirectional over first half, causal after), RoPE on q/k.

Sharding over 8 cores: data-parallel on batch (2) x tensor-parallel on
heads (4 heads per core). Each core computes a partial output projection
(its heads' contribution); partials are summed on host.

v2 design (vs v1): bf16 data everywhere (validated 6.6e-3 rel err), x
resident in SBUF once (no second DMA pass), RoPE fused per-(m,chunk)
into stage A so DVE work hides under the QKV matmuls, attention exp
batched 2 key-tiles wide on ACT, softmax denominator via DVE-accumulated
pp sum + 4 tiny transposed matmuls + [128,4] reciprocal (replaces the
per-tile ones-matmuls and the 3.3us single-lane [1,512] reciprocal),
and the output projection interleaved into attention as PE filler.

Per-core dataflow:
  A. qkT[m] = W_{q,k}^T @ x^T per 512-chunk; RoPE combine per tile:
     rope = qkT*cos + (R @ qkT)*sin  (R = pair swap w/ sign)
  C. v[t-tile] = x @ Wv (natural layout)
  D. per (I, h): S'[j,i] tiles via k^T-tile x q-chunk, exp on ACT
     (2 tiles per ACTIVATE), pp accumulated on DVE for the denominator,
     PV accumulation into y^T psum; dT = pp_acc^T @ ones via 4 M=1
     matmuls, reciprocal, PE-transpose, gpsimd row broadcast, normalize.
  E. out[mt, n] = sum_hk yT[hk]^T @ Wp[hk], emitted as PE filler between
     attention batches; evacuation alternates ACT/DVE.
"""
import math

import numpy as np

N_HEAD = 16
B = 2
T = 2048
C = 2048
HD = 128
HPC = 4          # heads per core
CL = HPC * HD    # local C = 512
TC = 512         # chunk width (matmul moving free dim / psum bank)
NT = T // TC     # 4 chunks
KT = C // 128    # 16 contraction tiles over C
TT = T // 128    # 16 T tiles
SCALE = 1.0 / math.sqrt(HD)

# Per query-chunk I: batches of two 128-key tiles (j0, j0+1); mp indexes
# the two 1024-wide diagonal mask pairs, None for fully-allowed batches.
_BATCHES = {
    0: [(0, None), (2, None), (4, None), (6, None)],
    1: [(0, None), (2, None), (4, None), (6, None)],
    2: [(0, None), (2, None), (4, None), (6, None), (8, 0), (10, 1)],
    3: [(0, None), (2, None), (4, None), (6, None), (8, None), (10, None),
        (12, 0), (14, 1)],
}

_CACHE = {}


def _build_nc():
    from collections import deque

    import concourse.tile as tile
    import concourse.mybir as mybir
    from concourse import bacc

    f32 = mybir.dt.float32
    f32r = mybir.dt.float32r
    bf = mybir.dt.bfloat16
    Exp = mybir.ActivationFunctionType.Exp

    nc = bacc.Bacc(None, target_bir_lowering=False)

    xT = nc.dram_tensor("xT", [C, T], bf, kind="ExternalInput")
    wqk = nc.dram_tensor("wqk", [C, 2 * CL], bf, kind="ExternalInput")
    wv = nc.dram_tensor("wv", [C, CL], bf, kind="ExternalInput")
    wp = nc.dram_tensor("wp", [CL, C], bf, kind="ExternalInput")
    cosP = nc.dram_tensor("cosP", [HD, T], bf, kind="ExternalInput")
    sinP = nc.dram_tensor("sinP", [HD, T], bf, kind="ExternalInput")
    rt = nc.dram_tensor("rt", [HD, HD], bf, kind="ExternalInput")
    masks = nc.dram_tensor("masks", [2, 128, 2 * TC], bf, kind="ExternalInput")
    ones = nc.dram_tensor("ones", [128, 1], bf, kind="ExternalInput")
    ident = nc.dram_tensor("ident", [128, 128], f32, kind="ExternalInput")
    out = nc.dram_tensor("out", [T, C], f32, kind="ExternalOutput")

    xT3 = xT.rearrange("(kt p) t -> p kt t", p=128)
    wqk3 = wqk.rearrange("(kt p) m -> p kt m", p=128)
    wv3 = wv.rearrange("(kt p) m -> p kt m", p=128)
    wp3 = wp.rearrange("(hk p) m -> p hk m", p=128)
    masks3 = masks.rearrange("g p u -> p g u")

    with tile.TileContext(nc) as tc:
        # Left stack: mpool/rope (long-lived), then x (..stage C), then the
        # per-phase pools on top in LIFO order. Right stack: v/wp/yT which
        # outlive x. PSUM pools form their own stack.
        mpool = tc.alloc_tile_pool(name="misc", bufs=1)
        rope_pool = tc.alloc_tile_pool(name="rope", bufs=1)   # ..attention
        xpool = tc.alloc_tile_pool(name="x_sb", bufs=1)       # ..stage C

        rt_sb = mpool.tile([HD, HD], bf)
        ones_sb = mpool.tile([128, 1], bf)
        ident_sb = mpool.tile([128, 128], f32)
        mask_sb = mpool.tile([128, 2, 2 * TC], bf)
        warm_sb = mpool.tile([128, TC], bf)
        dume_sb = mpool.tile([128, 2], bf)

        # HAM warmup: PE matmuls on memset data while input DMAs stream,
        # so stage A starts at K=8/8. Also pre-trigger the exp table load
        # and the gpsimd library load (first partition_broadcast otherwise
        # costs ~9us mid-attention).
        nc.vector.memset(warm_sb, 0.0)
        nc.scalar.activation(out=dume_sb, in_=warm_sb[:, 0:2], func=Exp)
        dumg_sb = mpool.tile([128, 4], bf)
        nc.gpsimd.partition_broadcast(dumg_sb, warm_sb[0:1, 0:4])
        ps_w = tc.alloc_tile_pool(name="ps_warm", bufs=1, space="PSUM")
        for _ in range(26):
            pw = ps_w.tile([128, TC], f32, tag="pw", name="pw")
            nc.tensor.matmul(pw, warm_sb[:, 0:128], warm_sb, start=True,
                             stop=True)
        ps_w.release()

        # ---- input DMAs (sync-queue order = arrival order) ----
        wpool = tc.alloc_tile_pool(name="wqk_sb", bufs=1)     # ..stage A
        tpool = tc.alloc_tile_pool(name="trig", bufs=1)       # ..stage A
        qk_pool = tc.alloc_tile_pool(name="qk", bufs=1)       # ..stage A

        w_t = []
        x_t = {}
        for k in range(KT):
            wt = wpool.tile([128, 2 * CL], bf, tag=f"w{k}", name=f"w{k}")
            nc.sync.dma_start(out=wt, in_=wqk3[:, k])
            w_t.append(wt)
            xt = xpool.tile([128, TC], bf, tag=f"x{k}_0", name=f"x{k}_0")
            nc.sync.dma_start(out=xt, in_=xT3[:, k, 0:TC])
            x_t[(k, 0)] = xt
        cos_sb = tpool.tile([HD, T], bf)
        sin_sb = tpool.tile([HD, T], bf)
        nc.sync.dma_start(out=rt_sb, in_=rt[:, :])
        nc.sync.dma_start(out=ident_sb, in_=ident[:, :])
        nc.sync.dma_start(out=ones_sb, in_=ones[:, :])
        nc.sync.dma_start(out=cos_sb, in_=cosP[:, :])
        nc.sync.dma_start(out=sin_sb, in_=sinP[:, :])
        for n in range(1, NT):
            for k in range(KT):
                xt = xpool.tile([128, TC], bf, tag=f"x{k}_{n}",
                                name=f"x{k}_{n}")
                nc.sync.dma_start(out=xt, in_=xT3[:, k, n * TC:(n + 1) * TC])
                x_t[(k, n)] = xt

        # ---- stage A: qkT + fused RoPE ----
        ps_a = tc.alloc_tile_pool(name="ps_a", bufs=4, space="PSUM")
        ps_r = tc.alloc_tile_pool(name="ps_rot", bufs=2, space="PSUM")
        rtmp = tc.alloc_tile_pool(name="rope_tmp", bufs=1)

        qk_sb = [qk_pool.tile([128, T], bf, tag=f"qk{m}", name=f"qk{m}")
                 for m in range(8)]
        rope_sb = [rope_pool.tile([128, T], bf, tag=f"ro{m}", name=f"ro{m}")
                   for m in range(8)]

        def emit_rope(m, nsl):
            # R @ qk on PE (pair swap w/ sign), combine on DVE in bf16.
            psr = ps_r.tile([128, TC], f32, tag="ps_r", name="ps_r")
            nc.tensor.matmul(psr, rt_sb, qk_sb[m][:, nsl], start=True,
                             stop=True)
            t1 = rtmp.tile([128, TC], bf, tag="t1", name="t1", bufs=2)
            nc.vector.tensor_mul(t1, psr, sin_sb[:, nsl])
            t2 = rtmp.tile([128, TC], bf, tag="t2", name="t2", bufs=2)
            nc.vector.tensor_mul(t2, qk_sb[m][:, nsl], cos_sb[:, nsl])
            nc.vector.tensor_add(rope_sb[m][:, nsl], t1, t2)

        pending_rope = None
        for n in range(NT):
            nsl = slice(n * TC, (n + 1) * TC)
            for m in range(8):
                ps = ps_a.tile([128, TC], f32, tag="ps_a", name="ps_a")
                for k in range(KT):
                    nc.tensor.matmul(ps, w_t[k][:, m * 128:(m + 1) * 128],
                                     x_t[(k, n)],
                                     start=(k == 0), stop=(k == KT - 1))
                nc.scalar.copy(out=qk_sb[m][:, nsl], in_=ps)
                # rope of the PREVIOUS tile: its ACT copy finished during
                # this group's 16 matmuls, so the R-matmul never stalls PE.
                if pending_rope is not None:
                    emit_rope(*pending_rope)
                pending_rope = (m, nsl)
        emit_rope(*pending_rope)

        rtmp.release()
        ps_r.release()
        ps_a.release()
        qk_pool.release()
        tpool.release()
        wpool.release()

        # ---- stage C: v = x @ Wv; wp/masks DMAs land during this phase ----
        wvpool = tc.alloc_tile_pool(name="wv_sb", bufs=1)     # ..stage C
        v_pool = tc.alloc_tile_pool(name="v_sb", bufs=1, side="right")
        wppool = tc.alloc_tile_pool(name="wp_sb", bufs=1, side="right")

        wv_t = []
        for k in range(KT):
            wt = wvpool.tile([128, CL], bf, tag=f"wv{k}", name=f"wv{k}")
            nc.sync.dma_start(out=wt, in_=wv3[:, k])
            wv_t.append(wt)
        wp_t = []
        for hk in range(HPC):
            wt = wppool.tile([128, C], bf, tag=f"wp{hk}", name=f"wp{hk}")
            nc.sync.dma_start(out=wt, in_=wp3[:, hk])
            wp_t.append(wt)
        nc.sync.dma_start(out=mask_sb, in_=masks3)

        v_t = [v_pool.tile([128, CL], bf, tag=f"v{mt}", name=f"v{mt}")
               for mt in range(TT)]
        ps_c = tc.alloc_tile_pool(name="ps_c", bufs=4, space="PSUM")
        for mt in range(TT):
            ps = ps_c.tile([128, CL], f32, tag="ps_c", name="ps_c")
            n, off = mt // 4, (mt % 4) * 128
            for k in range(KT):
                nc.tensor.matmul(ps, x_t[(k, n)][:, off:off + 128], wv_t[k],
                                 start=(k == 0), stop=(k == KT - 1))
            nc.scalar.copy(out=v_t[mt], in_=ps)
        ps_c.release()
        wvpool.release()
        xpool.release()

        # ---- stage D attention + stage E (proj) as PE filler ----
        y_pool = tc.alloc_tile_pool(name="yT_sb", bufs=1, side="right")
        yT = [y_pool.tile([128, T], bf, tag=f"yT{h}", name=f"yT{h}")
              for h in range(HPC)]

        pp_pool = tc.alloc_tile_pool(name="pp", bufs=1)
        acc_pool = tc.alloc_tile_pool(name="accp", bufs=1)
        sm_pool = tc.alloc_tile_pool(name="small", bufs=1)
        o_pool = tc.alloc_tile_pool(name="ostage", bufs=1)
        ps_s = tc.alloc_tile_pool(name="ps_s", bufs=2, space="PSUM")
        ps_y = tc.alloc_tile_pool(name="ps_y", bufs=1, space="PSUM")
        ps_d = tc.alloc_tile_pool(name="ps_d", bufs=1, space="PSUM")
        ps_o = tc.alloc_tile_pool(name="ps_o", bufs=1, space="PSUM")

        e_jobs = deque()
        e_count = [0]
        e_pool = [ps_o]

        def emit_e_group():
            if not e_jobs:
                return
            mt, n2 = e_jobs.popleft()
            msl = slice(mt * 128, (mt + 1) * 128)
            nsl = slice(n2 * TC, (n2 + 1) * TC)
            pso = e_pool[0].tile([128, TC], f32, tag="o", name="o_ps")
            for hk in range(HPC):
                nc.tensor.matmul(pso, yT[hk][:, msl], wp_t[hk][:, nsl],
                                 start=(hk == 0), stop=(hk == HPC - 1))
            ot = o_pool.tile([128, TC], f32, tag="ot", name="ot", bufs=4)
            # alternate evacuation engine to balance ACT vs DVE load
            if e_count[0] % 2 == 0:
                nc.scalar.copy(out=ot, in_=pso)
            else:
                nc.vector.tensor_copy(out=ot, in_=pso)
            e_count[0] += 1
            nc.sync.dma_start(out=out[msl, nsl], in_=ot)

        for I in range(NT):
            isl = slice(I * TC, (I + 1) * TC)
            for h in range(HPC):
                q_h = rope_sb[h]
                k_h = rope_sb[4 + h]
                bt = _BATCHES[I]
                nb = len(bt)
                acc = acc_pool.tile([128, TC], bf, tag="acc", name="acc",
                                    bufs=2)
                y_ps = ps_y.tile([128, TC], f32, tag="y", name="y_ps")
                for bi, (j0, mp) in enumerate(bt):
                    s_ps = ps_s.tile([128, 2 * TC], f32, tag="s", name="s_ps")
                    for half in range(2):
                        J = j0 + half
                        nc.tensor.matmul(
                            s_ps[:, half * TC:(half + 1) * TC],
                            k_h[:, J * 128:(J + 1) * 128], q_h[:, isl],
                            start=True, stop=True, skip_group_check=True,
                        )
                    pp = pp_pool.tile([128, 2 * TC], bf, tag="pp", name="pp",
                                      bufs=2)
                    nc.scalar.activation(out=pp, in_=s_ps, func=Exp,
                                         scale=SCALE)
                    # PE filler between S and PV so the PV matmuls never
                    # head-of-line block on the exp latency
                    emit_e_group()
                    src = pp
                    if mp is not None:
                        ppm = pp_pool.tile([128, 2 * TC], bf, tag="ppm",
                                           name="ppm", bufs=2)
                        nc.vector.tensor_mul(ppm, pp, mask_sb[:, mp])
                        src = ppm
                    if bi == 0:
                        nc.vector.tensor_copy(out=acc, in_=src[:, 0:TC])
                    else:
                        nc.vector.tensor_add(acc, acc, src[:, 0:TC])
                    nc.gpsimd.tensor_add(acc, acc, src[:, TC:2 * TC])
                    for half in range(2):
                        J = j0 + half
                        nc.tensor.matmul(
                            y_ps, v_t[J][:, h * 128:(h + 1) * 128],
                            src[:, half * TC:(half + 1) * TC],
                            start=(bi == 0 and half == 0),
                            stop=(bi == nb - 1 and half == 1),
                        )
                # denominator (transposed layout) + normalize
                d_ps = ps_d.tile([128, 4], f32, tag="d", name="d_ps")
                for qq in range(4):
                    nc.tensor.matmul(d_ps[:, qq:qq + 1],
                                     acc[:, qq * 128:(qq + 1) * 128],
                                     ones_sb, start=True, stop=True,
                                     skip_group_check=True)
                recip = sm_pool.tile([128, 4], f32, tag="recip",
                                     name="recip", bufs=2)
                nc.vector.reciprocal(out=recip, in_=d_ps)
                # 4 column transposes into one [1, 512] psum row so the
                # gpsimd broadcast reads from partition 0 in one shot
                tT_ps = ps_d.tile([1, TC], f32, tag="tT", name="tT_ps")
                for qq in range(4):
                    nc.tensor.transpose(tT_ps[:, qq * 128:(qq + 1) * 128],
                                        recip[:, qq:qq + 1], ident_sb)
                recipT = sm_pool.tile([1, TC], f32, tag="recipT",
                                      name="recipT", bufs=2)
                nc.vector.tensor_copy(out=recipT, in_=tT_ps)
                recipB = sm_pool.tile([128, TC], f32, tag="recipB",
                                      name="recipB", bufs=2)
                nc.gpsimd.partition_broadcast(recipB, recipT)
                nc.vector.tensor_mul(yT[h][:, isl], y_ps, recipB)
            for ml in range(4):
                for n2 in range(NT):
                    e_jobs.append((4 * I + ml, n2))
        # tail: attention psum pools are done; hand the remaining E groups
        # a 4-deep psum pool so the group->evacuate->DMA chain pipelines
        ps_o.release()
        ps_d.release()
        ps_y.release()
        ps_tail = tc.alloc_tile_pool(name="ps_tail", bufs=4, space="PSUM")
        e_pool[0] = ps_tail
        while e_jobs:
            emit_e_group()

        for p in (o_pool, sm_pool, acc_pool, pp_pool, rope_pool, mpool,
                  y_pool, wppool, v_pool, ps_tail, ps_s):
            p.release()
    nc.compile()
    return nc


def _host_prep(x, w_qkv, w_proj, freqs_cis):
    """Build per-core input maps (slicing + layout + dtype prep only)."""
    import ml_dtypes
    BF = ml_dtypes.bfloat16

    x = np.asarray(x, dtype=np.float32)
    w_qkv = np.asarray(w_qkv, dtype=np.float32)
    w_proj = np.asarray(w_proj, dtype=np.float32)
    fc = np.asarray(freqs_cis, dtype=np.float32)

    xTb = [np.ascontiguousarray(x[b].T).astype(BF) for b in range(B)]

    cos = fc[:, :, 0].T  # [64, T]
    sin = fc[:, :, 1].T
    cosP = np.repeat(cos, 2, axis=0).astype(BF)  # [128, T]
    sinP = np.repeat(sin, 2, axis=0).astype(BF)

    rt = np.zeros((HD, HD), dtype=np.float32)
    for d in range(HD // 2):
        rt[2 * d, 2 * d + 1] = 1.0
        rt[2 * d + 1, 2 * d] = -1.0
    rt = rt.astype(BF)

    # masks[p][jj, u]: pair p covers diagonal tiles d = 2p + u//TC
    masks = np.zeros((2, 128, 2 * TC), dtype=np.float32)
    jj = np.arange(128)[:, None]
    for p in range(2):
        for tp in range(2):
            d = 2 * p + tp
            ii = np.arange(TC)[None, :]
            masks[p][:, tp * TC:(tp + 1) * TC] = (ii >= jj + 128 * d)
    masks = masks.astype(BF)

    ones = np.ones((128, 1), dtype=np.float32).astype(BF)
    ident = np.eye(128, dtype=np.float32)

    in_maps = []
    for core in range(8):
        b = core // 4
        g = core % 4
        qc = w_qkv[:, 512 * g: 512 * (g + 1)]
        kc = w_qkv[:, 2048 + 512 * g: 2048 + 512 * (g + 1)]
        vc = w_qkv[:, 4096 + 512 * g: 4096 + 512 * (g + 1)]
        wqk_c = np.concatenate([qc, kc], axis=1).astype(BF)
        wv_c = np.ascontiguousarray(vc).astype(BF)
        wp_c = np.ascontiguousarray(
            w_proj[512 * g: 512 * (g + 1), :]).astype(BF)
        in_maps.append({
            "xT": xTb[b],
            "wqk": wqk_c,
            "wv": wv_c,
            "wp": wp_c,
            "cosP": cosP,
            "sinP": sinP,
            "rt": rt,
            "masks": masks,
            "ones": ones,
            "ident": ident,
        })
    return in_maps


def _get_nc():
    if "nc" not in _CACHE:
        _CACHE["nc"] = _build_nc()
    return _CACHE["nc"]


def kernel(x, w_qkv, w_proj, freqs_cis, attn_mask, _trace=False):
    from concourse.bass_utils import run_bass_kernel_spmd

    in_maps = _host_prep(x, w_qkv, w_proj, freqs_cis)
    nc = _get_nc()
    res = run_bass_kernel_spmd(
        nc, in_maps, core_ids=list(range(8)), trace=_trace,
    )
    outs = [r["out"].astype(np.float64) for r in res.results]
    full = np.stack([
        outs[0] + outs[1] + outs[2] + outs[3],
        outs[4] + outs[5] + outs[6] + outs[7],
    ]).astype(np.float32)
    if _trace:
        kernel._last_results = res
    return full


# revision 15
# speedup vs baseline: 1.5710x; 1.5710x over previous
"""Trainium2 Bass kernel for prefix-LM CausalSelfAttention (v2).

Problem: B=2, T=2048, C=2048, H=16 heads (hd=128), prefix-LM mask
(bidirectional over first half, causal after), RoPE on q/k.

Sharding over 8 cores: data-parallel on batch (2) x tensor-parallel on
heads (4 heads per core). Each core computes a partial output projection
(its heads' contribution); partials are summed on host.

v2 design (vs v1): bf16 data everywhere (validated 6.6e-3 rel err), x
resident in SBUF once (no second DMA pass), RoPE fused per-(m,chunk)
into stage A so DVE work hides under the QKV matmuls, attention exp
batched 2 key-tiles wide on ACT, softmax denominator via DVE-accumulated
pp sum + 4 tiny transposed matmuls + [128,4] reciprocal (replaces the
per-tile ones-matmuls and the 3.3us single-lane [1,512] reciprocal),
and the output projection interleaved into attention as PE filler.

Per-core dataflow:
  A. qkT[m] = W_{q,k}^T @ x^T per 512-chunk; RoPE combine per tile:
     rope = qkT*cos + (R @ qkT)*sin  (R = pair swap w/ sign)
  C. v[t-tile] = x @ Wv (natural layout)
  D. per (I, h): S'[j,i] tiles via k^T-tile x q-chunk, exp on ACT
     (2 tiles per ACTIVATE), pp accumulated on DVE for the denominator,
     PV accumulation into y^T psum; dT = pp_acc^T @ ones via 4 M=1
     matmuls, reciprocal, PE-transpose, gpsimd row broadcast, normalize.
  E. out[mt, n] = sum_hk yT[hk]^T @ Wp[hk], emitted as PE filler between
     attention batches; evacuation alternates ACT/DVE.
"""
import math

import numpy as np

N_HEAD = 16
B = 2
T = 2048
C = 2048
HD = 128
HPC = 4          # heads per core
CL = HPC * HD    # local C = 512
TC = 512         # chunk width (matmul moving free dim / psum bank)
NT = T // TC     # 4 chunks
KT = C // 128    # 16 contraction tiles over C
TT = T // 128    # 16 T tiles
SCALE = 1.0 / math.sqrt(HD)

# Per query-chunk I: batches of two 128-key tiles (j0, j0+1); mp indexes
# the two 1024-wide diagonal mask pairs, None for fully-allowed batches.
_BATCHES = {
    0: [(0, None), (2, None), (4, None), (6, None)],
    1: [(0, None), (2, None), (4, None), (6, None)],
    2: [(0, None), (2, None), (4, None), (6, None), (8, 0), (10, 1)],
    3: [(0, None), (2, None), (4, None), (6, None), (8, None), (10, None),
        (12, 0), (14, 1)],
}

_CACHE = {}


def _build_nc():
    from collections import deque

    import concourse.tile as tile
    import concourse.mybir as mybir
    from concourse import bacc

    f32 = mybir.dt.float32
    f32r = mybir.dt.float32r
    bf = mybir.dt.bfloat16
    Exp = mybir.ActivationFunctionType.Exp

    nc = bacc.Bacc(None, target_bir_lowering=False)

    xT = nc.dram_tensor("xT", [C, T], bf, kind="ExternalInput")
    wqk = nc.dram_tensor("wqk", [C, 2 * CL], bf, kind="ExternalInput")
    wv = nc.dram_tensor("wv", [C, CL], bf, kind="ExternalInput")
    wp = nc.dram_tensor("wp", [CL, C], bf, kind="ExternalInput")
    cosP = nc.dram_tensor("cosP", [HD, T], bf, kind="ExternalInput")
    sinP = nc.dram_tensor("sinP", [HD, T], bf, kind="ExternalInput")
    rt = nc.dram_tensor("rt", [HD, HD], bf, kind="ExternalInput")
    masks = nc.dram_tensor("masks", [2, 128, 2 * TC], bf, kind="ExternalInput")
    ones = nc.dram_tensor("ones", [128, 1], bf, kind="ExternalInput")
    ident = nc.dram_tensor("ident", [128, 128], f32, kind="ExternalInput")
    out = nc.dram_tensor("out", [T, C], f32, kind="ExternalOutput")

    xT3 = xT.rearrange("(kt p) t -> p kt t", p=128)
    wqk3 = wqk.rearrange("(kt p) m -> p kt m", p=128)
    wv3 = wv.rearrange("(kt p) m -> p kt m", p=128)
    wp3 = wp.rearrange("(hk p) m -> p hk m", p=128)
    masks3 = masks.rearrange("g p u -> p g u")

    with tile.TileContext(nc) as tc:
        # Left stack: mpool/rope (long-lived), then x (..stage C), then the
        # per-phase pools on top in LIFO order. Right stack: v/wp/yT which
        # outlive x. PSUM pools form their own stack.
        mpool = tc.alloc_tile_pool(name="misc", bufs=1)
        rope_pool = tc.alloc_tile_pool(name="rope", bufs=1)   # ..attention
        xpool = tc.alloc_tile_pool(name="x_sb", bufs=1)       # ..stage C

        rt_sb = mpool.tile([HD, HD], bf)
        ones_sb = mpool.tile([128, 1], bf)
        ident_sb = mpool.tile([128, 128], f32)
        mask_sb = mpool.tile([128, 2, 2 * TC], bf)
        warm_sb = mpool.tile([128, TC], bf)
        dume_sb = mpool.tile([128, 2], bf)

        # HAM warmup: PE matmuls on memset data while input DMAs stream,
        # so stage A starts at K=8/8. Also pre-trigger the exp table load
        # and the gpsimd library load (first partition_broadcast otherwise
        # costs ~9us mid-attention).
        nc.vector.memset(warm_sb, 0.0)
        nc.scalar.activation(out=dume_sb, in_=warm_sb[:, 0:2], func=Exp)
        dumg_sb = mpool.tile([128, 4], bf)
        nc.gpsimd.partition_broadcast(dumg_sb, warm_sb[0:1, 0:4])
        ps_w = tc.alloc_tile_pool(name="ps_warm", bufs=1, space="PSUM")
        for _ in range(26):
            pw = ps_w.tile([128, TC], f32, tag="pw", name="pw")
            nc.tensor.matmul(pw, warm_sb[:, 0:128], warm_sb, start=True,
                             stop=True)
        ps_w.release()

        # ---- input DMAs (sync-queue order = arrival order) ----
        wpool = tc.alloc_tile_pool(name="wqk_sb", bufs=1)     # ..stage A
        tpool = tc.alloc_tile_pool(name="trig", bufs=1)       # ..stage A
        qk_pool = tc.alloc_tile_pool(name="qk", bufs=1)       # ..stage A

        w_t = []
        x_t = {}
        for k in range(KT):
            wt = wpool.tile([128, 2 * CL], bf, tag=f"w{k}", name=f"w{k}")
            nc.sync.dma_start(out=wt, in_=wqk3[:, k])
            w_t.append(wt)
            xt = xpool.tile([128, TC], bf, tag=f"x{k}_0", name=f"x{k}_0")
            nc.sync.dma_start(out=xt, in_=xT3[:, k, 0:TC])
            x_t[(k, 0)] = xt
        cos_sb = tpool.tile([HD, T], bf)
        sin_sb = tpool.tile([HD, T], bf)
        nc.sync.dma_start(out=rt_sb, in_=rt[:, :])
        nc.sync.dma_start(out=ident_sb, in_=ident[:, :])
        nc.sync.dma_start(out=ones_sb, in_=ones[:, :])
        nc.sync.dma_start(out=cos_sb, in_=cosP[:, :])
        nc.sync.dma_start(out=sin_sb, in_=sinP[:, :])
        for n in range(1, NT):
            for k in range(KT):
                xt = xpool.tile([128, TC], bf, tag=f"x{k}_{n}",
                                name=f"x{k}_{n}")
                nc.sync.dma_start(out=xt, in_=xT3[:, k, n * TC:(n + 1) * TC])
                x_t[(k, n)] = xt

        # ---- stage A: qkT + fused RoPE ----
        ps_a = tc.alloc_tile_pool(name="ps_a", bufs=4, space="PSUM")
        ps_r = tc.alloc_tile_pool(name="ps_rot", bufs=2, space="PSUM")
        rtmp = tc.alloc_tile_pool(name="rope_tmp", bufs=1)

        qk_sb = [qk_pool.tile([128, T], bf, tag=f"qk{m}", name=f"qk{m}")
                 for m in range(8)]
        rope_sb = [rope_pool.tile([128, T], bf, tag=f"ro{m}", name=f"ro{m}")
                   for m in range(8)]

        def emit_rope(m, nsl):
            # R @ qk on PE (pair swap w/ sign), combine on DVE in bf16.
            psr = ps_r.tile([128, TC], f32, tag="ps_r", name="ps_r")
            nc.tensor.matmul(psr, rt_sb, qk_sb[m][:, nsl], start=True,
                             stop=True)
            t1 = rtmp.tile([128, TC], bf, tag="t1", name="t1", bufs=2)
            nc.vector.tensor_mul(t1, psr, sin_sb[:, nsl])
            t2 = rtmp.tile([128, TC], bf, tag="t2", name="t2", bufs=2)
            nc.vector.tensor_mul(t2, qk_sb[m][:, nsl], cos_sb[:, nsl])
            nc.vector.tensor_add(rope_sb[m][:, nsl], t1, t2)

        pending_rope = None
        for n in range(NT):
            nsl = slice(n * TC, (n + 1) * TC)
            for m in range(8):
                ps = ps_a.tile([128, TC], f32, tag="ps_a", name="ps_a")
                for k in range(KT):
                    nc.tensor.matmul(ps, w_t[k][:, m * 128:(m + 1) * 128],
                                     x_t[(k, n)],
                                     start=(k == 0), stop=(k == KT - 1))
                nc.scalar.copy(out=qk_sb[m][:, nsl], in_=ps)
                # rope of the PREVIOUS tile: its ACT copy finished during
                # this group's 16 matmuls, so the R-matmul never stalls PE.
                if pending_rope is not None:
                    emit_rope(*pending_rope)
                pending_rope = (m, nsl)
        emit_rope(*pending_rope)

        rtmp.release()
        ps_r.release()
        ps_a.release()
        qk_pool.release()
        tpool.release()
        wpool.release()

        # ---- stage C: v = x @ Wv; wp/masks DMAs land during this phase ----
        wvpool = tc.alloc_tile_pool(name="wv_sb", bufs=1)     # ..stage C
        v_pool = tc.alloc_tile_pool(name="v_sb", bufs=1, side="right")
        wppool = tc.alloc_tile_pool(name="wp_sb", bufs=1, side="right")

        wv_t = []
        for k in range(KT):
            wt = wvpool.tile([128, CL], bf, tag=f"wv{k}", name=f"wv{k}")
            nc.sync.dma_start(out=wt, in_=wv3[:, k])
            wv_t.append(wt)
        wp_t = []
        for hk in range(HPC):
            wt = wppool.tile([128, C], bf, tag=f"wp{hk}", name=f"wp{hk}")
            nc.sync.dma_start(out=wt, in_=wp3[:, hk])
            wp_t.append(wt)
        nc.sync.dma_start(out=mask_sb, in_=masks3)

        v_t = [v_pool.tile([128, CL], bf, tag=f"v{mt}", name=f"v{mt}")
               for mt in range(TT)]
        ps_c = tc.alloc_tile_pool(name="ps_c", bufs=4, space="PSUM")
        for mt in range(TT):
            ps = ps_c.tile([128, CL], f32, tag="ps_c", name="ps_c")
            n, off = mt // 4, (mt % 4) * 128
            for k in range(KT):
                nc.tensor.matmul(ps, x_t[(k, n)][:, off:off + 128], wv_t[k],
                                 start=(k == 0), stop=(k == KT - 1))
            nc.scalar.copy(out=v_t[mt], in_=ps)
        ps_c.release()
        wvpool.release()
        xpool.release()

        # ---- stage D attention + stage E (proj) as PE filler ----
        y_pool = tc.alloc_tile_pool(name="yT_sb", bufs=1, side="right")
        yT = [y_pool.tile([128, T], bf, tag=f"yT{h}", name=f"yT{h}")
              for h in range(HPC)]

        pp_pool = tc.alloc_tile_pool(name="pp", bufs=1)
        acc_pool = tc.alloc_tile_pool(name="accp", bufs=1)
        sm_pool = tc.alloc_tile_pool(name="small", bufs=1)
        o_pool = tc.alloc_tile_pool(name="ostage", bufs=1)
        ps_s = tc.alloc_tile_pool(name="ps_s", bufs=2, space="PSUM")
        ps_y = tc.alloc_tile_pool(name="ps_y", bufs=1, space="PSUM")
        ps_d = tc.alloc_tile_pool(name="ps_d", bufs=1, space="PSUM")
        ps_o = tc.alloc_tile_pool(name="ps_o", bufs=1, space="PSUM")

        e_jobs = deque()
        e_count = [0]
        e_pool = [ps_o]

        def emit_e_group():
            if not e_jobs:
                return
            mt, n2 = e_jobs.popleft()
            msl = slice(mt * 128, (mt + 1) * 128)
            nsl = slice(n2 * TC, (n2 + 1) * TC)
            pso = e_pool[0].tile([128, TC], f32, tag="o", name="o_ps")
            for hk in range(HPC):
                nc.tensor.matmul(pso, yT[hk][:, msl], wp_t[hk][:, nsl],
                                 start=(hk == 0), stop=(hk == HPC - 1))
            ot = o_pool.tile([128, TC], f32, tag="ot", name="ot", bufs=4)
            # alternate evacuation engine to balance ACT vs DVE load
            if e_count[0] % 2 == 0:
                nc.scalar.copy(out=ot, in_=pso)
            else:
                nc.vector.tensor_copy(out=ot, in_=pso)
            e_count[0] += 1
            nc.sync.dma_start(out=out[msl, nsl], in_=ot)

        for I in range(NT):
            isl = slice(I * TC, (I + 1) * TC)
            for h in range(HPC):
                q_h = rope_sb[h]
                k_h = rope_sb[4 + h]
                bt = _BATCHES[I]
                nb = len(bt)
                acc = acc_pool.tile([128, TC], bf, tag="acc", name="acc",
                                    bufs=2)
                y_ps = ps_y.tile([128, TC], f32, tag="y", name="y_ps")
                for bi, (j0, mp) in enumerate(bt):
                    s_ps = ps_s.tile([128, 2 * TC], f32, tag="s", name="s_ps")
                    for half in range(2):
                        J = j0 + half
                        nc.tensor.matmul(
                            s_ps[:, half * TC:(half + 1) * TC],
                            k_h[:, J * 128:(J + 1) * 128], q_h[:, isl],
                            start=True, stop=True, skip_group_check=True,
                        )
                    pp = pp_pool.tile([128, 2 * TC], bf, tag="pp", name="pp",
                                      bufs=2)
                    nc.scalar.activation(out=pp, in_=s_ps, func=Exp,
                                         scale=SCALE)
                    # PE filler between S and PV so the PV matmuls never
                    # head-of-line block on the exp latency
                    emit_e_group()
                    src = pp
                    if mp is not None:
                        ppm = pp_pool.tile([128, 2 * TC], bf, tag="ppm",
                                           name="ppm", bufs=2)
                        nc.vector.tensor_mul(ppm, pp, mask_sb[:, mp])
                        src = ppm
                    if bi == 0:
                        nc.vector.tensor_copy(out=acc, in_=src[:, 0:TC])
                    else:
                        nc.vector.tensor_add(acc, acc, src[:, 0:TC])
                    nc.vector.tensor_add(acc, acc, src[:, TC:2 * TC])
                    for half in range(2):
                        J = j0 + half
                        nc.tensor.matmul(
                            y_ps, v_t[J][:, h * 128:(h + 1) * 128],
                            src[:, half * TC:(half + 1) * TC],
                            start=(bi == 0 and half == 0),
                            stop=(bi == nb - 1 and half == 1),
                        )
                # denominator (transposed layout) + normalize
                d_ps = ps_d.tile([128, 4], f32, tag="d", name="d_ps")
                for qq in range(4):
                    nc.tensor.matmul(d_ps[:, qq:qq + 1],
                                     acc[:, qq * 128:(qq + 1) * 128],
                                     ones_sb, start=True, stop=True,
                                     skip_group_check=True)
                recip = sm_pool.tile([128, 4], f32, tag="recip",
                                     name="recip", bufs=2)
                nc.vector.reciprocal(out=recip, in_=d_ps)
                # 4 column transposes into one [1, 512] psum row so the
                # gpsimd broadcast reads from partition 0 in one shot
                tT_ps = ps_d.tile([1, TC], f32, tag="tT", name="tT_ps")
                for qq in range(4):
                    nc.tensor.transpose(tT_ps[:, qq * 128:(qq + 1) * 128],
                                        recip[:, qq:qq + 1], ident_sb)
                recipT = sm_pool.tile([1, TC], f32, tag="recipT",
                                      name="recipT", bufs=2)
                nc.vector.tensor_copy(out=recipT, in_=tT_ps)
                recipB = sm_pool.tile([128, TC], f32, tag="recipB",
                                      name="recipB", bufs=2)
                nc.gpsimd.partition_broadcast(recipB, recipT)
                nc.vector.tensor_mul(yT[h][:, isl], y_ps, recipB)
            for ml in range(4):
                for n2 in range(NT):
                    e_jobs.append((4 * I + ml, n2))
        # tail: attention psum pools are done; hand the remaining E groups
        # a 4-deep psum pool so the group->evacuate->DMA chain pipelines
        ps_o.release()
        ps_d.release()
        ps_y.release()
        ps_tail = tc.alloc_tile_pool(name="ps_tail", bufs=4, space="PSUM")
        e_pool[0] = ps_tail
        while e_jobs:
            emit_e_group()

        for p in (o_pool, sm_pool, acc_pool, pp_pool, rope_pool, mpool,
                  y_pool, wppool, v_pool, ps_tail, ps_s):
            p.release()
    nc.compile()
    return nc


def _host_prep(x, w_qkv, w_proj, freqs_cis):
    """Build per-core input maps (slicing + layout + dtype prep only)."""
    import ml_dtypes
    BF = ml_dtypes.bfloat16

    x = np.asarray(x, dtype=np.float32)
    w_qkv = np.asarray(w_qkv, dtype=np.float32)
    w_proj = np.asarray(w_proj, dtype=np.float32)
    fc = np.asarray(freqs_cis, dtype=np.float32)

    xTb = [np.ascontiguousarray(x[b].T).astype(BF) for b in range(B)]

    cos = fc[:, :, 0].T  # [64, T]
    sin = fc[:, :, 1].T
    cosP = np.repeat(cos, 2, axis=0).astype(BF)  # [128, T]
    sinP = np.repeat(sin, 2, axis=0).astype(BF)

    rt = np.zeros((HD, HD), dtype=np.float32)
    for d in range(HD // 2):
        rt[2 * d, 2 * d + 1] = 1.0
        rt[2 * d + 1, 2 * d] = -1.0
    rt = rt.astype(BF)

    # masks[p][jj, u]: pair p covers diagonal tiles d = 2p + u//TC
    masks = np.zeros((2, 128, 2 * TC), dtype=np.float32)
    jj = np.arange(128)[:, None]
    for p in range(2):
        for tp in range(2):
            d = 2 * p + tp
            ii = np.arange(TC)[None, :]
            masks[p][:, tp * TC:(tp + 1) * TC] = (ii >= jj + 128 * d)
    masks = masks.astype(BF)

    ones = np.ones((128, 1), dtype=np.float32).astype(BF)
    ident = np.eye(128, dtype=np.float32)

    in_maps = []
    for core in range(8):
        b = core // 4
        g = core % 4
        qc = w_qkv[:, 512 * g: 512 * (g + 1)]
        kc = w_qkv[:, 2048 + 512 * g: 2048 + 512 * (g + 1)]
        vc = w_qkv[:, 4096 + 512 * g: 4096 + 512 * (g + 1)]
        wqk_c = np.concatenate([qc, kc], axis=1).astype(BF)
        wv_c = np.ascontiguousarray(vc).astype(BF)
        wp_c = np.ascontiguousarray(
            w_proj[512 * g: 512 * (g + 1), :]).astype(BF)
        in_maps.append({
            "xT": xTb[b],
            "wqk": wqk_c,
            "wv": wv_c,
            "wp": wp_c,
            "cosP": cosP,
            "sinP": sinP,
            "rt": rt,
            "masks": masks,
            "ones": ones,
            "ident": ident,
        })
    return in_maps


def _get_nc():
    if "nc" not in _CACHE:
        _CACHE["nc"] = _build_nc()
    return _CACHE["nc"]


def kernel(x, w_qkv, w_proj, freqs_cis, attn_mask, _trace=False):
    from concourse.bass_utils import run_bass_kernel_spmd

    in_maps = _host_prep(x, w_qkv, w_proj, freqs_cis)
    nc = _get_nc()
    res = run_bass_kernel_spmd(
        nc, in_maps, core_ids=list(range(8)), trace=_trace,
    )
    outs = [r["out"].astype(np.float64) for r in res.results]
    full = np.stack([
        outs[0] + outs[1] + outs[2] + outs[3],
        outs[4] + outs[5] + outs[6] + outs[7],
    ]).astype(np.float32)
    if _trace:
        kernel._last_results = res
    return full


# revision 22
# speedup vs baseline: 1.6836x; 1.0717x over previous
"""Trainium2 Bass kernel for prefix-LM CausalSelfAttention (v2).

Problem: B=2, T=2048, C=2048, H=16 heads (hd=128), prefix-LM mask
(bidirectional over first half, causal after), RoPE on q/k.

Sharding over 8 cores: data-parallel on batch (2) x tensor-parallel on
heads (4 heads per core). Each core computes a partial output projection
(its heads' contribution); partials are summed on host.

v2 design (vs v1): bf16 data everywhere (validated 6.6e-3 rel err), x
resident in SBUF once (no second DMA pass), RoPE fused per-(m,chunk)
into stage A so DVE work hides under the QKV matmuls, attention exp
batched 2 key-tiles wide on ACT, softmax denominator via DVE-accumulated
pp sum + 4 tiny transposed matmuls + [128,4] reciprocal (replaces the
per-tile ones-matmuls and the 3.3us single-lane [1,512] reciprocal),
and the output projection interleaved into attention as PE filler.

Per-core dataflow:
  A. qkT[m] = W_{q,k}^T @ x^T per 512-chunk; RoPE combine per tile:
     rope = qkT*cos + (R @ qkT)*sin  (R = pair swap w/ sign)
  C. v[t-tile] = x @ Wv (natural layout)
  D. per (I, h): S'[j,i] tiles via k^T-tile x q-chunk, exp on ACT
     (2 tiles per ACTIVATE), pp accumulated on DVE for the denominator,
     PV accumulation into y^T psum; dT = pp_acc^T @ ones via 4 M=1
     matmuls, reciprocal, PE-transpose, gpsimd row broadcast, normalize.
  E. out[mt, n] = sum_hk yT[hk]^T @ Wp[hk], emitted as PE filler between
     attention batches; evacuation alternates ACT/DVE.
"""
import math

import numpy as np

N_HEAD = 16
B = 2
T = 2048
C = 2048
HD = 128
HPC = 4          # heads per core
CL = HPC * HD    # local C = 512
TC = 512         # chunk width (matmul moving free dim / psum bank)
NT = T // TC     # 4 chunks
KT = C // 128    # 16 contraction tiles over C
TT = T // 128    # 16 T tiles
SCALE = 1.0 / math.sqrt(HD)

# Per query-chunk I: batches of two 128-key tiles (j0, j0+1); mp indexes
# the two 1024-wide diagonal mask pairs, None for fully-allowed batches.
_BATCHES = {
    0: [(0, None), (2, None), (4, None), (6, None)],
    1: [(0, None), (2, None), (4, None), (6, None)],
    2: [(0, None), (2, None), (4, None), (6, None), (8, 0), (10, 1)],
    3: [(0, None), (2, None), (4, None), (6, None), (8, None), (10, None),
        (12, 0), (14, 1)],
}

_CACHE = {}


def _build_nc():
    from collections import deque

    import concourse.tile as tile
    import concourse.mybir as mybir
    from concourse import bacc

    f32 = mybir.dt.float32
    f32r = mybir.dt.float32r
    bf = mybir.dt.bfloat16
    Exp = mybir.ActivationFunctionType.Exp

    nc = bacc.Bacc(None, target_bir_lowering=False)

    xT = nc.dram_tensor("xT", [C, T], bf, kind="ExternalInput")
    wqk = nc.dram_tensor("wqk", [C, 2 * CL], bf, kind="ExternalInput")
    wv = nc.dram_tensor("wv", [C, CL], bf, kind="ExternalInput")
    wp = nc.dram_tensor("wp", [CL, C], bf, kind="ExternalInput")
    cosP = nc.dram_tensor("cosP", [HD, T], bf, kind="ExternalInput")
    sinP = nc.dram_tensor("sinP", [HD, T], bf, kind="ExternalInput")
    rt = nc.dram_tensor("rt", [HD, HD], bf, kind="ExternalInput")
    masks = nc.dram_tensor("masks", [2, 128, 2 * TC], bf, kind="ExternalInput")
    ones = nc.dram_tensor("ones", [128, 1], bf, kind="ExternalInput")
    ident = nc.dram_tensor("ident", [128, 128], f32, kind="ExternalInput")
    out = nc.dram_tensor("out", [T, C], f32, kind="ExternalOutput")

    xT3 = xT.rearrange("(kt p) t -> p kt t", p=128)
    wqk3 = wqk.rearrange("(kt p) m -> p kt m", p=128)
    wv3 = wv.rearrange("(kt p) m -> p kt m", p=128)
    wp3 = wp.rearrange("(hk p) m -> p hk m", p=128)
    masks3 = masks.rearrange("g p u -> p g u")

    with tile.TileContext(nc) as tc:
        # Left stack: mpool/rope (long-lived), then x (..stage C), then the
        # per-phase pools on top in LIFO order. Right stack: v/wp/yT which
        # outlive x. PSUM pools form their own stack.
        mpool = tc.alloc_tile_pool(name="misc", bufs=1)
        rope_pool = tc.alloc_tile_pool(name="rope", bufs=1)   # ..attention
        xpool = tc.alloc_tile_pool(name="x_sb", bufs=1)       # ..stage C

        rt_sb = mpool.tile([HD, HD], bf)
        ones_sb = mpool.tile([128, 1], bf)
        ident_sb = mpool.tile([128, 128], f32)
        warm_sb = mpool.tile([128, 128], bf)
        dume_sb = mpool.tile([128, 2], bf)

        # HAM warmup: PE matmuls on memset data while input DMAs stream,
        # so stage A starts at K=8/8. Also pre-trigger the exp table load
        # and the gpsimd library load (first partition_broadcast otherwise
        # costs ~9us mid-attention).
        nc.vector.memset(warm_sb, 0.0)
        nc.scalar.activation(out=dume_sb, in_=warm_sb[:, 0:2], func=Exp)
        dumg_sb = mpool.tile([128, 4], bf)
        nc.gpsimd.partition_broadcast(dumg_sb, warm_sb[0:1, 0:4])
        ps_w = tc.alloc_tile_pool(name="ps_warm", bufs=1, space="PSUM")
        for _ in range(44):
            pw = ps_w.tile([128, 128], f32, tag="pw", name="pw")
            nc.tensor.matmul(pw, warm_sb, warm_sb, start=True, stop=True)
        ps_w.release()

        # Long-lived pools on the right stack so their DMAs land in fresh
        # address space (no WAR on released stage-A pools) and can be
        # emitted early in the sync queue.
        wvpool = tc.alloc_tile_pool(name="wv_sb", bufs=1, side="right")

        # ---- input DMAs (sync-queue order = arrival order) ----
        wpool = tc.alloc_tile_pool(name="wqk_sb", bufs=1)     # ..stage A
        tpool = tc.alloc_tile_pool(name="trig", bufs=1)       # ..stage A
        qk_pool = tc.alloc_tile_pool(name="qk", bufs=1)       # ..stage A

        w_t = []
        x_t = {}
        for k in range(KT):
            wt = wpool.tile([128, 2 * CL], bf, tag=f"w{k}", name=f"w{k}")
            nc.sync.dma_start(out=wt, in_=wqk3[:, k])
            w_t.append(wt)
            xt = xpool.tile([128, TC], bf, tag=f"x{k}_0", name=f"x{k}_0")
            nc.sync.dma_start(out=xt, in_=xT3[:, k, 0:TC])
            x_t[(k, 0)] = xt
        cos_sb = tpool.tile([HD, T], bf)
        sin_sb = tpool.tile([HD, T], bf)
        nc.sync.dma_start(out=rt_sb, in_=rt[:, :])
        nc.sync.dma_start(out=ident_sb, in_=ident[:, :])
        nc.sync.dma_start(out=ones_sb, in_=ones[:, :])
        nc.sync.dma_start(out=cos_sb, in_=cosP[:, :])
        nc.sync.dma_start(out=sin_sb, in_=sinP[:, :])
        wv_t = []
        wp_t = []
        for n in range(1, NT):
            for k in range(KT):
                xt = xpool.tile([128, TC], bf, tag=f"x{k}_{n}",
                                name=f"x{k}_{n}")
                nc.sync.dma_start(out=xt, in_=xT3[:, k, n * TC:(n + 1) * TC])
                x_t[(k, n)] = xt
            if n == 1:
                for k in range(KT):
                    wt = wvpool.tile([128, CL], bf, tag=f"wv{k}",
                                     name=f"wv{k}")
                    nc.sync.dma_start(out=wt, in_=wv3[:, k])
                    wv_t.append(wt)

        # ---- stage A: qkT + fused RoPE ----
        ps_a = tc.alloc_tile_pool(name="ps_a", bufs=4, space="PSUM")
        ps_r = tc.alloc_tile_pool(name="ps_rot", bufs=2, space="PSUM")
        rtmp = tc.alloc_tile_pool(name="rope_tmp", bufs=1)

        rope_sb = [rope_pool.tile([128, T], bf, tag=f"ro{m}", name=f"ro{m}")
                   for m in range(8)]

        def emit_rope(m, nsl, qkt):
            # R @ qk on PE (pair swap w/ sign), combine on DVE in bf16.
            psr = ps_r.tile([128, TC], f32, tag="ps_r", name="ps_r")
            nc.tensor.matmul(psr, rt_sb, qkt, start=True, stop=True)
            t1 = rtmp.tile([128, TC], bf, tag="t1", name="t1", bufs=2)
            nc.vector.tensor_mul(t1, psr, sin_sb[:, nsl])
            t2 = rtmp.tile([128, TC], bf, tag="t2", name="t2", bufs=2)
            nc.vector.tensor_mul(t2, qkt, cos_sb[:, nsl])
            nc.vector.tensor_add(rope_sb[m][:, nsl], t1, t2)

        pending_rope = None
        for n in range(NT):
            nsl = slice(n * TC, (n + 1) * TC)
            for m in range(8):
                ps = ps_a.tile([128, TC], f32, tag="ps_a", name="ps_a")
                for k in range(KT):
                    nc.tensor.matmul(ps, w_t[k][:, m * 128:(m + 1) * 128],
                                     x_t[(k, n)],
                                     start=(k == 0), stop=(k == KT - 1))
                qkt = qk_pool.tile([128, TC], bf, tag="qkt", name="qkt",
                                   bufs=2)
                nc.scalar.copy(out=qkt, in_=ps)
                # rope of the PREVIOUS tile: its ACT copy finished during
                # this group's 16 matmuls, so the R-matmul never stalls PE.
                if pending_rope is not None:
                    emit_rope(*pending_rope)
                pending_rope = (m, nsl, qkt)
        emit_rope(*pending_rope)

        rtmp.release()
        ps_r.release()
        ps_a.release()
        qk_pool.release()
        tpool.release()
        wpool.release()

        # ---- stage C: v = x @ Wv ----
        v_pool = tc.alloc_tile_pool(name="v_sb", bufs=1, side="right")
        wppool = tc.alloc_tile_pool(name="wp_sb", bufs=1, side="right")
        y_pool = tc.alloc_tile_pool(name="yT_sb", bufs=1, side="right")
        for hk in range(HPC):
            wt = wppool.tile([128, C], bf, tag=f"wp{hk}", name=f"wp{hk}")
            nc.sync.dma_start(out=wt, in_=wp3[:, hk])
            wp_t.append(wt)
        v_t = [v_pool.tile([128, CL], bf, tag=f"v{mt}", name=f"v{mt}")
               for mt in range(TT)]
        ps_c = tc.alloc_tile_pool(name="ps_c", bufs=4, space="PSUM")
        for mt in range(TT):
            ps = ps_c.tile([128, CL], f32, tag="ps_c", name="ps_c")
            n, off = mt // 4, (mt % 4) * 128
            for k in range(KT):
                nc.tensor.matmul(ps, x_t[(k, n)][:, off:off + 128], wv_t[k],
                                 start=(k == 0), stop=(k == KT - 1))
            nc.scalar.copy(out=v_t[mt], in_=ps)
        ps_c.release()
        xpool.release()

        # ---- stage D attention + stage E (proj) as PE filler ----
        yT = [y_pool.tile([128, T], bf, tag=f"yT{h}", name=f"yT{h}")
              for h in range(HPC)]

        maskpool = tc.alloc_tile_pool(name="maskp", bufs=1)
        mask_sb = maskpool.tile([128, 2, 2 * TC], bf, name="mask_sb")
        nc.sync.dma_start(out=mask_sb, in_=masks3)
        pp_pool = tc.alloc_tile_pool(name="pp", bufs=1)
        acc_pool = tc.alloc_tile_pool(name="accp", bufs=1)
        sm_pool = tc.alloc_tile_pool(name="small", bufs=1)
        o_pool = tc.alloc_tile_pool(name="ostage", bufs=1)
        ps_s = tc.alloc_tile_pool(name="ps_s", bufs=2, space="PSUM")
        ps_y = tc.alloc_tile_pool(name="ps_y", bufs=2, space="PSUM")
        ps_d = tc.alloc_tile_pool(name="ps_d", bufs=1, space="PSUM")
        ps_o = tc.alloc_tile_pool(name="ps_o", bufs=1, space="PSUM")

        e_jobs = deque()
        e_count = [0]
        e_pool = [ps_o]

        def emit_e_group():
            if not e_jobs:
                return
            mt, n2 = e_jobs.popleft()
            msl = slice(mt * 128, (mt + 1) * 128)
            nsl = slice(n2 * TC, (n2 + 1) * TC)
            pso = e_pool[0].tile([128, TC], f32, tag="o", name="o_ps")
            for hk in range(HPC):
                nc.tensor.matmul(pso, yT[hk][:, msl], wp_t[hk][:, nsl],
                                 start=(hk == 0), stop=(hk == HPC - 1))
            ot = o_pool.tile([128, TC], f32, tag="ot", name="ot", bufs=4)
            # alternate evacuation engine to balance ACT vs DVE load
            if e_count[0] % 2 == 0:
                nc.scalar.copy(out=ot, in_=pso)
            else:
                nc.vector.tensor_copy(out=ot, in_=pso)
            e_count[0] += 1
            nc.sync.dma_start(out=out[msl, nsl], in_=ot)

        for I in range(NT):
            isl = slice(I * TC, (I + 1) * TC)
            for h in range(HPC):
                q_h = rope_sb[h]
                k_h = rope_sb[4 + h]
                bt = _BATCHES[I]
                nb = len(bt)
                acc = acc_pool.tile([128, TC], bf, tag="acc", name="acc",
                                    bufs=2)
                y_ps = ps_y.tile([128, TC], f32, tag="y", name="y_ps")
                for bi, (j0, mp) in enumerate(bt):
                    s_ps = ps_s.tile([128, 2 * TC], f32, tag="s", name="s_ps")
                    for half in range(2):
                        J = j0 + half
                        nc.tensor.matmul(
                            s_ps[:, half * TC:(half + 1) * TC],
                            k_h[:, J * 128:(J + 1) * 128], q_h[:, isl],
                            start=True, stop=True, skip_group_check=True,
                        )
                    pp = pp_pool.tile([128, 2 * TC], bf, tag="pp", name="pp",
                                      bufs=2)
                    nc.scalar.activation(out=pp, in_=s_ps, func=Exp,
                                         scale=SCALE)
                    # PE filler between S and PV so the PV matmuls never
                    # head-of-line block on the exp latency
                    emit_e_group()
                    src = pp
                    if mp is not None:
                        ppm = pp_pool.tile([128, 2 * TC], bf, tag="ppm",
                                           name="ppm", bufs=2)
                        nc.vector.tensor_mul(ppm, pp, mask_sb[:, mp])
                        src = ppm
                    if bi == 0:
                        nc.vector.tensor_copy(out=acc, in_=src[:, 0:TC])
                    else:
                        nc.vector.tensor_add(acc, acc, src[:, 0:TC])
                    nc.vector.tensor_add(acc, acc, src[:, TC:2 * TC])
                    for half in range(2):
                        J = j0 + half
                        nc.tensor.matmul(
                            y_ps, v_t[J][:, h * 128:(h + 1) * 128],
                            src[:, half * TC:(half + 1) * TC],
                            start=(bi == 0 and half == 0),
                            stop=(bi == nb - 1 and half == 1),
                        )
                # denominator (transposed layout) + normalize
                d_ps = ps_d.tile([128, 4], f32, tag="d", name="d_ps")
                for qq in range(4):
                    nc.tensor.matmul(d_ps[:, qq:qq + 1],
                                     acc[:, qq * 128:(qq + 1) * 128],
                                     ones_sb, start=True, stop=True,
                                     skip_group_check=True)
                recip = sm_pool.tile([128, 4], f32, tag="recip",
                                     name="recip", bufs=2)
                nc.vector.reciprocal(out=recip, in_=d_ps)
                # 4 column transposes into one [1, 512] psum row so the
                # gpsimd broadcast reads from partition 0 in one shot;
                # shares ps_o's tag slot (bank) with the E staging psum
                tT_ps = ps_o.tile([1, TC], f32, tag="o", name="tT_ps")
                for qq in range(4):
                    nc.tensor.transpose(tT_ps[:, qq * 128:(qq + 1) * 128],
                                        recip[:, qq:qq + 1], ident_sb)
                recipT = sm_pool.tile([1, TC], f32, tag="recipT",
                                      name="recipT", bufs=2)
                nc.vector.tensor_copy(out=recipT, in_=tT_ps)
                recipB = sm_pool.tile([128, TC], f32, tag="recipB",
                                      name="recipB", bufs=2)
                nc.gpsimd.partition_broadcast(recipB, recipT)
                nc.vector.tensor_mul(yT[h][:, isl], y_ps, recipB)
            for ml in range(4):
                for n2 in range(NT):
                    e_jobs.append((4 * I + ml, n2))
        # tail: attention psum pools are done; hand the remaining E groups
        # a 4-deep psum pool so the group->evacuate->DMA chain pipelines
        ps_o.release()
        ps_d.release()
        ps_y.release()
        ps_tail = tc.alloc_tile_pool(name="ps_tail", bufs=4, space="PSUM")
        e_pool[0] = ps_tail
        while e_jobs:
            emit_e_group()

        for p in (o_pool, sm_pool, acc_pool, pp_pool, maskpool, rope_pool,
                  mpool, y_pool, wppool, v_pool, wvpool, ps_tail, ps_s):
            p.release()
    nc.compile()
    return nc


def _host_prep(x, w_qkv, w_proj, freqs_cis):
    """Build per-core input maps (slicing + layout + dtype prep only)."""
    import ml_dtypes
    BF = ml_dtypes.bfloat16

    x = np.asarray(x, dtype=np.float32)
    w_qkv = np.asarray(w_qkv, dtype=np.float32)
    w_proj = np.asarray(w_proj, dtype=np.float32)
    fc = np.asarray(freqs_cis, dtype=np.float32)

    xTb = [np.ascontiguousarray(x[b].T).astype(BF) for b in range(B)]

    cos = fc[:, :, 0].T  # [64, T]
    sin = fc[:, :, 1].T
    cosP = np.repeat(cos, 2, axis=0).astype(BF)  # [128, T]
    sinP = np.repeat(sin, 2, axis=0).astype(BF)

    rt = np.zeros((HD, HD), dtype=np.float32)
    for d in range(HD // 2):
        rt[2 * d, 2 * d + 1] = 1.0
        rt[2 * d + 1, 2 * d] = -1.0
    rt = rt.astype(BF)

    # masks[p][jj, u]: pair p covers diagonal tiles d = 2p + u//TC
    masks = np.zeros((2, 128, 2 * TC), dtype=np.float32)
    jj = np.arange(128)[:, None]
    for p in range(2):
        for tp in range(2):
            d = 2 * p + tp
            ii = np.arange(TC)[None, :]
            masks[p][:, tp * TC:(tp + 1) * TC] = (ii >= jj + 128 * d)
    masks = masks.astype(BF)

    ones = np.ones((128, 1), dtype=np.float32).astype(BF)
    ident = np.eye(128, dtype=np.float32)

    in_maps = []
    for core in range(8):
        b = core // 4
        g = core % 4
        qc = w_qkv[:, 512 * g: 512 * (g + 1)]
        kc = w_qkv[:, 2048 + 512 * g: 2048 + 512 * (g + 1)]
        vc = w_qkv[:, 4096 + 512 * g: 4096 + 512 * (g + 1)]
        wqk_c = np.concatenate([qc, kc], axis=1).astype(BF)
        wv_c = np.ascontiguousarray(vc).astype(BF)
        wp_c = np.ascontiguousarray(
            w_proj[512 * g: 512 * (g + 1), :]).astype(BF)
        in_maps.append({
            "xT": xTb[b],
            "wqk": wqk_c,
            "wv": wv_c,
            "wp": wp_c,
            "cosP": cosP,
            "sinP": sinP,
            "rt": rt,
            "masks": masks,
            "ones": ones,
            "ident": ident,
        })
    return in_maps


def _get_nc():
    if "nc" not in _CACHE:
        _CACHE["nc"] = _build_nc()
    return _CACHE["nc"]


def kernel(x, w_qkv, w_proj, freqs_cis, attn_mask, _trace=False):
    from concourse.bass_utils import run_bass_kernel_spmd

    in_maps = _host_prep(x, w_qkv, w_proj, freqs_cis)
    nc = _get_nc()
    res = run_bass_kernel_spmd(
        nc, in_maps, core_ids=list(range(8)), trace=_trace,
    )
    outs = [r["out"].astype(np.float64) for r in res.results]
    full = np.stack([
        outs[0] + outs[1] + outs[2] + outs[3],
        outs[4] + outs[5] + outs[6] + outs[7],
    ]).astype(np.float32)
    if _trace:
        kernel._last_results = res
    return full


# revision 23
# speedup vs baseline: 1.7422x; 1.0348x over previous
"""Trainium2 Bass kernel for prefix-LM CausalSelfAttention (v2).

Problem: B=2, T=2048, C=2048, H=16 heads (hd=128), prefix-LM mask
(bidirectional over first half, causal after), RoPE on q/k.

Sharding over 8 cores: data-parallel on batch (2) x tensor-parallel on
heads (4 heads per core). Each core computes a partial output projection
(its heads' contribution); partials are summed on host.

v2 design (vs v1): bf16 data everywhere (validated 6.6e-3 rel err), x
resident in SBUF once (no second DMA pass), RoPE fused per-(m,chunk)
into stage A so DVE work hides under the QKV matmuls, attention exp
batched 2 key-tiles wide on ACT, softmax denominator via DVE-accumulated
pp sum + 4 tiny transposed matmuls + [128,4] reciprocal (replaces the
per-tile ones-matmuls and the 3.3us single-lane [1,512] reciprocal),
and the output projection interleaved into attention as PE filler.

Per-core dataflow:
  A. qkT[m] = W_{q,k}^T @ x^T per 512-chunk; RoPE combine per tile:
     rope = qkT*cos + (R @ qkT)*sin  (R = pair swap w/ sign)
  C. v[t-tile] = x @ Wv (natural layout)
  D. per (I, h): S'[j,i] tiles via k^T-tile x q-chunk, exp on ACT
     (2 tiles per ACTIVATE), pp accumulated on DVE for the denominator,
     PV accumulation into y^T psum; dT = pp_acc^T @ ones via 4 M=1
     matmuls, reciprocal, PE-transpose, gpsimd row broadcast, normalize.
  E. out[mt, n] = sum_hk yT[hk]^T @ Wp[hk], emitted as PE filler between
     attention batches; evacuation alternates ACT/DVE.
"""
import math

import numpy as np

N_HEAD = 16
B = 2
T = 2048
C = 2048
HD = 128
HPC = 4          # heads per core
CL = HPC * HD    # local C = 512
TC = 512         # chunk width (matmul moving free dim / psum bank)
NT = T // TC     # 4 chunks
KT = C // 128    # 16 contraction tiles over C
TT = T // 128    # 16 T tiles
SCALE = 1.0 / math.sqrt(HD)

# Per query-chunk I: batches of two 128-key tiles (j0, j0+1); mp indexes
# the two 1024-wide diagonal mask pairs, None for fully-allowed batches.
_BATCHES = {
    0: [(0, None), (2, None), (4, None), (6, None)],
    1: [(0, None), (2, None), (4, None), (6, None)],
    2: [(0, None), (2, None), (4, None), (6, None), (8, 0), (10, 1)],
    3: [(0, None), (2, None), (4, None), (6, None), (8, None), (10, None),
        (12, 0), (14, 1)],
}

_CACHE = {}


def _build_nc():
    from collections import deque

    import concourse.tile as tile
    import concourse.mybir as mybir
    from concourse import bacc

    f32 = mybir.dt.float32
    f32r = mybir.dt.float32r
    bf = mybir.dt.bfloat16
    Exp = mybir.ActivationFunctionType.Exp

    nc = bacc.Bacc(None, target_bir_lowering=False)

    xT = nc.dram_tensor("xT", [C, T], bf, kind="ExternalInput")
    wqk = nc.dram_tensor("wqk", [C, 2 * CL], bf, kind="ExternalInput")
    wv = nc.dram_tensor("wv", [C, CL], bf, kind="ExternalInput")
    wp = nc.dram_tensor("wp", [CL, C], bf, kind="ExternalInput")
    cosP = nc.dram_tensor("cosP", [HD, T], bf, kind="ExternalInput")
    sinP = nc.dram_tensor("sinP", [HD, T], bf, kind="ExternalInput")
    rt = nc.dram_tensor("rt", [HD, HD], bf, kind="ExternalInput")
    masks = nc.dram_tensor("masks", [2, 128, 2 * TC], bf, kind="ExternalInput")
    ones = nc.dram_tensor("ones", [128, 1], bf, kind="ExternalInput")
    ident = nc.dram_tensor("ident", [128, 128], f32, kind="ExternalInput")
    out = nc.dram_tensor("out", [T, C], f32, kind="ExternalOutput")

    xT3 = xT.rearrange("(kt p) t -> p kt t", p=128)
    wqk3 = wqk.rearrange("(kt p) m -> p kt m", p=128)
    wv3 = wv.rearrange("(kt p) m -> p kt m", p=128)
    wp3 = wp.rearrange("(hk p) m -> p hk m", p=128)
    masks3 = masks.rearrange("g p u -> p g u")

    with tile.TileContext(nc) as tc:
        # Left stack: mpool/rope (long-lived), then x (..stage C), then the
        # per-phase pools on top in LIFO order. Right stack: v/wp/yT which
        # outlive x. PSUM pools form their own stack.
        mpool = tc.alloc_tile_pool(name="misc", bufs=1)
        rope_pool = tc.alloc_tile_pool(name="rope", bufs=1)   # ..attention
        xpool = tc.alloc_tile_pool(name="x_sb", bufs=1)       # ..stage C

        rt_sb = mpool.tile([HD, HD], bf)
        ones_sb = mpool.tile([128, 1], bf)
        ident_sb = mpool.tile([128, 128], f32)
        warm_sb = mpool.tile([128, 128], bf)
        dume_sb = mpool.tile([128, 2], bf)

        # HAM warmup: PE matmuls on memset data while input DMAs stream,
        # so stage A starts at K=8/8. Also pre-trigger the exp table load
        # and the gpsimd library load (first partition_broadcast otherwise
        # costs ~9us mid-attention).
        nc.vector.memset(warm_sb, 0.0)
        nc.scalar.activation(out=dume_sb, in_=warm_sb[:, 0:2], func=Exp)
        dumg_sb = mpool.tile([128, 4], bf)
        nc.gpsimd.partition_broadcast(dumg_sb, warm_sb[0:1, 0:4])
        ps_w = tc.alloc_tile_pool(name="ps_warm", bufs=1, space="PSUM")
        for _ in range(44):
            pw = ps_w.tile([128, 128], f32, tag="pw", name="pw")
            nc.tensor.matmul(pw, warm_sb, warm_sb, start=True, stop=True)
        ps_w.release()

        # Long-lived pools on the right stack so their DMAs land in fresh
        # address space (no WAR on released stage-A pools) and can be
        # emitted early in the sync queue.
        wvpool = tc.alloc_tile_pool(name="wv_sb", bufs=1, side="right")

        # ---- input DMAs (sync-queue order = arrival order) ----
        wpool = tc.alloc_tile_pool(name="wqk_sb", bufs=1)     # ..stage A
        tpool = tc.alloc_tile_pool(name="trig", bufs=1)       # ..stage A
        qk_pool = tc.alloc_tile_pool(name="qk", bufs=1)       # ..stage A

        w_t = []
        x_t = {}
        for k in range(KT):
            wt = wpool.tile([128, 2 * CL], bf, tag=f"w{k}", name=f"w{k}")
            nc.sync.dma_start(out=wt, in_=wqk3[:, k])
            w_t.append(wt)
            xt = xpool.tile([128, TC], bf, tag=f"x{k}_0", name=f"x{k}_0")
            nc.sync.dma_start(out=xt, in_=xT3[:, k, 0:TC])
            x_t[(k, 0)] = xt
        cos_sb = tpool.tile([HD, T], bf)
        sin_sb = tpool.tile([HD, T], bf)
        nc.sync.dma_start(out=rt_sb, in_=rt[:, :])
        nc.sync.dma_start(out=ident_sb, in_=ident[:, :])
        nc.sync.dma_start(out=ones_sb, in_=ones[:, :])
        nc.sync.dma_start(out=cos_sb, in_=cosP[:, :])
        nc.sync.dma_start(out=sin_sb, in_=sinP[:, :])
        wv_t = []
        wp_t = []
        for n in range(1, NT):
            for k in range(KT):
                xt = xpool.tile([128, TC], bf, tag=f"x{k}_{n}",
                                name=f"x{k}_{n}")
                nc.sync.dma_start(out=xt, in_=xT3[:, k, n * TC:(n + 1) * TC])
                x_t[(k, n)] = xt
            if n == 1:
                for k in range(KT):
                    wt = wvpool.tile([128, CL], bf, tag=f"wv{k}",
                                     name=f"wv{k}")
                    nc.sync.dma_start(out=wt, in_=wv3[:, k])
                    wv_t.append(wt)

        # ---- stage A: qkT + fused RoPE ----
        ps_a8 = tc.alloc_tile_pool(name="ps_a8", bufs=1, space="PSUM")
        rtmp = tc.alloc_tile_pool(name="rope_tmp", bufs=1)

        rope_sb = [rope_pool.tile([128, T], bf, tag=f"ro{m}", name=f"ro{m}")
                   for m in range(8)]

        def emit_rope(m, nsl, qkt):
            # R @ qk on PE (pair swap w/ sign), combine on DVE in bf16.
            psr = ps_r.tile([128, TC], f32, tag="ps_r", name="ps_r")
            nc.tensor.matmul(psr, rt_sb, qkt, start=True, stop=True)
            t1 = rtmp.tile([128, TC], bf, tag="t1", name="t1", bufs=2)
            nc.vector.tensor_mul(t1, psr, sin_sb[:, nsl])
            t2 = rtmp.tile([128, TC], bf, tag="t2", name="t2", bufs=2)
            nc.vector.tensor_mul(t2, qkt, cos_sb[:, nsl])
            nc.vector.tensor_add(rope_sb[m][:, nsl], t1, t2)

        # n=0 runs k-outer with 8 simultaneously-open psum groups so the
        # first matmul issues as soon as the first (w[k], x[k,0]) DMA pair
        # lands, instead of waiting for the full 6MB stage-A working set.
        pend = deque()
        nsl0 = slice(0, TC)
        ps8 = [ps_a8.tile([128, TC], f32, tag=f"a8_{m}", name=f"a8_{m}")
               for m in range(8)]
        for k in range(KT):
            for m in range(8):
                nc.tensor.matmul(ps8[m], w_t[k][:, m * 128:(m + 1) * 128],
                                 x_t[(k, 0)], start=(k == 0),
                                 stop=(k == KT - 1), skip_group_check=True)
        for m in range(8):
            qkt = qk_pool.tile([128, TC], bf, tag="qkt", name="qkt", bufs=12)
            nc.scalar.copy(out=qkt, in_=ps8[m])
            pend.append((m, nsl0, qkt))
        ps_a8.release()
        ps_a = tc.alloc_tile_pool(name="ps_a", bufs=4, space="PSUM")
        ps_r = tc.alloc_tile_pool(name="ps_rot", bufs=2, space="PSUM")
        for n in range(1, NT):
            nsl = slice(n * TC, (n + 1) * TC)
            for m in range(8):
                ps = ps_a.tile([128, TC], f32, tag="ps_a", name="ps_a")
                for k in range(KT):
                    nc.tensor.matmul(ps, w_t[k][:, m * 128:(m + 1) * 128],
                                     x_t[(k, n)],
                                     start=(k == 0), stop=(k == KT - 1))
                qkt = qk_pool.tile([128, TC], bf, tag="qkt", name="qkt",
                                   bufs=12)
                nc.scalar.copy(out=qkt, in_=ps)
                # rope of an earlier tile: its ACT copy finished during
                # this group's 16 matmuls, so the R-matmul never stalls PE.
                if pend:
                    emit_rope(*pend.popleft())
                pend.append((m, nsl, qkt))
        while pend:
            emit_rope(*pend.popleft())

        rtmp.release()
        ps_r.release()
        ps_a.release()
        qk_pool.release()
        tpool.release()
        wpool.release()

        # ---- stage C: v = x @ Wv ----
        v_pool = tc.alloc_tile_pool(name="v_sb", bufs=1, side="right")
        wppool = tc.alloc_tile_pool(name="wp_sb", bufs=1, side="right")
        y_pool = tc.alloc_tile_pool(name="yT_sb", bufs=1, side="right")
        for hk in range(HPC):
            wt = wppool.tile([128, C], bf, tag=f"wp{hk}", name=f"wp{hk}")
            nc.sync.dma_start(out=wt, in_=wp3[:, hk])
            wp_t.append(wt)
        v_t = [v_pool.tile([128, CL], bf, tag=f"v{mt}", name=f"v{mt}")
               for mt in range(TT)]
        ps_c = tc.alloc_tile_pool(name="ps_c", bufs=4, space="PSUM")
        for mt in range(TT):
            ps = ps_c.tile([128, CL], f32, tag="ps_c", name="ps_c")
            n, off = mt // 4, (mt % 4) * 128
            for k in range(KT):
                nc.tensor.matmul(ps, x_t[(k, n)][:, off:off + 128], wv_t[k],
                                 start=(k == 0), stop=(k == KT - 1))
            nc.scalar.copy(out=v_t[mt], in_=ps)
        ps_c.release()
        xpool.release()

        # ---- stage D attention + stage E (proj) as PE filler ----
        yT = [y_pool.tile([128, T], bf, tag=f"yT{h}", name=f"yT{h}")
              for h in range(HPC)]

        maskpool = tc.alloc_tile_pool(name="maskp", bufs=1)
        mask_sb = maskpool.tile([128, 2, 2 * TC], bf, name="mask_sb")
        nc.sync.dma_start(out=mask_sb, in_=masks3)
        pp_pool = tc.alloc_tile_pool(name="pp", bufs=1)
        acc_pool = tc.alloc_tile_pool(name="accp", bufs=1)
        sm_pool = tc.alloc_tile_pool(name="small", bufs=1)
        o_pool = tc.alloc_tile_pool(name="ostage", bufs=1)
        ps_s = tc.alloc_tile_pool(name="ps_s", bufs=2, space="PSUM")
        ps_y = tc.alloc_tile_pool(name="ps_y", bufs=2, space="PSUM")
        ps_d = tc.alloc_tile_pool(name="ps_d", bufs=1, space="PSUM")
        ps_o = tc.alloc_tile_pool(name="ps_o", bufs=1, space="PSUM")

        e_jobs = deque()
        e_count = [0]
        e_pool = [ps_o]

        def emit_e_group():
            if not e_jobs:
                return
            mt, n2 = e_jobs.popleft()
            msl = slice(mt * 128, (mt + 1) * 128)
            nsl = slice(n2 * TC, (n2 + 1) * TC)
            pso = e_pool[0].tile([128, TC], f32, tag="o", name="o_ps")
            for hk in range(HPC):
                nc.tensor.matmul(pso, yT[hk][:, msl], wp_t[hk][:, nsl],
                                 start=(hk == 0), stop=(hk == HPC - 1))
            ot = o_pool.tile([128, TC], f32, tag="ot", name="ot", bufs=4)
            # alternate evacuation engine to balance ACT vs DVE load
            if e_count[0] % 2 == 0:
                nc.scalar.copy(out=ot, in_=pso)
            else:
                nc.vector.tensor_copy(out=ot, in_=pso)
            e_count[0] += 1
            nc.sync.dma_start(out=out[msl, nsl], in_=ot)

        for I in range(NT):
            isl = slice(I * TC, (I + 1) * TC)
            for h in range(HPC):
                q_h = rope_sb[h]
                k_h = rope_sb[4 + h]
                bt = _BATCHES[I]
                nb = len(bt)
                acc = acc_pool.tile([128, TC], bf, tag="acc", name="acc",
                                    bufs=2)
                y_ps = ps_y.tile([128, TC], f32, tag="y", name="y_ps")
                for bi, (j0, mp) in enumerate(bt):
                    s_ps = ps_s.tile([128, 2 * TC], f32, tag="s", name="s_ps")
                    for half in range(2):
                        J = j0 + half
                        nc.tensor.matmul(
                            s_ps[:, half * TC:(half + 1) * TC],
                            k_h[:, J * 128:(J + 1) * 128], q_h[:, isl],
                            start=True, stop=True, skip_group_check=True,
                        )
                    pp = pp_pool.tile([128, 2 * TC], bf, tag="pp", name="pp",
                                      bufs=2)
                    nc.scalar.activation(out=pp, in_=s_ps, func=Exp,
                                         scale=SCALE)
                    # PE filler between S and PV so the PV matmuls never
                    # head-of-line block on the exp latency
                    emit_e_group()
                    src = pp
                    if mp is not None:
                        ppm = pp_pool.tile([128, 2 * TC], bf, tag="ppm",
                                           name="ppm", bufs=2)
                        nc.vector.tensor_mul(ppm, pp, mask_sb[:, mp])
                        src = ppm
                    if bi == 0:
                        nc.vector.tensor_copy(out=acc, in_=src[:, 0:TC])
                    else:
                        nc.vector.tensor_add(acc, acc, src[:, 0:TC])
                    nc.vector.tensor_add(acc, acc, src[:, TC:2 * TC])
                    for half in range(2):
                        J = j0 + half
                        nc.tensor.matmul(
                            y_ps, v_t[J][:, h * 128:(h + 1) * 128],
                            src[:, half * TC:(half + 1) * TC],
                            start=(bi == 0 and half == 0),
                            stop=(bi == nb - 1 and half == 1),
                        )
                # denominator (transposed layout) + normalize
                d_ps = ps_d.tile([128, 4], f32, tag="d", name="d_ps")
                for qq in range(4):
                    nc.tensor.matmul(d_ps[:, qq:qq + 1],
                                     acc[:, qq * 128:(qq + 1) * 128],
                                     ones_sb, start=True, stop=True,
                                     skip_group_check=True)
                recip = sm_pool.tile([128, 4], f32, tag="recip",
                                     name="recip", bufs=2)
                nc.vector.reciprocal(out=recip, in_=d_ps)
                # 4 column transposes into one [1, 512] psum row so the
                # gpsimd broadcast reads from partition 0 in one shot;
                # shares ps_d's bank (sequential with d_ps by data deps)
                tT_ps = ps_d.tile([1, TC], f32, tag="d", name="tT_ps")
                for qq in range(4):
                    nc.tensor.transpose(tT_ps[:, qq * 128:(qq + 1) * 128],
                                        recip[:, qq:qq + 1], ident_sb)
                recipT = sm_pool.tile([1, TC], f32, tag="recipT",
                                      name="recipT", bufs=2)
                nc.vector.tensor_copy(out=recipT, in_=tT_ps)
                recipB = sm_pool.tile([128, TC], f32, tag="recipB",
                                      name="recipB", bufs=2)
                nc.gpsimd.partition_broadcast(recipB, recipT)
                nc.vector.tensor_mul(yT[h][:, isl], y_ps, recipB)
            for ml in range(4):
                for n2 in range(NT):
                    e_jobs.append((4 * I + ml, n2))
        # tail: attention psum pools are done; hand the remaining E groups
        # a 4-deep psum pool so the group->evacuate->DMA chain pipelines
        ps_o.release()
        ps_d.release()
        ps_y.release()
        ps_tail = tc.alloc_tile_pool(name="ps_tail", bufs=4, space="PSUM")
        e_pool[0] = ps_tail
        while e_jobs:
            emit_e_group()

        for p in (o_pool, sm_pool, acc_pool, pp_pool, maskpool, rope_pool,
                  mpool, y_pool, wppool, v_pool, wvpool, ps_tail, ps_s):
            p.release()
    nc.compile()
    return nc


def _host_prep(x, w_qkv, w_proj, freqs_cis):
    """Build per-core input maps (slicing + layout + dtype prep only)."""
    import ml_dtypes
    BF = ml_dtypes.bfloat16

    x = np.asarray(x, dtype=np.float32)
    w_qkv = np.asarray(w_qkv, dtype=np.float32)
    w_proj = np.asarray(w_proj, dtype=np.float32)
    fc = np.asarray(freqs_cis, dtype=np.float32)

    xTb = [np.ascontiguousarray(x[b].T).astype(BF) for b in range(B)]

    cos = fc[:, :, 0].T  # [64, T]
    sin = fc[:, :, 1].T
    cosP = np.repeat(cos, 2, axis=0).astype(BF)  # [128, T]
    sinP = np.repeat(sin, 2, axis=0).astype(BF)

    rt = np.zeros((HD, HD), dtype=np.float32)
    for d in range(HD // 2):
        rt[2 * d, 2 * d + 1] = 1.0
        rt[2 * d + 1, 2 * d] = -1.0
    rt = rt.astype(BF)

    # masks[p][jj, u]: pair p covers diagonal tiles d = 2p + u//TC
    masks = np.zeros((2, 128, 2 * TC), dtype=np.float32)
    jj = np.arange(128)[:, None]
    for p in range(2):
        for tp in range(2):
            d = 2 * p + tp
            ii = np.arange(TC)[None, :]
            masks[p][:, tp * TC:(tp + 1) * TC] = (ii >= jj + 128 * d)
    masks = masks.astype(BF)

    ones = np.ones((128, 1), dtype=np.float32).astype(BF)
    ident = np.eye(128, dtype=np.float32)

    in_maps = []
    for core in range(8):
        b = core // 4
        g = core % 4
        qc = w_qkv[:, 512 * g: 512 * (g + 1)]
        kc = w_qkv[:, 2048 + 512 * g: 2048 + 512 * (g + 1)]
        vc = w_qkv[:, 4096 + 512 * g: 4096 + 512 * (g + 1)]
        wqk_c = np.concatenate([qc, kc], axis=1).astype(BF)
        wv_c = np.ascontiguousarray(vc).astype(BF)
        wp_c = np.ascontiguousarray(
            w_proj[512 * g: 512 * (g + 1), :]).astype(BF)
        in_maps.append({
            "xT": xTb[b],
            "wqk": wqk_c,
            "wv": wv_c,
            "wp": wp_c,
            "cosP": cosP,
            "sinP": sinP,
            "rt": rt,
            "masks": masks,
            "ones": ones,
            "ident": ident,
        })
    return in_maps


def _get_nc():
    if "nc" not in _CACHE:
        _CACHE["nc"] = _build_nc()
    return _CACHE["nc"]


def kernel(x, w_qkv, w_proj, freqs_cis, attn_mask, _trace=False):
    from concourse.bass_utils import run_bass_kernel_spmd

    in_maps = _host_prep(x, w_qkv, w_proj, freqs_cis)
    nc = _get_nc()
    res = run_bass_kernel_spmd(
        nc, in_maps, core_ids=list(range(8)), trace=_trace,
    )
    outs = [r["out"].astype(np.float64) for r in res.results]
    full = np.stack([
        outs[0] + outs[1] + outs[2] + outs[3],
        outs[4] + outs[5] + outs[6] + outs[7],
    ]).astype(np.float32)
    if _trace:
        kernel._last_results = res
    return full


# revision 24
# speedup vs baseline: 1.7704x; 1.0162x over previous
"""Trainium2 Bass kernel for prefix-LM CausalSelfAttention (v2).

Problem: B=2, T=2048, C=2048, H=16 heads (hd=128), prefix-LM mask
(bidirectional over first half, causal after), RoPE on q/k.

Sharding over 8 cores: data-parallel on batch (2) x tensor-parallel on
heads (4 heads per core). Each core computes a partial output projection
(its heads' contribution); partials are summed on host.

v2 design (vs v1): bf16 data everywhere (validated 6.6e-3 rel err), x
resident in SBUF once (no second DMA pass), RoPE fused per-(m,chunk)
into stage A so DVE work hides under the QKV matmuls, attention exp
batched 2 key-tiles wide on ACT, softmax denominator via DVE-accumulated
pp sum + 4 tiny transposed matmuls + [128,4] reciprocal (replaces the
per-tile ones-matmuls and the 3.3us single-lane [1,512] reciprocal),
and the output projection interleaved into attention as PE filler.

Per-core dataflow:
  A. qkT[m] = W_{q,k}^T @ x^T per 512-chunk; RoPE combine per tile:
     rope = qkT*cos + (R @ qkT)*sin  (R = pair swap w/ sign)
  C. v[t-tile] = x @ Wv (natural layout)
  D. per (I, h): S'[j,i] tiles via k^T-tile x q-chunk, exp on ACT
     (2 tiles per ACTIVATE), pp accumulated on DVE for the denominator,
     PV accumulation into y^T psum; dT = pp_acc^T @ ones via 4 M=1
     matmuls, reciprocal, PE-transpose, gpsimd row broadcast, normalize.
  E. out[mt, n] = sum_hk yT[hk]^T @ Wp[hk], emitted as PE filler between
     attention batches; evacuation alternates ACT/DVE.
"""
import math

import numpy as np

N_HEAD = 16
B = 2
T = 2048
C = 2048
HD = 128
HPC = 4          # heads per core
CL = HPC * HD    # local C = 512
TC = 512         # chunk width (matmul moving free dim / psum bank)
NT = T // TC     # 4 chunks
KT = C // 128    # 16 contraction tiles over C
TT = T // 128    # 16 T tiles
SCALE = 1.0 / math.sqrt(HD)

# Per query-chunk I: batches of two 128-key tiles (j0, j0+1); mp indexes
# the two 1024-wide diagonal mask pairs, None for fully-allowed batches.
_BATCHES = {
    0: [(0, None), (2, None), (4, None), (6, None)],
    1: [(0, None), (2, None), (4, None), (6, None)],
    2: [(0, None), (2, None), (4, None), (6, None), (8, 0), (10, 1)],
    3: [(0, None), (2, None), (4, None), (6, None), (8, None), (10, None),
        (12, 0), (14, 1)],
}

_CACHE = {}


def _build_nc():
    from collections import deque

    import concourse.tile as tile
    import concourse.mybir as mybir
    from concourse import bacc

    f32 = mybir.dt.float32
    f32r = mybir.dt.float32r
    bf = mybir.dt.bfloat16
    Exp = mybir.ActivationFunctionType.Exp

    nc = bacc.Bacc(None, target_bir_lowering=False)

    xT = nc.dram_tensor("xT", [C, T], bf, kind="ExternalInput")
    wqk = nc.dram_tensor("wqk", [C, 2 * CL], bf, kind="ExternalInput")
    wv = nc.dram_tensor("wv", [C, CL], bf, kind="ExternalInput")
    wp = nc.dram_tensor("wp", [CL, C], bf, kind="ExternalInput")
    cosP = nc.dram_tensor("cosP", [HD, T], bf, kind="ExternalInput")
    sinP = nc.dram_tensor("sinP", [HD, T], bf, kind="ExternalInput")
    rt = nc.dram_tensor("rt", [HD, HD], bf, kind="ExternalInput")
    masks = nc.dram_tensor("masks", [2, 128, 2 * TC], bf, kind="ExternalInput")
    ones = nc.dram_tensor("ones", [128, 1], bf, kind="ExternalInput")
    ident = nc.dram_tensor("ident", [128, 128], bf, kind="ExternalInput")
    out = nc.dram_tensor("out", [T, C], f32, kind="ExternalOutput")

    xT3 = xT.rearrange("(kt p) t -> p kt t", p=128)
    wqk3 = wqk.rearrange("(kt p) m -> p kt m", p=128)
    wv3 = wv.rearrange("(kt p) m -> p kt m", p=128)
    wp3 = wp.rearrange("(hk p) m -> p hk m", p=128)
    masks3 = masks.rearrange("g p u -> p g u")

    with tile.TileContext(nc) as tc:
        # Left stack: mpool/rope (long-lived), then x (..stage C), then the
        # per-phase pools on top in LIFO order. Right stack: v/wp/yT which
        # outlive x. PSUM pools form their own stack.
        mpool = tc.alloc_tile_pool(name="misc", bufs=1)
        rope_pool = tc.alloc_tile_pool(name="rope", bufs=1)   # ..attention
        xpool = tc.alloc_tile_pool(name="x_sb", bufs=1)       # ..stage C

        rt_sb = mpool.tile([HD, HD], bf)
        ones_sb = mpool.tile([128, 1], bf)
        ident_bf = mpool.tile([128, 128], bf)
        warm_sb = mpool.tile([128, 128], bf)
        dume_sb = mpool.tile([128, 2], bf)

        # HAM warmup: PE matmuls on memset data while input DMAs stream,
        # so stage A starts at K=8/8. Also pre-trigger the exp table load
        # and the gpsimd library load (first partition_broadcast otherwise
        # costs ~9us mid-attention).
        nc.vector.memset(warm_sb, 0.0)
        nc.scalar.activation(out=dume_sb, in_=warm_sb[:, 0:2], func=Exp)
        dumg_sb = mpool.tile([128, 4], bf)
        nc.gpsimd.partition_broadcast(dumg_sb, warm_sb[0:1, 0:4])
        ps_w = tc.alloc_tile_pool(name="ps_warm", bufs=1, space="PSUM")
        for _ in range(44):
            pw = ps_w.tile([128, 128], f32, tag="pw", name="pw")
            nc.tensor.matmul(pw, warm_sb, warm_sb, start=True, stop=True)
        ps_w.release()

        # Long-lived pools on the right stack so their DMAs land in fresh
        # address space (no WAR on released stage-A pools) and can be
        # emitted early in the sync queue.
        wvpool = tc.alloc_tile_pool(name="wv_sb", bufs=1, side="right")

        # ---- input DMAs (sync-queue order = arrival order) ----
        wpool = tc.alloc_tile_pool(name="wqk_sb", bufs=1)     # ..stage A
        tpool = tc.alloc_tile_pool(name="trig", bufs=1)       # ..stage A
        qk_pool = tc.alloc_tile_pool(name="qk", bufs=1)       # ..stage A

        w_t = []
        x_t = {}
        for k in range(KT):
            wt = wpool.tile([128, 2 * CL], bf, tag=f"w{k}", name=f"w{k}")
            nc.sync.dma_start(out=wt, in_=wqk3[:, k])
            w_t.append(wt)
            xt = xpool.tile([128, TC], bf, tag=f"x{k}_0", name=f"x{k}_0")
            nc.sync.dma_start(out=xt, in_=xT3[:, k, 0:TC])
            x_t[(k, 0)] = xt
        cos_sb = tpool.tile([HD, T], bf)
        sin_sb = tpool.tile([HD, T], bf)
        nc.sync.dma_start(out=rt_sb, in_=rt[:, :])
        nc.sync.dma_start(out=ident_bf, in_=ident[:, :])
        nc.sync.dma_start(out=ones_sb, in_=ones[:, :])
        nc.sync.dma_start(out=cos_sb, in_=cosP[:, :])
        nc.sync.dma_start(out=sin_sb, in_=sinP[:, :])
        wv_t = []
        wp_t = []
        for n in range(1, NT):
            for k in range(KT):
                xt = xpool.tile([128, TC], bf, tag=f"x{k}_{n}",
                                name=f"x{k}_{n}")
                nc.sync.dma_start(out=xt, in_=xT3[:, k, n * TC:(n + 1) * TC])
                x_t[(k, n)] = xt
            if n == 1:
                for k in range(KT):
                    wt = wvpool.tile([128, CL], bf, tag=f"wv{k}",
                                     name=f"wv{k}")
                    nc.sync.dma_start(out=wt, in_=wv3[:, k])
                    wv_t.append(wt)

        # ---- stage A: qkT + fused RoPE ----
        ps_a8 = tc.alloc_tile_pool(name="ps_a8", bufs=1, space="PSUM")
        rtmp = tc.alloc_tile_pool(name="rope_tmp", bufs=1)

        rope_sb = [rope_pool.tile([128, T], bf, tag=f"ro{m}", name=f"ro{m}")
                   for m in range(8)]

        def emit_rope(m, nsl, qkt):
            # R @ qk on PE (pair swap w/ sign), combine on DVE in bf16.
            psr = ps_r.tile([128, TC], f32, tag="ps_r", name="ps_r")
            nc.tensor.matmul(psr, rt_sb, qkt, start=True, stop=True)
            t1 = rtmp.tile([128, TC], bf, tag="t1", name="t1", bufs=2)
            nc.vector.tensor_mul(t1, psr, sin_sb[:, nsl])
            t2 = rtmp.tile([128, TC], bf, tag="t2", name="t2", bufs=2)
            nc.vector.tensor_mul(t2, qkt, cos_sb[:, nsl])
            nc.vector.tensor_add(rope_sb[m][:, nsl], t1, t2)

        # n=0 runs k-outer with 8 simultaneously-open psum groups so the
        # first matmul issues as soon as the first (w[k], x[k,0]) DMA pair
        # lands, instead of waiting for the full 6MB stage-A working set.
        pend = deque()
        nsl0 = slice(0, TC)
        ps8 = [ps_a8.tile([128, TC], f32, tag=f"a8_{m}", name=f"a8_{m}")
               for m in range(8)]
        for k in range(KT):
            for m in range(8):
                nc.tensor.matmul(ps8[m], w_t[k][:, m * 128:(m + 1) * 128],
                                 x_t[(k, 0)], start=(k == 0),
                                 stop=(k == KT - 1), skip_group_check=True)
        for m in range(8):
            qkt = qk_pool.tile([128, TC], bf, tag="qkt", name="qkt", bufs=12)
            nc.scalar.copy(out=qkt, in_=ps8[m])
            pend.append((m, nsl0, qkt))
        ps_a8.release()
        ps_a = tc.alloc_tile_pool(name="ps_a", bufs=4, space="PSUM")
        ps_r = tc.alloc_tile_pool(name="ps_rot", bufs=2, space="PSUM")
        for n in range(1, NT):
            nsl = slice(n * TC, (n + 1) * TC)
            for m in range(8):
                ps = ps_a.tile([128, TC], f32, tag="ps_a", name="ps_a")
                for k in range(KT):
                    nc.tensor.matmul(ps, w_t[k][:, m * 128:(m + 1) * 128],
                                     x_t[(k, n)],
                                     start=(k == 0), stop=(k == KT - 1))
                qkt = qk_pool.tile([128, TC], bf, tag="qkt", name="qkt",
                                   bufs=12)
                nc.scalar.copy(out=qkt, in_=ps)
                # rope of an earlier tile: its ACT copy finished during
                # this group's 16 matmuls, so the R-matmul never stalls PE.
                if pend:
                    emit_rope(*pend.popleft())
                pend.append((m, nsl, qkt))
        while pend:
            emit_rope(*pend.popleft())

        rtmp.release()
        ps_r.release()
        ps_a.release()
        qk_pool.release()
        tpool.release()
        wpool.release()

        # ---- stage C: v = x @ Wv ----
        v_pool = tc.alloc_tile_pool(name="v_sb", bufs=1, side="right")
        wppool = tc.alloc_tile_pool(name="wp_sb", bufs=1, side="right")
        y_pool = tc.alloc_tile_pool(name="yT_sb", bufs=1, side="right")
        for hk in range(HPC):
            wt = wppool.tile([128, C], bf, tag=f"wp{hk}", name=f"wp{hk}")
            nc.sync.dma_start(out=wt, in_=wp3[:, hk])
            wp_t.append(wt)
        v_t = [v_pool.tile([128, CL], bf, tag=f"v{mt}", name=f"v{mt}")
               for mt in range(TT)]
        ps_c = tc.alloc_tile_pool(name="ps_c", bufs=4, space="PSUM")
        for mt in range(TT):
            ps = ps_c.tile([128, CL], f32, tag="ps_c", name="ps_c")
            n, off = mt // 4, (mt % 4) * 128
            for k in range(KT):
                nc.tensor.matmul(ps, x_t[(k, n)][:, off:off + 128], wv_t[k],
                                 start=(k == 0), stop=(k == KT - 1))
            nc.scalar.copy(out=v_t[mt], in_=ps)
        ps_c.release()
        xpool.release()

        # ---- stage D attention + stage E (proj) as PE filler ----
        yT = [y_pool.tile([128, T], bf, tag=f"yT{h}", name=f"yT{h}")
              for h in range(HPC)]

        maskpool = tc.alloc_tile_pool(name="maskp", bufs=1)
        mask_sb = maskpool.tile([128, 2, 2 * TC], bf, name="mask_sb")
        nc.sync.dma_start(out=mask_sb, in_=masks3)
        pp_pool = tc.alloc_tile_pool(name="pp", bufs=1)
        acc_pool = tc.alloc_tile_pool(name="accp", bufs=1)
        sm_pool = tc.alloc_tile_pool(name="small", bufs=1)
        o_pool = tc.alloc_tile_pool(name="ostage", bufs=1)
        ps_s = tc.alloc_tile_pool(name="ps_s", bufs=2, space="PSUM")
        ps_y = tc.alloc_tile_pool(name="ps_y", bufs=2, space="PSUM")
        ps_d = tc.alloc_tile_pool(name="ps_d", bufs=1, space="PSUM")
        ps_o = tc.alloc_tile_pool(name="ps_o", bufs=1, space="PSUM")

        e_jobs = deque()
        e_count = [0]
        e_pool = [ps_o]

        def emit_e_group():
            if not e_jobs:
                return
            mt, n2 = e_jobs.popleft()
            msl = slice(mt * 128, (mt + 1) * 128)
            nsl = slice(n2 * TC, (n2 + 1) * TC)
            pso = e_pool[0].tile([128, TC], f32, tag="o", name="o_ps")
            for hk in range(HPC):
                nc.tensor.matmul(pso, yT[hk][:, msl], wp_t[hk][:, nsl],
                                 start=(hk == 0), stop=(hk == HPC - 1))
            ot = o_pool.tile([128, TC], f32, tag="ot", name="ot", bufs=8)
            # alternate evacuation engine to balance ACT vs DVE load
            if e_count[0] % 2 == 0:
                nc.scalar.copy(out=ot, in_=pso)
            else:
                nc.vector.tensor_copy(out=ot, in_=pso)
            e_count[0] += 1
            nc.sync.dma_start(out=out[msl, nsl], in_=ot)

        for I in range(NT):
            isl = slice(I * TC, (I + 1) * TC)
            for h in range(HPC):
                q_h = rope_sb[h]
                k_h = rope_sb[4 + h]
                bt = _BATCHES[I]
                nb = len(bt)
                acc = acc_pool.tile([128, TC], bf, tag="acc", name="acc",
                                    bufs=2)
                y_ps = ps_y.tile([128, TC], f32, tag="y", name="y_ps")
                for bi, (j0, mp) in enumerate(bt):
                    s_ps = ps_s.tile([128, 2 * TC], f32, tag="s", name="s_ps")
                    for half in range(2):
                        J = j0 + half
                        nc.tensor.matmul(
                            s_ps[:, half * TC:(half + 1) * TC],
                            k_h[:, J * 128:(J + 1) * 128], q_h[:, isl],
                            start=True, stop=True, skip_group_check=True,
                        )
                    pp = pp_pool.tile([128, 2 * TC], bf, tag="pp", name="pp",
                                      bufs=2)
                    nc.scalar.activation(out=pp, in_=s_ps, func=Exp,
                                         scale=SCALE)
                    # PE filler between S and PV so the PV matmuls never
                    # head-of-line block on the exp latency
                    emit_e_group()
                    src = pp
                    if mp is not None:
                        ppm = pp_pool.tile([128, 2 * TC], bf, tag="ppm",
                                           name="ppm", bufs=2)
                        nc.vector.tensor_mul(ppm, pp, mask_sb[:, mp])
                        src = ppm
                    if bi == 0:
                        nc.vector.tensor_copy(out=acc, in_=src[:, 0:TC])
                    else:
                        nc.vector.tensor_add(acc, acc, src[:, 0:TC])
                    nc.vector.tensor_add(acc, acc, src[:, TC:2 * TC])
                    for half in range(2):
                        J = j0 + half
                        nc.tensor.matmul(
                            y_ps, v_t[J][:, h * 128:(h + 1) * 128],
                            src[:, half * TC:(half + 1) * TC],
                            start=(bi == 0 and half == 0),
                            stop=(bi == nb - 1 and half == 1),
                        )
                # denominator (transposed layout) + normalize
                d_ps = ps_d.tile([128, 4], f32, tag="d", name="d_ps")
                for qq in range(4):
                    nc.tensor.matmul(d_ps[:, qq:qq + 1],
                                     acc[:, qq * 128:(qq + 1) * 128],
                                     ones_sb, start=True, stop=True,
                                     skip_group_check=True)
                recip = sm_pool.tile([128, 4], bf, tag="recip",
                                     name="recip", bufs=2)
                with nc.allow_low_precision(
                        reason="1/d in bf16: 0.4% on softmax scale, "
                               "validated 6e-3 rel err end to end"):
                    nc.vector.reciprocal(out=recip, in_=d_ps)
                # 4 column transposes into one [1, 512] psum row so the
                # gpsimd broadcast reads from partition 0 in one shot;
                # shares ps_d's bank (sequential with d_ps by data deps).
                # bf16 keeps the transposes single-pass (fp32 is LOW_HIGH).
                tT_ps = ps_d.tile([1, TC], bf, tag="d", name="tT_ps")
                for qq in range(4):
                    nc.tensor.transpose(tT_ps[:, qq * 128:(qq + 1) * 128],
                                        recip[:, qq:qq + 1], ident_bf)
                recipT = sm_pool.tile([1, TC], bf, tag="recipT",
                                      name="recipT", bufs=2)
                nc.vector.tensor_copy(out=recipT, in_=tT_ps)
                recipB = sm_pool.tile([128, TC], bf, tag="recipB",
                                      name="recipB", bufs=2)
                nc.gpsimd.partition_broadcast(recipB, recipT)
                nc.vector.tensor_mul(yT[h][:, isl], y_ps, recipB)
            for ml in range(4):
                for n2 in range(NT):
                    e_jobs.append((4 * I + ml, n2))
        # tail: attention psum pools are done; hand the remaining E groups
        # a 4-deep psum pool so the group->evacuate->DMA chain pipelines
        ps_o.release()
        ps_d.release()
        ps_y.release()
        ps_s.release()
        ps_tail = tc.alloc_tile_pool(name="ps_tail", bufs=6, space="PSUM")
        e_pool[0] = ps_tail
        while e_jobs:
            emit_e_group()

        for p in (o_pool, sm_pool, acc_pool, pp_pool, maskpool, rope_pool,
                  mpool, y_pool, wppool, v_pool, wvpool, ps_tail):
            p.release()
    nc.compile()
    return nc


def _host_prep(x, w_qkv, w_proj, freqs_cis):
    """Build per-core input maps (slicing + layout + dtype prep only)."""
    import ml_dtypes
    BF = ml_dtypes.bfloat16

    x = np.asarray(x, dtype=np.float32)
    w_qkv = np.asarray(w_qkv, dtype=np.float32)
    w_proj = np.asarray(w_proj, dtype=np.float32)
    fc = np.asarray(freqs_cis, dtype=np.float32)

    xTb = [np.ascontiguousarray(x[b].T).astype(BF) for b in range(B)]

    cos = fc[:, :, 0].T  # [64, T]
    sin = fc[:, :, 1].T
    cosP = np.repeat(cos, 2, axis=0).astype(BF)  # [128, T]
    sinP = np.repeat(sin, 2, axis=0).astype(BF)

    rt = np.zeros((HD, HD), dtype=np.float32)
    for d in range(HD // 2):
        rt[2 * d, 2 * d + 1] = 1.0
        rt[2 * d + 1, 2 * d] = -1.0
    rt = rt.astype(BF)

    # masks[p][jj, u]: pair p covers diagonal tiles d = 2p + u//TC
    masks = np.zeros((2, 128, 2 * TC), dtype=np.float32)
    jj = np.arange(128)[:, None]
    for p in range(2):
        for tp in range(2):
            d = 2 * p + tp
            ii = np.arange(TC)[None, :]
            masks[p][:, tp * TC:(tp + 1) * TC] = (ii >= jj + 128 * d)
    masks = masks.astype(BF)

    ones = np.ones((128, 1), dtype=np.float32).astype(BF)
    ident = np.eye(128, dtype=np.float32).astype(BF)

    in_maps = []
    for core in range(8):
        b = core // 4
        g = core % 4
        qc = w_qkv[:, 512 * g: 512 * (g + 1)]
        kc = w_qkv[:, 2048 + 512 * g: 2048 + 512 * (g + 1)]
        vc = w_qkv[:, 4096 + 512 * g: 4096 + 512 * (g + 1)]
        wqk_c = np.concatenate([qc, kc], axis=1).astype(BF)
        wv_c = np.ascontiguousarray(vc).astype(BF)
        wp_c = np.ascontiguousarray(
            w_proj[512 * g: 512 * (g + 1), :]).astype(BF)
        in_maps.append({
            "xT": xTb[b],
            "wqk": wqk_c,
            "wv": wv_c,
            "wp": wp_c,
            "cosP": cosP,
            "sinP": sinP,
            "rt": rt,
            "masks": masks,
            "ones": ones,
            "ident": ident,
        })
    return in_maps


def _get_nc():
    if "nc" not in _CACHE:
        _CACHE["nc"] = _build_nc()
    return _CACHE["nc"]


def kernel(x, w_qkv, w_proj, freqs_cis, attn_mask, _trace=False):
    from concourse.bass_utils import run_bass_kernel_spmd

    in_maps = _host_prep(x, w_qkv, w_proj, freqs_cis)
    nc = _get_nc()
    res = run_bass_kernel_spmd(
        nc, in_maps, core_ids=list(range(8)), trace=_trace,
    )
    outs = [r["out"].astype(np.float64) for r in res.results]
    full = np.stack([
        outs[0] + outs[1] + outs[2] + outs[3],
        outs[4] + outs[5] + outs[6] + outs[7],
    ]).astype(np.float32)
    if _trace:
        kernel._last_results = res
    return full


# revision 26
# speedup vs baseline: 1.8013x; 1.0175x over previous
"""Trainium2 Bass kernel for prefix-LM CausalSelfAttention (v2).

Problem: B=2, T=2048, C=2048, H=16 heads (hd=128), prefix-LM mask
(bidirectional over first half, causal after), RoPE on q/k.

Sharding over 8 cores: data-parallel on batch (2) x tensor-parallel on
heads (4 heads per core). Each core computes a partial output projection
(its heads' contribution); partials are summed on host.

v2 design (vs v1): bf16 data everywhere (validated 6.6e-3 rel err), x
resident in SBUF once (no second DMA pass), RoPE fused per-(m,chunk)
into stage A so DVE work hides under the QKV matmuls, attention exp
batched 2 key-tiles wide on ACT, softmax denominator via DVE-accumulated
pp sum + 4 tiny transposed matmuls + [128,4] reciprocal (replaces the
per-tile ones-matmuls and the 3.3us single-lane [1,512] reciprocal),
and the output projection interleaved into attention as PE filler.

Per-core dataflow:
  A. qkT[m] = W_{q,k}^T @ x^T per 512-chunk; RoPE combine per tile:
     rope = qkT*cos + (R @ qkT)*sin  (R = pair swap w/ sign)
  C. v[t-tile] = x @ Wv (natural layout)
  D. per (I, h): S'[j,i] tiles via k^T-tile x q-chunk, exp on ACT
     (2 tiles per ACTIVATE), pp accumulated on DVE for the denominator,
     PV accumulation into y^T psum; dT = pp_acc^T @ ones via 4 M=1
     matmuls, reciprocal, PE-transpose, gpsimd row broadcast, normalize.
  E. out[mt, n] = sum_hk yT[hk]^T @ Wp[hk], emitted as PE filler between
     attention batches; evacuation alternates ACT/DVE.
"""
import math

import numpy as np

N_HEAD = 16
B = 2
T = 2048
C = 2048
HD = 128
HPC = 4          # heads per core
CL = HPC * HD    # local C = 512
TC = 512         # chunk width (matmul moving free dim / psum bank)
NT = T // TC     # 4 chunks
KT = C // 128    # 16 contraction tiles over C
TT = T // 128    # 16 T tiles
SCALE = 1.0 / math.sqrt(HD)

# Per query-chunk I: batches of two 128-key tiles (j0, j0+1); mp indexes
# the two 1024-wide diagonal mask pairs, None for fully-allowed batches.
_BATCHES = {
    0: [(0, None), (2, None), (4, None), (6, None)],
    1: [(0, None), (2, None), (4, None), (6, None)],
    2: [(0, None), (2, None), (4, None), (6, None), (8, 0), (10, 1)],
    3: [(0, None), (2, None), (4, None), (6, None), (8, None), (10, None),
        (12, 0), (14, 1)],
}

_CACHE = {}


def _build_nc():
    from collections import deque

    import concourse.tile as tile
    import concourse.mybir as mybir
    from concourse import bacc

    f32 = mybir.dt.float32
    f32r = mybir.dt.float32r
    bf = mybir.dt.bfloat16
    Exp = mybir.ActivationFunctionType.Exp

    nc = bacc.Bacc(None, target_bir_lowering=False)

    xT = nc.dram_tensor("xT", [C, T], bf, kind="ExternalInput")
    wqk = nc.dram_tensor("wqk", [C, 2 * CL], bf, kind="ExternalInput")
    wv = nc.dram_tensor("wv", [C, CL], bf, kind="ExternalInput")
    wp = nc.dram_tensor("wp", [CL, C], bf, kind="ExternalInput")
    cosP = nc.dram_tensor("cosP", [HD, T], bf, kind="ExternalInput")
    sinP = nc.dram_tensor("sinP", [HD, T], bf, kind="ExternalInput")
    rt = nc.dram_tensor("rt", [HD, HD], bf, kind="ExternalInput")
    masks = nc.dram_tensor("masks", [2, 128, 2 * TC], bf, kind="ExternalInput")
    ones = nc.dram_tensor("ones", [128, 1], bf, kind="ExternalInput")
    ident = nc.dram_tensor("ident", [128, 128], bf, kind="ExternalInput")
    out = nc.dram_tensor("out", [T, C], f32, kind="ExternalOutput")

    xT3 = xT.rearrange("(kt p) t -> p kt t", p=128)
    wqk3 = wqk.rearrange("(kt p) m -> p kt m", p=128)
    wv3 = wv.rearrange("(kt p) m -> p kt m", p=128)
    wp3 = wp.rearrange("(hk p) m -> p hk m", p=128)
    masks3 = masks.rearrange("g p u -> p g u")

    with tile.TileContext(nc) as tc:
        # Left stack: mpool/rope (long-lived), then x (..stage C), then the
        # per-phase pools on top in LIFO order. Right stack: v/wp/yT which
        # outlive x. PSUM pools form their own stack.
        mpool = tc.alloc_tile_pool(name="misc", bufs=1)
        rope_pool = tc.alloc_tile_pool(name="rope", bufs=1)   # ..attention
        xpool = tc.alloc_tile_pool(name="x_sb", bufs=1)       # ..stage C

        rt_sb = mpool.tile([HD, HD], bf)
        ones_sb = mpool.tile([128, 1], bf)
        ident_bf = mpool.tile([128, 128], bf)
        warm_sb = mpool.tile([128, 128], bf)
        dume_sb = mpool.tile([128, 2], bf)

        # HAM warmup: PE matmuls on memset data while input DMAs stream,
        # so stage A starts at K=8/8. Also pre-trigger the exp table load
        # and the gpsimd library load (first partition_broadcast otherwise
        # costs ~9us mid-attention).
        nc.vector.memset(warm_sb, 0.0)
        nc.scalar.activation(out=dume_sb, in_=warm_sb[:, 0:2], func=Exp)
        dumg_sb = mpool.tile([128, 4], bf)
        nc.gpsimd.partition_broadcast(dumg_sb, warm_sb[0:1, 0:4])
        ps_w = tc.alloc_tile_pool(name="ps_warm", bufs=1, space="PSUM")
        for _ in range(44):
            pw = ps_w.tile([128, 128], f32, tag="pw", name="pw")
            nc.tensor.matmul(pw, warm_sb, warm_sb, start=True, stop=True)
        ps_w.release()

        # Long-lived pools on the right stack so their DMAs land in fresh
        # address space (no WAR on released stage-A pools) and can be
        # emitted early in the sync queue.
        wvpool = tc.alloc_tile_pool(name="wv_sb", bufs=1, side="right")

        # ---- input DMAs (sync-queue order = arrival order) ----
        wpool = tc.alloc_tile_pool(name="wqk_sb", bufs=1)     # ..stage A
        tpool = tc.alloc_tile_pool(name="trig", bufs=1)       # ..stage A
        qk_pool = tc.alloc_tile_pool(name="qk", bufs=1)       # ..stage A

        w_t = []
        x_t = {}
        for k in range(KT):
            wt = wpool.tile([128, 2 * CL], bf, tag=f"w{k}", name=f"w{k}")
            nc.sync.dma_start(out=wt, in_=wqk3[:, k])
            w_t.append(wt)
            xt = xpool.tile([128, TC], bf, tag=f"x{k}_0", name=f"x{k}_0")
            nc.sync.dma_start(out=xt, in_=xT3[:, k, 0:TC])
            x_t[(k, 0)] = xt
        cos_sb = tpool.tile([HD, T], bf)
        sin_sb = tpool.tile([HD, T], bf)
        nc.sync.dma_start(out=rt_sb, in_=rt[:, :])
        nc.sync.dma_start(out=ident_bf, in_=ident[:, :])
        nc.sync.dma_start(out=ones_sb, in_=ones[:, :])
        nc.sync.dma_start(out=cos_sb, in_=cosP[:, :])
        nc.sync.dma_start(out=sin_sb, in_=sinP[:, :])
        wv_t = []
        wp_t = []
        for n in range(1, NT):
            for k in range(KT):
                xt = xpool.tile([128, TC], bf, tag=f"x{k}_{n}",
                                name=f"x{k}_{n}")
                nc.sync.dma_start(out=xt, in_=xT3[:, k, n * TC:(n + 1) * TC])
                x_t[(k, n)] = xt
            if n == 1:
                for k in range(KT):
                    wt = wvpool.tile([128, CL], bf, tag=f"wv{k}",
                                     name=f"wv{k}")
                    nc.sync.dma_start(out=wt, in_=wv3[:, k])
                    wv_t.append(wt)

        # ---- stage A: qkT + fused RoPE ----
        ps_a8 = tc.alloc_tile_pool(name="ps_a8", bufs=1, space="PSUM")
        rtmp = tc.alloc_tile_pool(name="rope_tmp", bufs=1)

        rope_sb = [rope_pool.tile([128, T], bf, tag=f"ro{m}", name=f"ro{m}")
                   for m in range(8)]

        def emit_rope(m, nsl, qkt):
            # R @ qk on PE (pair swap w/ sign), combine on DVE in bf16.
            psr = ps_r.tile([128, TC], f32, tag="ps_r", name="ps_r")
            nc.tensor.matmul(psr, rt_sb, qkt, start=True, stop=True)
            t1 = rtmp.tile([128, TC], bf, tag="t1", name="t1", bufs=2)
            nc.vector.tensor_mul(t1, psr, sin_sb[:, nsl])
            t2 = rtmp.tile([128, TC], bf, tag="t2", name="t2", bufs=2)
            nc.vector.tensor_mul(t2, qkt, cos_sb[:, nsl])
            nc.vector.tensor_add(rope_sb[m][:, nsl], t1, t2)

        # n=0 runs k-outer with 8 simultaneously-open psum groups so the
        # first matmul issues as soon as the first (w[k], x[k,0]) DMA pair
        # lands, instead of waiting for the full 6MB stage-A working set.
        pend = deque()
        nsl0 = slice(0, TC)
        ps8 = [ps_a8.tile([128, TC], f32, tag=f"a8_{m}", name=f"a8_{m}")
               for m in range(8)]
        for k in range(KT - 1):
            for m in range(8):
                nc.tensor.matmul(ps8[m], w_t[k][:, m * 128:(m + 1) * 128],
                                 x_t[(k, 0)], start=(k == 0), stop=False,
                                 skip_group_check=True)
        for m in range(8):
            # close the groups one at a time so the evacuation copies
            # stagger across the closing matmuls instead of bursting
            nc.tensor.matmul(ps8[m], w_t[KT - 1][:, m * 128:(m + 1) * 128],
                             x_t[(KT - 1, 0)], start=False, stop=True,
                             skip_group_check=True)
            qkt = qk_pool.tile([128, TC], bf, tag="qkt", name="qkt", bufs=12)
            nc.scalar.copy(out=qkt, in_=ps8[m])
            pend.append((m, nsl0, qkt))
        ps_a8.release()
        ps_a = tc.alloc_tile_pool(name="ps_a", bufs=4, space="PSUM")
        ps_r = tc.alloc_tile_pool(name="ps_rot", bufs=2, space="PSUM")
        for n in range(1, NT):
            nsl = slice(n * TC, (n + 1) * TC)
            for m in range(8):
                ps = ps_a.tile([128, TC], f32, tag="ps_a", name="ps_a")
                for k in range(KT):
                    nc.tensor.matmul(ps, w_t[k][:, m * 128:(m + 1) * 128],
                                     x_t[(k, n)],
                                     start=(k == 0), stop=(k == KT - 1))
                qkt = qk_pool.tile([128, TC], bf, tag="qkt", name="qkt",
                                   bufs=12)
                nc.scalar.copy(out=qkt, in_=ps)
                # rope of an earlier tile: its ACT copy finished during
                # this group's 16 matmuls, so the R-matmul never stalls PE.
                if pend:
                    emit_rope(*pend.popleft())
                pend.append((m, nsl, qkt))
        while pend:
            emit_rope(*pend.popleft())

        rtmp.release()
        ps_r.release()
        ps_a.release()
        qk_pool.release()
        tpool.release()
        wpool.release()

        # ---- stage C: v = x @ Wv ----
        v_pool = tc.alloc_tile_pool(name="v_sb", bufs=1, side="right")
        wppool = tc.alloc_tile_pool(name="wp_sb", bufs=1, side="right")
        y_pool = tc.alloc_tile_pool(name="yT_sb", bufs=1, side="right")
        for hk in range(HPC):
            wt = wppool.tile([128, C], bf, tag=f"wp{hk}", name=f"wp{hk}")
            nc.sync.dma_start(out=wt, in_=wp3[:, hk])
            wp_t.append(wt)
        v_t = [v_pool.tile([128, CL], bf, tag=f"v{mt}", name=f"v{mt}")
               for mt in range(TT)]
        ps_c = tc.alloc_tile_pool(name="ps_c", bufs=4, space="PSUM")
        for mt in range(TT):
            ps = ps_c.tile([128, CL], f32, tag="ps_c", name="ps_c")
            n, off = mt // 4, (mt % 4) * 128
            for k in range(KT):
                nc.tensor.matmul(ps, x_t[(k, n)][:, off:off + 128], wv_t[k],
                                 start=(k == 0), stop=(k == KT - 1))
            nc.scalar.copy(out=v_t[mt], in_=ps)
        ps_c.release()
        xpool.release()

        # ---- stage D attention + stage E (proj) as PE filler ----
        yT = [y_pool.tile([128, T], bf, tag=f"yT{h}", name=f"yT{h}")
              for h in range(HPC)]

        maskpool = tc.alloc_tile_pool(name="maskp", bufs=1)
        mask_sb = maskpool.tile([128, 2, 2 * TC], bf, name="mask_sb")
        nc.sync.dma_start(out=mask_sb, in_=masks3)
        pp_pool = tc.alloc_tile_pool(name="pp", bufs=1)
        acc_pool = tc.alloc_tile_pool(name="accp", bufs=1)
        sm_pool = tc.alloc_tile_pool(name="small", bufs=1)
        o_pool = tc.alloc_tile_pool(name="ostage", bufs=1)
        ps_s = tc.alloc_tile_pool(name="ps_s", bufs=2, space="PSUM")
        ps_y = tc.alloc_tile_pool(name="ps_y", bufs=2, space="PSUM")
        ps_d = tc.alloc_tile_pool(name="ps_d", bufs=1, space="PSUM")
        ps_o = tc.alloc_tile_pool(name="ps_o", bufs=1, space="PSUM")

        e_jobs = deque()
        e_count = [0]
        e_pool = [ps_o]

        def emit_e_group():
            if not e_jobs:
                return
            mt, n2 = e_jobs.popleft()
            msl = slice(mt * 128, (mt + 1) * 128)
            nsl = slice(n2 * TC, (n2 + 1) * TC)
            pso = e_pool[0].tile([128, TC], f32, tag="o", name="o_ps")
            for hk in range(HPC):
                nc.tensor.matmul(pso, yT[hk][:, msl], wp_t[hk][:, nsl],
                                 start=(hk == 0), stop=(hk == HPC - 1))
            ot = o_pool.tile([128, TC], f32, tag="ot", name="ot", bufs=8)
            # alternate evacuation engine to balance ACT vs DVE load
            if e_count[0] % 2 == 0:
                nc.scalar.copy(out=ot, in_=pso)
            else:
                nc.vector.tensor_copy(out=ot, in_=pso)
            e_count[0] += 1
            nc.sync.dma_start(out=out[msl, nsl], in_=ot)

        # Attention runs as one flat software pipeline over all
        # (chunk, head) batches: PV matmuls lag the S matmuls by LAG
        # batches, so exp latency never head-of-line blocks the PE —
        # including chunk I=0 (which has no E-filler yet) and at every
        # chunk boundary.
        def emit_norm(I, h, acc, y_ps):
            isl = slice(I * TC, (I + 1) * TC)
            d_ps = ps_d.tile([128, 4], f32, tag="d", name="d_ps")
            for qq in range(4):
                nc.tensor.matmul(d_ps[:, qq:qq + 1],
                                 acc[:, qq * 128:(qq + 1) * 128],
                                 ones_sb, start=True, stop=True,
                                 skip_group_check=True)
            recip = sm_pool.tile([128, 4], bf, tag="recip",
                                 name="recip", bufs=2)
            with nc.allow_low_precision(
                    reason="1/d in bf16: 0.4% on softmax scale, "
                           "validated 6e-3 rel err end to end"):
                nc.vector.reciprocal(out=recip, in_=d_ps)
            # 4 column transposes into one [1, 512] psum row so the
            # gpsimd broadcast reads from partition 0 in one shot;
            # shares ps_d's bank (sequential with d_ps by data deps).
            # bf16 keeps the transposes single-pass (fp32 is LOW_HIGH).
            tT_ps = ps_d.tile([1, TC], bf, tag="d", name="tT_ps")
            for qq in range(4):
                nc.tensor.transpose(tT_ps[:, qq * 128:(qq + 1) * 128],
                                    recip[:, qq:qq + 1], ident_bf)
            recipT = sm_pool.tile([1, TC], bf, tag="recipT",
                                  name="recipT", bufs=2)
            nc.vector.tensor_copy(out=recipT, in_=tT_ps)
            recipB = sm_pool.tile([128, TC], bf, tag="recipB",
                                  name="recipB", bufs=2)
            nc.gpsimd.partition_broadcast(recipB, recipT)
            nc.vector.tensor_mul(yT[h][:, isl], y_ps, recipB)

        steps = []
        for I in range(NT):
            bt = _BATCHES[I]
            for h in range(HPC):
                for bi, (j0, mp) in enumerate(bt):
                    steps.append((I, h, bi, len(bt), j0, mp))

        LAG = 2
        chunk_state = {}   # (I, h) -> [acc, y_ps]
        pv_q = deque()     # (I, h, bi, nb, j0, src)

        def pop_pv():
            I, h, bi, nb, j0, src = pv_q.popleft()
            if bi == 0:
                chunk_state[(I, h)][1] = ps_y.tile([128, TC], f32, tag="y",
                                                   name="y_ps")
            y_ps = chunk_state[(I, h)][1]
            for half in range(2):
                J = j0 + half
                nc.tensor.matmul(
                    y_ps, v_t[J][:, h * 128:(h + 1) * 128],
                    src[:, half * TC:(half + 1) * TC],
                    start=(bi == 0 and half == 0),
                    stop=(bi == nb - 1 and half == 1),
                )
            if bi == nb - 1:
                acc, y_ps = chunk_state.pop((I, h))
                emit_norm(I, h, acc, y_ps)
                if h == HPC - 1:
                    for ml in range(4):
                        for n2 in range(NT):
                            e_jobs.append((4 * I + ml, n2))

        for (I, h, bi, nb, j0, mp) in steps:
            isl = slice(I * TC, (I + 1) * TC)
            q_h = rope_sb[h]
            k_h = rope_sb[4 + h]
            s_ps = ps_s.tile([128, 2 * TC], f32, tag="s", name="s_ps")
            for half in range(2):
                J = j0 + half
                nc.tensor.matmul(
                    s_ps[:, half * TC:(half + 1) * TC],
                    k_h[:, J * 128:(J + 1) * 128], q_h[:, isl],
                    start=True, stop=True, skip_group_check=True,
                )
            pp = pp_pool.tile([128, 2 * TC], bf, tag="pp", name="pp",
                              bufs=LAG + 2)
            nc.scalar.activation(out=pp, in_=s_ps, func=Exp, scale=SCALE)
            emit_e_group()
            src = pp
            if mp is not None:
                ppm = pp_pool.tile([128, 2 * TC], bf, tag="ppm",
                                   name="ppm", bufs=LAG + 2)
                nc.vector.tensor_mul(ppm, pp, mask_sb[:, mp])
                src = ppm
            if bi == 0:
                acc = acc_pool.tile([128, TC], bf, tag="acc", name="acc",
                                    bufs=3)
                chunk_state[(I, h)] = [acc, None]
                nc.vector.tensor_copy(out=acc, in_=src[:, 0:TC])
            else:
                acc = chunk_state[(I, h)][0]
                nc.vector.tensor_add(acc, acc, src[:, 0:TC])
            nc.vector.tensor_add(acc, acc, src[:, TC:2 * TC])
            pv_q.append((I, h, bi, nb, j0, src))
            if len(pv_q) > LAG:
                pop_pv()
        while pv_q:
            pop_pv()
        # tail: attention psum pools are done; hand the remaining E groups
        # a 4-deep psum pool so the group->evacuate->DMA chain pipelines
        ps_o.release()
        ps_d.release()
        ps_y.release()
        ps_s.release()
        ps_tail = tc.alloc_tile_pool(name="ps_tail", bufs=6, space="PSUM")
        e_pool[0] = ps_tail
        while e_jobs:
            emit_e_group()

        for p in (o_pool, sm_pool, acc_pool, pp_pool, maskpool, rope_pool,
                  mpool, y_pool, wppool, v_pool, wvpool, ps_tail):
            p.release()
    nc.compile()
    return nc


def _host_prep(x, w_qkv, w_proj, freqs_cis):
    """Build per-core input maps (slicing + layout + dtype prep only)."""
    import ml_dtypes
    BF = ml_dtypes.bfloat16

    x = np.asarray(x, dtype=np.float32)
    w_qkv = np.asarray(w_qkv, dtype=np.float32)
    w_proj = np.asarray(w_proj, dtype=np.float32)
    fc = np.asarray(freqs_cis, dtype=np.float32)

    xTb = [np.ascontiguousarray(x[b].T).astype(BF) for b in range(B)]

    cos = fc[:, :, 0].T  # [64, T]
    sin = fc[:, :, 1].T
    cosP = np.repeat(cos, 2, axis=0).astype(BF)  # [128, T]
    sinP = np.repeat(sin, 2, axis=0).astype(BF)

    rt = np.zeros((HD, HD), dtype=np.float32)
    for d in range(HD // 2):
        rt[2 * d, 2 * d + 1] = 1.0
        rt[2 * d + 1, 2 * d] = -1.0
    rt = rt.astype(BF)

    # masks[p][jj, u]: pair p covers diagonal tiles d = 2p + u//TC
    masks = np.zeros((2, 128, 2 * TC), dtype=np.float32)
    jj = np.arange(128)[:, None]
    for p in range(2):
        for tp in range(2):
            d = 2 * p + tp
            ii = np.arange(TC)[None, :]
            masks[p][:, tp * TC:(tp + 1) * TC] = (ii >= jj + 128 * d)
    masks = masks.astype(BF)

    ones = np.ones((128, 1), dtype=np.float32).astype(BF)
    ident = np.eye(128, dtype=np.float32).astype(BF)

    in_maps = []
    for core in range(8):
        b = core // 4
        g = core % 4
        qc = w_qkv[:, 512 * g: 512 * (g + 1)]
        kc = w_qkv[:, 2048 + 512 * g: 2048 + 512 * (g + 1)]
        vc = w_qkv[:, 4096 + 512 * g: 4096 + 512 * (g + 1)]
        wqk_c = np.concatenate([qc, kc], axis=1).astype(BF)
        wv_c = np.ascontiguousarray(vc).astype(BF)
        wp_c = np.ascontiguousarray(
            w_proj[512 * g: 512 * (g + 1), :]).astype(BF)
        in_maps.append({
            "xT": xTb[b],
            "wqk": wqk_c,
            "wv": wv_c,
            "wp": wp_c,
            "cosP": cosP,
            "sinP": sinP,
            "rt": rt,
            "masks": masks,
            "ones": ones,
            "ident": ident,
        })
    return in_maps


def _get_nc():
    if "nc" not in _CACHE:
        _CACHE["nc"] = _build_nc()
    return _CACHE["nc"]


def kernel(x, w_qkv, w_proj, freqs_cis, attn_mask, _trace=False):
    from concourse.bass_utils import run_bass_kernel_spmd

    in_maps = _host_prep(x, w_qkv, w_proj, freqs_cis)
    nc = _get_nc()
    res = run_bass_kernel_spmd(
        nc, in_maps, core_ids=list(range(8)), trace=_trace,
    )
    outs = [r["out"].astype(np.float64) for r in res.results]
    full = np.stack([
        outs[0] + outs[1] + outs[2] + outs[3],
        outs[4] + outs[5] + outs[6] + outs[7],
    ]).astype(np.float32)
    if _trace:
        kernel._last_results = res
    return full


# revision 28
# speedup vs baseline: 1.8259x; 1.0137x over previous
"""Trainium2 Bass kernel for prefix-LM CausalSelfAttention (v2).

Problem: B=2, T=2048, C=2048, H=16 heads (hd=128), prefix-LM mask
(bidirectional over first half, causal after), RoPE on q/k.

Sharding over 8 cores: data-parallel on batch (2) x tensor-parallel on
heads (4 heads per core). Each core computes a partial output projection
(its heads' contribution); partials are summed on host.

v2 design (vs v1): bf16 data everywhere (validated 6.6e-3 rel err), x
resident in SBUF once (no second DMA pass), RoPE fused per-(m,chunk)
into stage A so DVE work hides under the QKV matmuls, attention exp
batched 2 key-tiles wide on ACT, softmax denominator via DVE-accumulated
pp sum + 4 tiny transposed matmuls + [128,4] reciprocal (replaces the
per-tile ones-matmuls and the 3.3us single-lane [1,512] reciprocal),
and the output projection interleaved into attention as PE filler.

Per-core dataflow:
  A. qkT[m] = W_{q,k}^T @ x^T per 512-chunk; RoPE combine per tile:
     rope = qkT*cos + (R @ qkT)*sin  (R = pair swap w/ sign)
  C. v[t-tile] = x @ Wv (natural layout)
  D. per (I, h): S'[j,i] tiles via k^T-tile x q-chunk, exp on ACT
     (2 tiles per ACTIVATE), pp accumulated on DVE for the denominator,
     PV accumulation into y^T psum; dT = pp_acc^T @ ones via 4 M=1
     matmuls, reciprocal, PE-transpose, gpsimd row broadcast, normalize.
  E. out[mt, n] = sum_hk yT[hk]^T @ Wp[hk], emitted as PE filler between
     attention batches; evacuation alternates ACT/DVE.
"""
import math

import numpy as np

N_HEAD = 16
B = 2
T = 2048
C = 2048
HD = 128
HPC = 4          # heads per core
CL = HPC * HD    # local C = 512
TC = 512         # chunk width (matmul moving free dim / psum bank)
NT = T // TC     # 4 chunks
KT = C // 128    # 16 contraction tiles over C
TT = T // 128    # 16 T tiles
SCALE = 1.0 / math.sqrt(HD)

# Per query-chunk I: batches of two 128-key tiles (j0, j0+1); mp indexes
# the two 1024-wide diagonal mask pairs, None for fully-allowed batches.
_BATCHES = {
    0: [(0, None), (2, None), (4, None), (6, None)],
    1: [(0, None), (2, None), (4, None), (6, None)],
    2: [(0, None), (2, None), (4, None), (6, None), (8, 0), (10, 1)],
    3: [(0, None), (2, None), (4, None), (6, None), (8, None), (10, None),
        (12, 0), (14, 1)],
}

_CACHE = {}


def _build_nc():
    from collections import deque

    import concourse.tile as tile
    import concourse.mybir as mybir
    from concourse import bacc

    f32 = mybir.dt.float32
    f32r = mybir.dt.float32r
    bf = mybir.dt.bfloat16
    Exp = mybir.ActivationFunctionType.Exp

    nc = bacc.Bacc(None, target_bir_lowering=False)

    xT = nc.dram_tensor("xT", [C, T], bf, kind="ExternalInput")
    wqk = nc.dram_tensor("wqk", [C, 2 * CL], bf, kind="ExternalInput")
    wv = nc.dram_tensor("wv", [C, CL], bf, kind="ExternalInput")
    wp = nc.dram_tensor("wp", [CL, C], bf, kind="ExternalInput")
    cosP = nc.dram_tensor("cosP", [HD, T], bf, kind="ExternalInput")
    sinP = nc.dram_tensor("sinP", [HD, T], bf, kind="ExternalInput")
    rt = nc.dram_tensor("rt", [HD, HD], bf, kind="ExternalInput")
    masks = nc.dram_tensor("masks", [2, 128, 2 * TC], bf, kind="ExternalInput")
    ones = nc.dram_tensor("ones", [128, 1], bf, kind="ExternalInput")
    ident = nc.dram_tensor("ident", [128, 128], bf, kind="ExternalInput")
    out = nc.dram_tensor("out", [T, C], f32, kind="ExternalOutput")

    xT3 = xT.rearrange("(kt p) t -> p kt t", p=128)
    wqk3 = wqk.rearrange("(kt p) m -> p kt m", p=128)
    wv3 = wv.rearrange("(kt p) m -> p kt m", p=128)
    wp3 = wp.rearrange("(hk p) m -> p hk m", p=128)
    masks3 = masks.rearrange("g p u -> p g u")

    with tile.TileContext(nc) as tc:
        # Left stack: mpool/rope (long-lived), then x (..stage C), then the
        # per-phase pools on top in LIFO order. Right stack: v/wp/yT which
        # outlive x. PSUM pools form their own stack.
        mpool = tc.alloc_tile_pool(name="misc", bufs=1)
        rope_pool = tc.alloc_tile_pool(name="rope", bufs=1)   # ..attention
        xpool = tc.alloc_tile_pool(name="x_sb", bufs=1)       # ..stage C

        rt_sb = mpool.tile([HD, HD], bf)
        ones_sb = mpool.tile([128, 1], bf)
        ident_bf = mpool.tile([128, 128], bf)
        warm_sb = mpool.tile([128, 128], bf)
        dume_sb = mpool.tile([128, 2], bf)

        # HAM warmup: PE matmuls on memset data while input DMAs stream,
        # so stage A starts at K=8/8. Also pre-trigger the exp table load
        # and the gpsimd library load (first partition_broadcast otherwise
        # costs ~9us mid-attention).
        nc.vector.memset(warm_sb, 0.0)
        nc.scalar.activation(out=dume_sb, in_=warm_sb[:, 0:2], func=Exp)
        dumg_sb = mpool.tile([128, 4], bf)
        nc.gpsimd.partition_broadcast(dumg_sb, warm_sb[0:1, 0:4])
        ps_w = tc.alloc_tile_pool(name="ps_warm", bufs=1, space="PSUM")
        for _ in range(44):
            pw = ps_w.tile([128, 128], f32, tag="pw", name="pw")
            nc.tensor.matmul(pw, warm_sb, warm_sb, start=True, stop=True)
        ps_w.release()

        # Long-lived pools on the right stack so their DMAs land in fresh
        # address space (no WAR on released stage-A pools) and can be
        # emitted early in the sync queue.
        wvpool = tc.alloc_tile_pool(name="wv_sb", bufs=1, side="right")

        # ---- input DMAs (sync-queue order = arrival order) ----
        wpool = tc.alloc_tile_pool(name="wqk_sb", bufs=1)     # ..stage A
        tpool = tc.alloc_tile_pool(name="trig", bufs=1)       # ..stage A
        qk_pool = tc.alloc_tile_pool(name="qk", bufs=1)       # ..stage A

        w_t = []
        x_t = {}
        for k in range(KT):
            wt = wpool.tile([128, 2 * CL], bf, tag=f"w{k}", name=f"w{k}")
            nc.sync.dma_start(out=wt, in_=wqk3[:, k])
            w_t.append(wt)
            xt = xpool.tile([128, TC], bf, tag=f"x{k}_0", name=f"x{k}_0")
            nc.sync.dma_start(out=xt, in_=xT3[:, k, 0:TC])
            x_t[(k, 0)] = xt
        cos_sb = tpool.tile([HD, T], bf)
        sin_sb = tpool.tile([HD, T], bf)
        nc.sync.dma_start(out=rt_sb, in_=rt[:, :])
        nc.sync.dma_start(out=ident_bf, in_=ident[:, :])
        nc.sync.dma_start(out=ones_sb, in_=ones[:, :])
        nc.sync.dma_start(out=cos_sb, in_=cosP[:, :])
        nc.sync.dma_start(out=sin_sb, in_=sinP[:, :])
        wv_t = []
        wp_t = []
        for n in range(1, NT):
            for k in range(KT):
                xt = xpool.tile([128, TC], bf, tag=f"x{k}_{n}",
                                name=f"x{k}_{n}")
                nc.sync.dma_start(out=xt, in_=xT3[:, k, n * TC:(n + 1) * TC])
                x_t[(k, n)] = xt
            if n == 1:
                for k in range(KT):
                    wt = wvpool.tile([128, CL], bf, tag=f"wv{k}",
                                     name=f"wv{k}")
                    nc.sync.dma_start(out=wt, in_=wv3[:, k])
                    wv_t.append(wt)

        # ---- stage A: qkT + fused RoPE ----
        rtmp = tc.alloc_tile_pool(name="rope_tmp", bufs=1)

        rope_sb = [rope_pool.tile([128, T], bf, tag=f"ro{m}", name=f"ro{m}")
                   for m in range(8)]

        def emit_rope(m, nsl, qkt):
            # R @ qk on PE (pair swap w/ sign), combine on DVE in bf16.
            psr = ps_r.tile([128, TC], f32, tag="ps_r", name="ps_r")
            nc.tensor.matmul(psr, rt_sb, qkt, start=True, stop=True)
            t1 = rtmp.tile([128, TC], bf, tag="t1", name="t1", bufs=2)
            nc.vector.tensor_mul(t1, psr, sin_sb[:, nsl])
            t2 = rtmp.tile([128, TC], bf, tag="t2", name="t2", bufs=2)
            nc.vector.tensor_mul(t2, qkt, cos_sb[:, nsl])
            nc.vector.tensor_add(rope_sb[m][:, nsl], t1, t2)

        # n=0 runs k-outer in two waves of 4 simultaneously-open psum
        # groups, so the first matmul issues as soon as the first
        # (w[k], x[k,0]) DMA pair lands instead of waiting for the full
        # 6MB stage-A working set. Copies alternate ACT/DVE so the group
        # closes drain across two engines.
        pend = deque()
        nsl0 = slice(0, TC)
        ps_a = tc.alloc_tile_pool(name="ps_a", bufs=4, space="PSUM")
        ps_r = tc.alloc_tile_pool(name="ps_rot", bufs=2, space="PSUM")
        for wave in range(2):
            psw = [ps_a.tile([128, TC], f32, tag="ps_a", name="ps_a")
                   for _ in range(4)]
            for k in range(KT - 1):
                for mi in range(4):
                    m = 4 * wave + mi
                    nc.tensor.matmul(psw[mi],
                                     w_t[k][:, m * 128:(m + 1) * 128],
                                     x_t[(k, 0)], start=(k == 0), stop=False,
                                     skip_group_check=True)
            for mi in range(4):
                # close one group at a time so evacuations stagger
                m = 4 * wave + mi
                nc.tensor.matmul(psw[mi],
                                 w_t[KT - 1][:, m * 128:(m + 1) * 128],
                                 x_t[(KT - 1, 0)], start=False, stop=True,
                                 skip_group_check=True)
                qkt = qk_pool.tile([128, TC], bf, tag="qkt", name="qkt",
                                   bufs=12)
                if mi % 2 == 0:
                    nc.scalar.copy(out=qkt, in_=psw[mi])
                else:
                    nc.vector.tensor_copy(out=qkt, in_=psw[mi])
                pend.append((m, nsl0, qkt))
        for n in range(1, NT):
            nsl = slice(n * TC, (n + 1) * TC)
            for m in range(8):
                ps = ps_a.tile([128, TC], f32, tag="ps_a", name="ps_a")
                for k in range(KT):
                    nc.tensor.matmul(ps, w_t[k][:, m * 128:(m + 1) * 128],
                                     x_t[(k, n)],
                                     start=(k == 0), stop=(k == KT - 1))
                qkt = qk_pool.tile([128, TC], bf, tag="qkt", name="qkt",
                                   bufs=12)
                nc.scalar.copy(out=qkt, in_=ps)
                # rope of an earlier tile: its ACT copy finished during
                # this group's 16 matmuls, so the R-matmul never stalls PE.
                if pend:
                    emit_rope(*pend.popleft())
                pend.append((m, nsl, qkt))
        while pend:
            emit_rope(*pend.popleft())

        rtmp.release()
        ps_r.release()
        ps_a.release()
        qk_pool.release()
        tpool.release()
        wpool.release()

        # ---- stage C: v = x @ Wv ----
        v_pool = tc.alloc_tile_pool(name="v_sb", bufs=1, side="right")
        wppool = tc.alloc_tile_pool(name="wp_sb", bufs=1, side="right")
        y_pool = tc.alloc_tile_pool(name="yT_sb", bufs=1, side="right")
        for hk in range(HPC):
            wt = wppool.tile([128, C], bf, tag=f"wp{hk}", name=f"wp{hk}")
            nc.sync.dma_start(out=wt, in_=wp3[:, hk])
            wp_t.append(wt)
        v_t = [v_pool.tile([128, CL], bf, tag=f"v{mt}", name=f"v{mt}")
               for mt in range(TT)]
        ps_c = tc.alloc_tile_pool(name="ps_c", bufs=4, space="PSUM")
        for mt in range(TT):
            ps = ps_c.tile([128, CL], f32, tag="ps_c", name="ps_c")
            n, off = mt // 4, (mt % 4) * 128
            for k in range(KT):
                nc.tensor.matmul(ps, x_t[(k, n)][:, off:off + 128], wv_t[k],
                                 start=(k == 0), stop=(k == KT - 1))
            nc.scalar.copy(out=v_t[mt], in_=ps)
        ps_c.release()
        xpool.release()

        # ---- stage D attention + stage E (proj) as PE filler ----
        yT = [y_pool.tile([128, T], bf, tag=f"yT{h}", name=f"yT{h}")
              for h in range(HPC)]

        maskpool = tc.alloc_tile_pool(name="maskp", bufs=1)
        mask_sb = maskpool.tile([128, 2, 2 * TC], bf, name="mask_sb")
        nc.sync.dma_start(out=mask_sb, in_=masks3)
        pp_pool = tc.alloc_tile_pool(name="pp", bufs=1)
        acc_pool = tc.alloc_tile_pool(name="accp", bufs=1)
        sm_pool = tc.alloc_tile_pool(name="small", bufs=1)
        o_pool = tc.alloc_tile_pool(name="ostage", bufs=1)
        ps_s = tc.alloc_tile_pool(name="ps_s", bufs=2, space="PSUM")
        ps_y = tc.alloc_tile_pool(name="ps_y", bufs=2, space="PSUM")
        ps_d = tc.alloc_tile_pool(name="ps_d", bufs=1, space="PSUM")
        ps_o = tc.alloc_tile_pool(name="ps_o", bufs=1, space="PSUM")

        e_jobs = deque()
        e_count = [0]
        e_pool = [ps_o]

        def emit_e_group():
            if not e_jobs:
                return
            mt, n2 = e_jobs.popleft()
            msl = slice(mt * 128, (mt + 1) * 128)
            nsl = slice(n2 * TC, (n2 + 1) * TC)
            pso = e_pool[0].tile([128, TC], f32, tag="o", name="o_ps")
            for hk in range(HPC):
                nc.tensor.matmul(pso, yT[hk][:, msl], wp_t[hk][:, nsl],
                                 start=(hk == 0), stop=(hk == HPC - 1))
            ot = o_pool.tile([128, TC], f32, tag="ot", name="ot", bufs=8)
            # alternate evacuation engine to balance ACT vs DVE load
            if e_count[0] % 2 == 0:
                nc.scalar.copy(out=ot, in_=pso)
            else:
                nc.vector.tensor_copy(out=ot, in_=pso)
            e_count[0] += 1
            nc.sync.dma_start(out=out[msl, nsl], in_=ot)

        # Attention runs as one flat software pipeline over all
        # (chunk, head) batches: PV matmuls lag the S matmuls by LAG
        # batches, so exp latency never head-of-line blocks the PE —
        # including chunk I=0 (which has no E-filler yet) and at every
        # chunk boundary.
        def emit_norm(I, h, acc, y_ps):
            isl = slice(I * TC, (I + 1) * TC)
            d_ps = ps_d.tile([128, 4], f32, tag="d", name="d_ps")
            for qq in range(4):
                nc.tensor.matmul(d_ps[:, qq:qq + 1],
                                 acc[:, qq * 128:(qq + 1) * 128],
                                 ones_sb, start=True, stop=True,
                                 skip_group_check=True)
            recip = sm_pool.tile([128, 4], bf, tag="recip",
                                 name="recip", bufs=2)
            with nc.allow_low_precision(
                    reason="1/d in bf16: 0.4% on softmax scale, "
                           "validated 6e-3 rel err end to end"):
                nc.vector.reciprocal(out=recip, in_=d_ps)
            # 4 column transposes into one [1, 512] psum row so the
            # gpsimd broadcast reads from partition 0 in one shot;
            # shares ps_d's bank (sequential with d_ps by data deps).
            # bf16 keeps the transposes single-pass (fp32 is LOW_HIGH).
            tT_ps = ps_d.tile([1, TC], bf, tag="d", name="tT_ps")
            for qq in range(4):
                nc.tensor.transpose(tT_ps[:, qq * 128:(qq + 1) * 128],
                                    recip[:, qq:qq + 1], ident_bf)
            recipT = sm_pool.tile([1, TC], bf, tag="recipT",
                                  name="recipT", bufs=2)
            nc.vector.tensor_copy(out=recipT, in_=tT_ps)
            recipB = sm_pool.tile([128, TC], bf, tag="recipB",
                                  name="recipB", bufs=2)
            nc.gpsimd.partition_broadcast(recipB, recipT)
            nc.vector.tensor_mul(yT[h][:, isl], y_ps, recipB)

        steps = []
        for I in range(NT):
            bt = _BATCHES[I]
            for h in range(HPC):
                for bi, (j0, mp) in enumerate(bt):
                    steps.append((I, h, bi, len(bt), j0, mp))

        LAG = 2
        chunk_state = {}   # (I, h) -> [acc, y_ps]
        pv_q = deque()     # (I, h, bi, nb, j0, src)

        def pop_pv():
            I, h, bi, nb, j0, src = pv_q.popleft()
            if bi == 0:
                chunk_state[(I, h)][1] = ps_y.tile([128, TC], f32, tag="y",
                                                   name="y_ps")
            y_ps = chunk_state[(I, h)][1]
            for half in range(2):
                J = j0 + half
                nc.tensor.matmul(
                    y_ps, v_t[J][:, h * 128:(h + 1) * 128],
                    src[:, half * TC:(half + 1) * TC],
                    start=(bi == 0 and half == 0),
                    stop=(bi == nb - 1 and half == 1),
                )
            if bi == nb - 1:
                acc, y_ps = chunk_state.pop((I, h))
                emit_norm(I, h, acc, y_ps)
                if h == HPC - 1:
                    for ml in range(4):
                        for n2 in range(NT):
                            e_jobs.append((4 * I + ml, n2))

        for (I, h, bi, nb, j0, mp) in steps:
            isl = slice(I * TC, (I + 1) * TC)
            q_h = rope_sb[h]
            k_h = rope_sb[4 + h]
            s_ps = ps_s.tile([128, 2 * TC], f32, tag="s", name="s_ps")
            for half in range(2):
                J = j0 + half
                nc.tensor.matmul(
                    s_ps[:, half * TC:(half + 1) * TC],
                    k_h[:, J * 128:(J + 1) * 128], q_h[:, isl],
                    start=True, stop=True, skip_group_check=True,
                )
            pp = pp_pool.tile([128, 2 * TC], bf, tag="pp", name="pp",
                              bufs=LAG + 2)
            nc.scalar.activation(out=pp, in_=s_ps, func=Exp, scale=SCALE)
            emit_e_group()
            src = pp
            if mp is not None:
                ppm = pp_pool.tile([128, 2 * TC], bf, tag="ppm",
                                   name="ppm", bufs=LAG + 2)
                nc.vector.tensor_mul(ppm, pp, mask_sb[:, mp])
                src = ppm
            if bi == 0:
                acc = acc_pool.tile([128, TC], bf, tag="acc", name="acc",
                                    bufs=3)
                chunk_state[(I, h)] = [acc, None]
                nc.vector.tensor_copy(out=acc, in_=src[:, 0:TC])
            else:
                acc = chunk_state[(I, h)][0]
                nc.vector.tensor_add(acc, acc, src[:, 0:TC])
            nc.vector.tensor_add(acc, acc, src[:, TC:2 * TC])
            pv_q.append((I, h, bi, nb, j0, src))
            if len(pv_q) > LAG:
                pop_pv()
        while pv_q:
            pop_pv()
        # tail: attention psum pools are done; hand the remaining E groups
        # a 4-deep psum pool so the group->evacuate->DMA chain pipelines
        ps_o.release()
        ps_d.release()
        ps_y.release()
        ps_s.release()
        ps_tail = tc.alloc_tile_pool(name="ps_tail", bufs=6, space="PSUM")
        e_pool[0] = ps_tail
        while e_jobs:
            emit_e_group()

        for p in (o_pool, sm_pool, acc_pool, pp_pool, maskpool, rope_pool,
                  mpool, y_pool, wppool, v_pool, wvpool, ps_tail):
            p.release()
    nc.compile()
    return nc


def _host_prep(x, w_qkv, w_proj, freqs_cis):
    """Build per-core input maps (slicing + layout + dtype prep only)."""
    import ml_dtypes
    BF = ml_dtypes.bfloat16

    x = np.asarray(x, dtype=np.float32)
    w_qkv = np.asarray(w_qkv, dtype=np.float32)
    w_proj = np.asarray(w_proj, dtype=np.float32)
    fc = np.asarray(freqs_cis, dtype=np.float32)

    xTb = [np.ascontiguousarray(x[b].T).astype(BF) for b in range(B)]

    cos = fc[:, :, 0].T  # [64, T]
    sin = fc[:, :, 1].T
    cosP = np.repeat(cos, 2, axis=0).astype(BF)  # [128, T]
    sinP = np.repeat(sin, 2, axis=0).astype(BF)

    rt = np.zeros((HD, HD), dtype=np.float32)
    for d in range(HD // 2):
        rt[2 * d, 2 * d + 1] = 1.0
        rt[2 * d + 1, 2 * d] = -1.0
    rt = rt.astype(BF)

    # masks[p][jj, u]: pair p covers diagonal tiles d = 2p + u//TC
    masks = np.zeros((2, 128, 2 * TC), dtype=np.float32)
    jj = np.arange(128)[:, None]
    for p in range(2):
        for tp in range(2):
            d = 2 * p + tp
            ii = np.arange(TC)[None, :]
            masks[p][:, tp * TC:(tp + 1) * TC] = (ii >= jj + 128 * d)
    masks = masks.astype(BF)

    ones = np.ones((128, 1), dtype=np.float32).astype(BF)
    ident = np.eye(128, dtype=np.float32).astype(BF)

    in_maps = []
    for core in range(8):
        b = core // 4
        g = core % 4
        qc = w_qkv[:, 512 * g: 512 * (g + 1)]
        kc = w_qkv[:, 2048 + 512 * g: 2048 + 512 * (g + 1)]
        vc = w_qkv[:, 4096 + 512 * g: 4096 + 512 * (g + 1)]
        wqk_c = np.concatenate([qc, kc], axis=1).astype(BF)
        wv_c = np.ascontiguousarray(vc).astype(BF)
        wp_c = np.ascontiguousarray(
            w_proj[512 * g: 512 * (g + 1), :]).astype(BF)
        in_maps.append({
            "xT": xTb[b],
            "wqk": wqk_c,
            "wv": wv_c,
            "wp": wp_c,
            "cosP": cosP,
            "sinP": sinP,
            "rt": rt,
            "masks": masks,
            "ones": ones,
            "ident": ident,
        })
    return in_maps


def _get_nc():
    if "nc" not in _CACHE:
        _CACHE["nc"] = _build_nc()
    return _CACHE["nc"]


def kernel(x, w_qkv, w_proj, freqs_cis, attn_mask, _trace=False):
    from concourse.bass_utils import run_bass_kernel_spmd

    in_maps = _host_prep(x, w_qkv, w_proj, freqs_cis)
    nc = _get_nc()
    res = run_bass_kernel_spmd(
        nc, in_maps, core_ids=list(range(8)), trace=_trace,
    )
    outs = [r["out"].astype(np.float64) for r in res.results]
    full = np.stack([
        outs[0] + outs[1] + outs[2] + outs[3],
        outs[4] + outs[5] + outs[6] + outs[7],
    ]).astype(np.float32)
    if _trace:
        kernel._last_results = res
    return full


# revision 30
# speedup vs baseline: 1.8600x; 1.0187x over previous
"""Trainium2 Bass kernel for prefix-LM CausalSelfAttention (v2).

Problem: B=2, T=2048, C=2048, H=16 heads (hd=128), prefix-LM mask
(bidirectional over first half, causal after), RoPE on q/k.

Sharding over 8 cores: data-parallel on batch (2) x tensor-parallel on
heads (4 heads per core). Each core computes a partial output projection
(its heads' contribution); partials are summed on host.

v2 design (vs v1): bf16 data everywhere (validated 6.6e-3 rel err), x
resident in SBUF once (no second DMA pass), RoPE fused per-(m,chunk)
into stage A so DVE work hides under the QKV matmuls, attention exp
batched 2 key-tiles wide on ACT, softmax denominator via DVE-accumulated
pp sum + 4 tiny transposed matmuls + [128,4] reciprocal (replaces the
per-tile ones-matmuls and the 3.3us single-lane [1,512] reciprocal),
and the output projection interleaved into attention as PE filler.

Per-core dataflow:
  A. qkT[m] = W_{q,k}^T @ x^T per 512-chunk; RoPE combine per tile:
     rope = qkT*cos + (R @ qkT)*sin  (R = pair swap w/ sign)
  C. v[t-tile] = x @ Wv (natural layout)
  D. per (I, h): S'[j,i] tiles via k^T-tile x q-chunk, exp on ACT
     (2 tiles per ACTIVATE), pp accumulated on DVE for the denominator,
     PV accumulation into y^T psum; dT = pp_acc^T @ ones via 4 M=1
     matmuls, reciprocal, PE-transpose, gpsimd row broadcast, normalize.
  E. out[mt, n] = sum_hk yT[hk]^T @ Wp[hk], emitted as PE filler between
     attention batches; evacuation alternates ACT/DVE.
"""
import math

import numpy as np

N_HEAD = 16
B = 2
T = 2048
C = 2048
HD = 128
HPC = 4          # heads per core
CL = HPC * HD    # local C = 512
TC = 512         # chunk width (matmul moving free dim / psum bank)
NT = T // TC     # 4 chunks
KT = C // 128    # 16 contraction tiles over C
TT = T // 128    # 16 T tiles
SCALE = 1.0 / math.sqrt(HD)

# Per query-chunk I: batches of two 128-key tiles (j0, j0+1); mp indexes
# the two 1024-wide diagonal mask pairs, None for fully-allowed batches.
_BATCHES = {
    0: [(0, None), (2, None), (4, None), (6, None)],
    1: [(0, None), (2, None), (4, None), (6, None)],
    2: [(0, None), (2, None), (4, None), (6, None), (8, 0), (10, 1)],
    3: [(0, None), (2, None), (4, None), (6, None), (8, None), (10, None),
        (12, 0), (14, 1)],
}

_CACHE = {}


def _build_nc():
    from collections import deque

    import concourse.tile as tile
    import concourse.mybir as mybir
    from concourse import bacc

    f32 = mybir.dt.float32
    f32r = mybir.dt.float32r
    bf = mybir.dt.bfloat16
    Exp = mybir.ActivationFunctionType.Exp

    nc = bacc.Bacc(None, target_bir_lowering=False)

    xT = nc.dram_tensor("xT", [C, T], bf, kind="ExternalInput")
    wqk = nc.dram_tensor("wqk", [C, 2 * CL], bf, kind="ExternalInput")
    wv = nc.dram_tensor("wv", [C, CL], bf, kind="ExternalInput")
    wp = nc.dram_tensor("wp", [CL, C], bf, kind="ExternalInput")
    cosP = nc.dram_tensor("cosP", [HD, T], bf, kind="ExternalInput")
    sinP = nc.dram_tensor("sinP", [HD, T], bf, kind="ExternalInput")
    rt = nc.dram_tensor("rt", [HD, HD], bf, kind="ExternalInput")
    masks = nc.dram_tensor("masks", [2, 128, 2 * TC], bf, kind="ExternalInput")
    ones = nc.dram_tensor("ones", [128, 1], bf, kind="ExternalInput")
    ident = nc.dram_tensor("ident", [128, 128], bf, kind="ExternalInput")
    out = nc.dram_tensor("out", [T, C], f32, kind="ExternalOutput")

    xT3 = xT.rearrange("(kt p) t -> p kt t", p=128)
    wqk3 = wqk.rearrange("(kt p) m -> p kt m", p=128)
    wv3 = wv.rearrange("(kt p) m -> p kt m", p=128)
    wp3 = wp.rearrange("(hk p) m -> p hk m", p=128)
    masks3 = masks.rearrange("g p u -> p g u")

    with tile.TileContext(nc) as tc:
        # Left stack: mpool/rope (long-lived), then x (..stage C), then the
        # per-phase pools on top in LIFO order. Right stack: v/wp/yT which
        # outlive x. PSUM pools form their own stack.
        mpool = tc.alloc_tile_pool(name="misc", bufs=1)
        rope_pool = tc.alloc_tile_pool(name="rope", bufs=1)   # ..attention
        xpool = tc.alloc_tile_pool(name="x_sb", bufs=1)       # ..stage C

        rt_sb = mpool.tile([HD, HD], bf)
        ones_sb = mpool.tile([128, 1], bf)
        ident_bf = mpool.tile([128, 128], bf)
        warm_sb = mpool.tile([128, 128], bf)
        dume_sb = mpool.tile([128, 2], bf)

        # HAM warmup: PE matmuls on memset data while input DMAs stream,
        # so stage A starts at K=8/8. Also pre-trigger the exp table load
        # and the gpsimd library load (first partition_broadcast otherwise
        # costs ~9us mid-attention).
        nc.vector.memset(warm_sb, 0.0)
        nc.scalar.activation(out=dume_sb, in_=warm_sb[:, 0:2], func=Exp)
        dumg_sb = mpool.tile([128, 4], bf)
        nc.gpsimd.partition_broadcast(dumg_sb, warm_sb[0:1, 0:4])
        ps_w = tc.alloc_tile_pool(name="ps_warm", bufs=1, space="PSUM")
        for _ in range(44):
            pw = ps_w.tile([128, 128], f32, tag="pw", name="pw")
            nc.tensor.matmul(pw, warm_sb, warm_sb, start=True, stop=True)
        ps_w.release()

        # Long-lived pools on the right stack so their DMAs land in fresh
        # address space (no WAR on released stage-A pools) and can be
        # emitted early in the sync queue.
        wvpool = tc.alloc_tile_pool(name="wv_sb", bufs=1, side="right")

        # ---- input DMAs (sync-queue order = arrival order) ----
        wpool = tc.alloc_tile_pool(name="wqk_sb", bufs=1)     # ..stage A
        tpool = tc.alloc_tile_pool(name="trig", bufs=1)       # ..stage A
        qk_pool = tc.alloc_tile_pool(name="qk", bufs=1)       # ..stage A

        w_t = []
        x_t = {}
        for k in range(KT):
            wt = wpool.tile([128, 2 * CL], bf, tag=f"w{k}", name=f"w{k}")
            nc.sync.dma_start(out=wt, in_=wqk3[:, k])
            w_t.append(wt)
            xt = xpool.tile([128, TC], bf, tag=f"x{k}_0", name=f"x{k}_0")
            nc.sync.dma_start(out=xt, in_=xT3[:, k, 0:TC])
            x_t[(k, 0)] = xt
        cos_sb = tpool.tile([HD, T], bf)
        sin_sb = tpool.tile([HD, T], bf)
        nc.sync.dma_start(out=rt_sb, in_=rt[:, :])
        nc.sync.dma_start(out=ident_bf, in_=ident[:, :])
        nc.sync.dma_start(out=ones_sb, in_=ones[:, :])
        nc.sync.dma_start(out=cos_sb, in_=cosP[:, :])
        nc.sync.dma_start(out=sin_sb, in_=sinP[:, :])
        wv_t = []
        wp_t = []
        for n in range(1, NT):
            for k in range(KT):
                xt = xpool.tile([128, TC], bf, tag=f"x{k}_{n}",
                                name=f"x{k}_{n}")
                nc.sync.dma_start(out=xt, in_=xT3[:, k, n * TC:(n + 1) * TC])
                x_t[(k, n)] = xt
            if n == 1:
                for k in range(KT):
                    wt = wvpool.tile([128, CL], bf, tag=f"wv{k}",
                                     name=f"wv{k}")
                    nc.sync.dma_start(out=wt, in_=wv3[:, k])
                    wv_t.append(wt)

        # ---- stage A: qkT + fused RoPE ----
        rtmp = tc.alloc_tile_pool(name="rope_tmp", bufs=1)

        rope_sb = [rope_pool.tile([128, T], bf, tag=f"ro{m}", name=f"ro{m}")
                   for m in range(8)]

        def emit_rope(m, nsl, qkt):
            # R @ qk on PE (pair swap w/ sign), combine on DVE in bf16.
            psr = ps_r.tile([128, TC], f32, tag="ps_r", name="ps_r")
            nc.tensor.matmul(psr, rt_sb, qkt, start=True, stop=True)
            t1 = rtmp.tile([128, TC], bf, tag="t1", name="t1", bufs=2)
            nc.vector.tensor_mul(t1, psr, sin_sb[:, nsl])
            t2 = rtmp.tile([128, TC], bf, tag="t2", name="t2", bufs=2)
            nc.vector.tensor_mul(t2, qkt, cos_sb[:, nsl])
            nc.vector.tensor_add(rope_sb[m][:, nsl], t1, t2)

        # n=0 runs k-outer in two waves of 4 simultaneously-open psum
        # groups, so the first matmul issues as soon as the first
        # (w[k], x[k,0]) DMA pair lands instead of waiting for the full
        # 6MB stage-A working set. Copies alternate ACT/DVE so the group
        # closes drain across two engines.
        pend = deque()
        nsl0 = slice(0, TC)
        ps_a = tc.alloc_tile_pool(name="ps_a", bufs=4, space="PSUM")
        ps_r = tc.alloc_tile_pool(name="ps_rot", bufs=2, space="PSUM")
        for wave in range(2):
            psw = [ps_a.tile([128, TC], f32, tag="ps_a", name="ps_a")
                   for _ in range(4)]
            for k in range(KT - 1):
                for mi in range(4):
                    m = 4 * wave + mi
                    nc.tensor.matmul(psw[mi],
                                     w_t[k][:, m * 128:(m + 1) * 128],
                                     x_t[(k, 0)], start=(k == 0), stop=False,
                                     skip_group_check=True)
            for mi in range(4):
                # close one group at a time so evacuations stagger
                m = 4 * wave + mi
                nc.tensor.matmul(psw[mi],
                                 w_t[KT - 1][:, m * 128:(m + 1) * 128],
                                 x_t[(KT - 1, 0)], start=False, stop=True,
                                 skip_group_check=True)
                qkt = qk_pool.tile([128, TC], bf, tag="qkt", name="qkt",
                                   bufs=12)
                if mi % 2 == 0:
                    nc.scalar.copy(out=qkt, in_=psw[mi])
                else:
                    nc.vector.tensor_copy(out=qkt, in_=psw[mi])
                pend.append((m, nsl0, qkt))
        for n in range(1, NT):
            nsl = slice(n * TC, (n + 1) * TC)
            for m in range(8):
                ps = ps_a.tile([128, TC], f32, tag="ps_a", name="ps_a")
                for k in range(KT):
                    nc.tensor.matmul(ps, w_t[k][:, m * 128:(m + 1) * 128],
                                     x_t[(k, n)],
                                     start=(k == 0), stop=(k == KT - 1))
                qkt = qk_pool.tile([128, TC], bf, tag="qkt", name="qkt",
                                   bufs=12)
                nc.scalar.copy(out=qkt, in_=ps)
                # rope of an earlier tile: its ACT copy finished during
                # this group's 16 matmuls, so the R-matmul never stalls PE.
                if pend:
                    emit_rope(*pend.popleft())
                pend.append((m, nsl, qkt))
        while pend:
            emit_rope(*pend.popleft())

        rtmp.release()
        ps_r.release()
        ps_a.release()
        qk_pool.release()
        tpool.release()
        wpool.release()

        # ---- stage C: v = x @ Wv ----
        v_pool = tc.alloc_tile_pool(name="v_sb", bufs=1, side="right")
        wppool = tc.alloc_tile_pool(name="wp_sb", bufs=1, side="right")
        y_pool = tc.alloc_tile_pool(name="yT_sb", bufs=1, side="right")
        for hk in range(HPC):
            wt = wppool.tile([128, C], bf, tag=f"wp{hk}", name=f"wp{hk}")
            nc.sync.dma_start(out=wt, in_=wp3[:, hk])
            wp_t.append(wt)
        v_t = [v_pool.tile([128, CL], bf, tag=f"v{mt}", name=f"v{mt}")
               for mt in range(TT)]
        ps_c = tc.alloc_tile_pool(name="ps_c", bufs=4, space="PSUM")
        for mt in range(TT):
            ps = ps_c.tile([128, CL], f32, tag="ps_c", name="ps_c")
            n, off = mt // 4, (mt % 4) * 128
            for k in range(KT):
                nc.tensor.matmul(ps, x_t[(k, n)][:, off:off + 128], wv_t[k],
                                 start=(k == 0), stop=(k == KT - 1))
            nc.scalar.copy(out=v_t[mt], in_=ps)
        ps_c.release()
        xpool.release()

        # ---- stage D attention + stage E (proj) as PE filler ----
        yT = [y_pool.tile([128, T], bf, tag=f"yT{h}", name=f"yT{h}")
              for h in range(HPC)]

        maskpool = tc.alloc_tile_pool(name="maskp", bufs=1)
        mask_sb = maskpool.tile([128, 2, 2 * TC], bf, name="mask_sb")
        nc.sync.dma_start(out=mask_sb, in_=masks3)
        pp_pool = tc.alloc_tile_pool(name="pp", bufs=1)
        acc_pool = tc.alloc_tile_pool(name="accp", bufs=1)
        sm_pool = tc.alloc_tile_pool(name="small", bufs=1)
        o_pool = tc.alloc_tile_pool(name="ostage", bufs=1)
        ps_s = tc.alloc_tile_pool(name="ps_s", bufs=2, space="PSUM")
        ps_y = tc.alloc_tile_pool(name="ps_y", bufs=2, space="PSUM")
        ps_d = tc.alloc_tile_pool(name="ps_d", bufs=1, space="PSUM")
        ps_o = tc.alloc_tile_pool(name="ps_o", bufs=1, space="PSUM")

        e_jobs = deque()
        e_count = [0]
        e_pool = [ps_o]

        def emit_e_group():
            if not e_jobs:
                return
            mt, n2 = e_jobs.popleft()
            msl = slice(mt * 128, (mt + 1) * 128)
            nsl = slice(n2 * TC, (n2 + 1) * TC)
            pso = e_pool[0].tile([128, TC], f32, tag="o", name="o_ps")
            for hk in range(HPC):
                nc.tensor.matmul(pso, yT[hk][:, msl], wp_t[hk][:, nsl],
                                 start=(hk == 0), stop=(hk == HPC - 1))
            ot = o_pool.tile([128, TC], f32, tag="ot", name="ot", bufs=8)
            # alternate evacuation engine to balance ACT vs DVE load
            if e_count[0] % 2 == 0:
                nc.scalar.copy(out=ot, in_=pso)
            else:
                nc.vector.tensor_copy(out=ot, in_=pso)
            e_count[0] += 1
            nc.sync.dma_start(out=out[msl, nsl], in_=ot)

        # Attention runs as one flat software pipeline over all
        # (chunk, head) batches: PV matmuls lag the S matmuls by LAG
        # batches, so exp latency never head-of-line blocks the PE —
        # including chunk I=0 (which has no E-filler yet) and at every
        # chunk boundary.
        def emit_norm(I, h, acc, y_ps):
            isl = slice(I * TC, (I + 1) * TC)
            d_ps = ps_d.tile([128, 4], f32, tag="d", name="d_ps")
            for qq in range(4):
                nc.tensor.matmul(d_ps[:, qq:qq + 1],
                                 acc[:, qq * 128:(qq + 1) * 128],
                                 ones_sb, start=True, stop=True,
                                 skip_group_check=True)
            recip = sm_pool.tile([128, 4], bf, tag="recip",
                                 name="recip", bufs=2)
            with nc.allow_low_precision(
                    reason="1/d in bf16: 0.4% on softmax scale, "
                           "validated 6e-3 rel err end to end"):
                nc.vector.reciprocal(out=recip, in_=d_ps)
            # 4 column transposes into one [1, 512] psum row so the
            # gpsimd broadcast reads from partition 0 in one shot;
            # shares ps_d's bank (sequential with d_ps by data deps).
            # bf16 keeps the transposes single-pass (fp32 is LOW_HIGH).
            tT_ps = ps_d.tile([1, TC], bf, tag="d", name="tT_ps")
            for qq in range(4):
                nc.tensor.transpose(tT_ps[:, qq * 128:(qq + 1) * 128],
                                    recip[:, qq:qq + 1], ident_bf)
            recipT = sm_pool.tile([1, TC], bf, tag="recipT",
                                  name="recipT", bufs=2)
            nc.vector.tensor_copy(out=recipT, in_=tT_ps)
            recipB = sm_pool.tile([128, TC], bf, tag="recipB",
                                  name="recipB", bufs=2)
            nc.gpsimd.partition_broadcast(recipB, recipT)
            nc.vector.tensor_mul(yT[h][:, isl], y_ps, recipB)

        steps = []
        for I in range(NT):
            bt = _BATCHES[I]
            for h in range(HPC):
                for bi, (j0, mp) in enumerate(bt):
                    steps.append((I, h, bi, len(bt), j0, mp))

        LAG = 2
        chunk_state = {}   # (I, h) -> [acc, y_ps]
        pv_q = deque()     # (I, h, bi, nb, j0, src)

        def pop_pv():
            I, h, bi, nb, j0, mp, src = pv_q.popleft()
            if bi == 0:
                chunk_state[(I, h)][1] = ps_y.tile([128, TC], f32, tag="y",
                                                   name="y_ps")
            y_ps = chunk_state[(I, h)][1]
            for half in range(2):
                J = j0 + half
                lo = 0 if mp is None else 128 * (2 * mp + half)
                nc.tensor.matmul(
                    y_ps[:, lo:TC], v_t[J][:, h * 128:(h + 1) * 128],
                    src[:, half * TC + lo:(half + 1) * TC],
                    start=(bi == 0 and half == 0),
                    stop=(bi == nb - 1 and half == 1),
                )
            if bi == nb - 1:
                acc, y_ps = chunk_state.pop((I, h))
                emit_norm(I, h, acc, y_ps)
                if h == HPC - 1:
                    for ml in range(4):
                        for n2 in range(NT):
                            e_jobs.append((4 * I + ml, n2))

        for (I, h, bi, nb, j0, mp) in steps:
            q_h = rope_sb[h]
            k_h = rope_sb[4 + h]
            s_ps = ps_s.tile([128, 2 * TC], f32, tag="s", name="s_ps")
            # For diagonal tiles d = 2*mp+half the first 128*d query
            # columns are fully masked: compute only the live range.
            los = [0, 0] if mp is None else [128 * (2 * mp + ha)
                                             for ha in range(2)]
            for half in range(2):
                J = j0 + half
                nc.tensor.matmul(
                    s_ps[:, half * TC + los[half]:(half + 1) * TC],
                    k_h[:, J * 128:(J + 1) * 128],
                    q_h[:, I * TC + los[half]:(I + 1) * TC],
                    start=True, stop=True, skip_group_check=True,
                )
            pp = pp_pool.tile([128, 2 * TC], bf, tag="pp", name="pp",
                              bufs=LAG + 2)
            if mp is None:
                nc.scalar.activation(out=pp, in_=s_ps, func=Exp, scale=SCALE)
            else:
                for half in range(2):
                    sl = slice(half * TC + los[half], (half + 1) * TC)
                    nc.scalar.activation(out=pp[:, sl], in_=s_ps[:, sl],
                                         func=Exp, scale=SCALE)
            emit_e_group()
            src = pp
            if mp is not None:
                ppm = pp_pool.tile([128, 2 * TC], bf, tag="ppm",
                                   name="ppm", bufs=LAG + 2)
                for half in range(2):
                    sl = slice(half * TC + los[half], (half + 1) * TC)
                    nc.vector.tensor_mul(ppm[:, sl], pp[:, sl],
                                         mask_sb[:, mp][:, sl])
                src = ppm
            if bi == 0:
                acc = acc_pool.tile([128, TC], bf, tag="acc", name="acc",
                                    bufs=3)
                chunk_state[(I, h)] = [acc, None]
                nc.vector.tensor_copy(out=acc, in_=src[:, 0:TC])
            else:
                acc = chunk_state[(I, h)][0]
                nc.vector.tensor_add(acc[:, los[0]:TC], acc[:, los[0]:TC],
                                     src[:, los[0]:TC])
            nc.vector.tensor_add(acc[:, los[1]:TC], acc[:, los[1]:TC],
                                 src[:, TC + los[1]:2 * TC])
            pv_q.append((I, h, bi, nb, j0, mp, src))
            if len(pv_q) > LAG:
                pop_pv()
        while pv_q:
            pop_pv()
        # tail: attention psum pools are done; hand the remaining E groups
        # a 4-deep psum pool so the group->evacuate->DMA chain pipelines
        ps_o.release()
        ps_d.release()
        ps_y.release()
        ps_s.release()
        ps_tail = tc.alloc_tile_pool(name="ps_tail", bufs=6, space="PSUM")
        e_pool[0] = ps_tail
        while e_jobs:
            emit_e_group()

        for p in (o_pool, sm_pool, acc_pool, pp_pool, maskpool, rope_pool,
                  mpool, y_pool, wppool, v_pool, wvpool, ps_tail):
            p.release()
    nc.compile()
    return nc


def _host_prep(x, w_qkv, w_proj, freqs_cis):
    """Build per-core input maps (slicing + layout + dtype prep only)."""
    import ml_dtypes
    BF = ml_dtypes.bfloat16

    x = np.asarray(x, dtype=np.float32)
    w_qkv = np.asarray(w_qkv, dtype=np.float32)
    w_proj = np.asarray(w_proj, dtype=np.float32)
    fc = np.asarray(freqs_cis, dtype=np.float32)

    xTb = [np.ascontiguousarray(x[b].T).astype(BF) for b in range(B)]

    cos = fc[:, :, 0].T  # [64, T]
    sin = fc[:, :, 1].T
    cosP = np.repeat(cos, 2, axis=0).astype(BF)  # [128, T]
    sinP = np.repeat(sin, 2, axis=0).astype(BF)

    rt = np.zeros((HD, HD), dtype=np.float32)
    for d in range(HD // 2):
        rt[2 * d, 2 * d + 1] = 1.0
        rt[2 * d + 1, 2 * d] = -1.0
    rt = rt.astype(BF)

    # masks[p][jj, u]: pair p covers diagonal tiles d = 2p + u//TC
    masks = np.zeros((2, 128, 2 * TC), dtype=np.float32)
    jj = np.arange(128)[:, None]
    for p in range(2):
        for tp in range(2):
            d = 2 * p + tp
            ii = np.arange(TC)[None, :]
            masks[p][:, tp * TC:(tp + 1) * TC] = (ii >= jj + 128 * d)
    masks = masks.astype(BF)

    ones = np.ones((128, 1), dtype=np.float32).astype(BF)
    ident = np.eye(128, dtype=np.float32).astype(BF)

    in_maps = []
    for core in range(8):
        b = core // 4
        g = core % 4
        qc = w_qkv[:, 512 * g: 512 * (g + 1)]
        kc = w_qkv[:, 2048 + 512 * g: 2048 + 512 * (g + 1)]
        vc = w_qkv[:, 4096 + 512 * g: 4096 + 512 * (g + 1)]
        wqk_c = np.concatenate([qc, kc], axis=1).astype(BF)
        wv_c = np.ascontiguousarray(vc).astype(BF)
        wp_c = np.ascontiguousarray(
            w_proj[512 * g: 512 * (g + 1), :]).astype(BF)
        in_maps.append({
            "xT": xTb[b],
            "wqk": wqk_c,
            "wv": wv_c,
            "wp": wp_c,
            "cosP": cosP,
            "sinP": sinP,
            "rt": rt,
            "masks": masks,
            "ones": ones,
            "ident": ident,
        })
    return in_maps


def _get_nc():
    if "nc" not in _CACHE:
        _CACHE["nc"] = _build_nc()
    return _CACHE["nc"]


def kernel(x, w_qkv, w_proj, freqs_cis, attn_mask, _trace=False):
    from concourse.bass_utils import run_bass_kernel_spmd

    in_maps = _host_prep(x, w_qkv, w_proj, freqs_cis)
    nc = _get_nc()
    res = run_bass_kernel_spmd(
        nc, in_maps, core_ids=list(range(8)), trace=_trace,
    )
    outs = [r["out"].astype(np.float64) for r in res.results]
    full = np.stack([
        outs[0] + outs[1] + outs[2] + outs[3],
        outs[4] + outs[5] + outs[6] + outs[7],
    ]).astype(np.float32)
    if _trace:
        kernel._last_results = res
    return full


# revision 31
# speedup vs baseline: 1.8705x; 1.0056x over previous
"""Trainium2 Bass kernel for prefix-LM CausalSelfAttention (v2).

Problem: B=2, T=2048, C=2048, H=16 heads (hd=128), prefix-LM mask
(bidirectional over first half, causal after), RoPE on q/k.

Sharding over 8 cores: data-parallel on batch (2) x tensor-parallel on
heads (4 heads per core). Each core computes a partial output projection
(its heads' contribution); partials are summed on host.

v2 design (vs v1): bf16 data everywhere (validated 6.6e-3 rel err), x
resident in SBUF once (no second DMA pass), RoPE fused per-(m,chunk)
into stage A so DVE work hides under the QKV matmuls, attention exp
batched 2 key-tiles wide on ACT, softmax denominator via DVE-accumulated
pp sum + 4 tiny transposed matmuls + [128,4] reciprocal (replaces the
per-tile ones-matmuls and the 3.3us single-lane [1,512] reciprocal),
and the output projection interleaved into attention as PE filler.

Per-core dataflow:
  A. qkT[m] = W_{q,k}^T @ x^T per 512-chunk; RoPE combine per tile:
     rope = qkT*cos + (R @ qkT)*sin  (R = pair swap w/ sign)
  C. v[t-tile] = x @ Wv (natural layout)
  D. per (I, h): S'[j,i] tiles via k^T-tile x q-chunk, exp on ACT
     (2 tiles per ACTIVATE), pp accumulated on DVE for the denominator,
     PV accumulation into y^T psum; dT = pp_acc^T @ ones via 4 M=1
     matmuls, reciprocal, PE-transpose, gpsimd row broadcast, normalize.
  E. out[mt, n] = sum_hk yT[hk]^T @ Wp[hk], emitted as PE filler between
     attention batches; evacuation alternates ACT/DVE.
"""
import math

import numpy as np

N_HEAD = 16
B = 2
T = 2048
C = 2048
HD = 128
HPC = 4          # heads per core
CL = HPC * HD    # local C = 512
TC = 512         # chunk width (matmul moving free dim / psum bank)
NT = T // TC     # 4 chunks
KT = C // 128    # 16 contraction tiles over C
TT = T // 128    # 16 T tiles
SCALE = 1.0 / math.sqrt(HD)

# Per query-chunk I: batches of two 128-key tiles (j0, j0+1); mp indexes
# the two 1024-wide diagonal mask pairs, None for fully-allowed batches.
_BATCHES = {
    0: [(0, None), (2, None), (4, None), (6, None)],
    1: [(0, None), (2, None), (4, None), (6, None)],
    2: [(0, None), (2, None), (4, None), (6, None), (8, 0), (10, 1)],
    3: [(0, None), (2, None), (4, None), (6, None), (8, None), (10, None),
        (12, 0), (14, 1)],
}

_CACHE = {}


def _build_nc():
    from collections import deque

    import concourse.tile as tile
    import concourse.mybir as mybir
    from concourse import bacc

    f32 = mybir.dt.float32
    f32r = mybir.dt.float32r
    bf = mybir.dt.bfloat16
    Exp = mybir.ActivationFunctionType.Exp

    nc = bacc.Bacc(None, target_bir_lowering=False)

    xT = nc.dram_tensor("xT", [C, T], bf, kind="ExternalInput")
    wqk = nc.dram_tensor("wqk", [C, 2 * CL], bf, kind="ExternalInput")
    wv = nc.dram_tensor("wv", [C, CL], bf, kind="ExternalInput")
    wp = nc.dram_tensor("wp", [CL, C], bf, kind="ExternalInput")
    cosP = nc.dram_tensor("cosP", [HD, T], bf, kind="ExternalInput")
    sinP = nc.dram_tensor("sinP", [HD, T], bf, kind="ExternalInput")
    rt = nc.dram_tensor("rt", [HD, HD], bf, kind="ExternalInput")
    masks = nc.dram_tensor("masks", [2, 128, 2 * TC], bf, kind="ExternalInput")
    ones = nc.dram_tensor("ones", [128, 1], bf, kind="ExternalInput")
    ident = nc.dram_tensor("ident", [128, 128], bf, kind="ExternalInput")
    out = nc.dram_tensor("out", [T, C], bf, kind="ExternalOutput")

    xT3 = xT.rearrange("(kt p) t -> p kt t", p=128)
    wqk3 = wqk.rearrange("(kt p) m -> p kt m", p=128)
    wv3 = wv.rearrange("(kt p) m -> p kt m", p=128)
    wp3 = wp.rearrange("(hk p) m -> p hk m", p=128)
    masks3 = masks.rearrange("g p u -> p g u")

    with tile.TileContext(nc) as tc:
        # Left stack: mpool/rope (long-lived), then x (..stage C), then the
        # per-phase pools on top in LIFO order. Right stack: v/wp/yT which
        # outlive x. PSUM pools form their own stack.
        mpool = tc.alloc_tile_pool(name="misc", bufs=1)
        rope_pool = tc.alloc_tile_pool(name="rope", bufs=1)   # ..attention
        xpool = tc.alloc_tile_pool(name="x_sb", bufs=1)       # ..stage C

        rt_sb = mpool.tile([HD, HD], bf)
        ones_sb = mpool.tile([128, 1], bf)
        ident_bf = mpool.tile([128, 128], bf)
        warm_sb = mpool.tile([128, 128], bf)
        dume_sb = mpool.tile([128, 2], bf)

        # HAM warmup: PE matmuls on memset data while input DMAs stream,
        # so stage A starts at K=8/8. Also pre-trigger the exp table load
        # and the gpsimd library load (first partition_broadcast otherwise
        # costs ~9us mid-attention).
        nc.vector.memset(warm_sb, 0.0)
        nc.scalar.activation(out=dume_sb, in_=warm_sb[:, 0:2], func=Exp)
        dumg_sb = mpool.tile([128, 4], bf)
        nc.gpsimd.partition_broadcast(dumg_sb, warm_sb[0:1, 0:4])
        ps_w = tc.alloc_tile_pool(name="ps_warm", bufs=1, space="PSUM")
        for _ in range(44):
            pw = ps_w.tile([128, 128], f32, tag="pw", name="pw")
            nc.tensor.matmul(pw, warm_sb, warm_sb, start=True, stop=True)
        ps_w.release()

        # Long-lived pools on the right stack so their DMAs land in fresh
        # address space (no WAR on released stage-A pools) and can be
        # emitted early in the sync queue.
        wvpool = tc.alloc_tile_pool(name="wv_sb", bufs=1, side="right")

        # ---- input DMAs (sync-queue order = arrival order) ----
        wpool = tc.alloc_tile_pool(name="wqk_sb", bufs=1)     # ..stage A
        tpool = tc.alloc_tile_pool(name="trig", bufs=1)       # ..stage A
        qk_pool = tc.alloc_tile_pool(name="qk", bufs=1)       # ..stage A

        w_t = []
        x_t = {}
        for k in range(KT):
            wt = wpool.tile([128, 2 * CL], bf, tag=f"w{k}", name=f"w{k}")
            nc.sync.dma_start(out=wt, in_=wqk3[:, k])
            w_t.append(wt)
            xt = xpool.tile([128, TC], bf, tag=f"x{k}_0", name=f"x{k}_0")
            nc.sync.dma_start(out=xt, in_=xT3[:, k, 0:TC])
            x_t[(k, 0)] = xt
        cos_sb = tpool.tile([HD, T], bf)
        sin_sb = tpool.tile([HD, T], bf)
        nc.sync.dma_start(out=rt_sb, in_=rt[:, :])
        nc.sync.dma_start(out=ident_bf, in_=ident[:, :])
        nc.sync.dma_start(out=ones_sb, in_=ones[:, :])
        nc.sync.dma_start(out=cos_sb, in_=cosP[:, :])
        nc.sync.dma_start(out=sin_sb, in_=sinP[:, :])
        wv_t = []
        wp_t = []
        for n in range(1, NT):
            for k in range(KT):
                xt = xpool.tile([128, TC], bf, tag=f"x{k}_{n}",
                                name=f"x{k}_{n}")
                nc.sync.dma_start(out=xt, in_=xT3[:, k, n * TC:(n + 1) * TC])
                x_t[(k, n)] = xt
            if n == 1:
                for k in range(KT):
                    wt = wvpool.tile([128, CL], bf, tag=f"wv{k}",
                                     name=f"wv{k}")
                    nc.sync.dma_start(out=wt, in_=wv3[:, k])
                    wv_t.append(wt)

        # ---- stage A: qkT + fused RoPE ----
        rtmp = tc.alloc_tile_pool(name="rope_tmp", bufs=1)

        rope_sb = [rope_pool.tile([128, T], bf, tag=f"ro{m}", name=f"ro{m}")
                   for m in range(8)]

        def emit_rope(m, nsl, qkt):
            # R @ qk on PE (pair swap w/ sign), combine on DVE in bf16.
            psr = ps_r.tile([128, TC], f32, tag="ps_r", name="ps_r")
            nc.tensor.matmul(psr, rt_sb, qkt, start=True, stop=True)
            t1 = rtmp.tile([128, TC], bf, tag="t1", name="t1", bufs=2)
            nc.vector.tensor_mul(t1, psr, sin_sb[:, nsl])
            t2 = rtmp.tile([128, TC], bf, tag="t2", name="t2", bufs=2)
            nc.vector.tensor_mul(t2, qkt, cos_sb[:, nsl])
            nc.vector.tensor_add(rope_sb[m][:, nsl], t1, t2)

        # n=0 runs k-outer in two waves of 4 simultaneously-open psum
        # groups, so the first matmul issues as soon as the first
        # (w[k], x[k,0]) DMA pair lands instead of waiting for the full
        # 6MB stage-A working set. Copies alternate ACT/DVE so the group
        # closes drain across two engines.
        pend = deque()
        nsl0 = slice(0, TC)
        ps_a = tc.alloc_tile_pool(name="ps_a", bufs=4, space="PSUM")
        ps_r = tc.alloc_tile_pool(name="ps_rot", bufs=2, space="PSUM")
        for wave in range(2):
            psw = [ps_a.tile([128, TC], f32, tag="ps_a", name="ps_a")
                   for _ in range(4)]
            for k0 in range(0, KT - 2, 2):
                for mi in range(4):
                    m = 4 * wave + mi
                    for k in (k0, k0 + 1):
                        nc.tensor.matmul(psw[mi],
                                         w_t[k][:, m * 128:(m + 1) * 128],
                                         x_t[(k, 0)], start=(k == 0),
                                         stop=False, skip_group_check=True)
            for mi in range(4):
                # close one group at a time so evacuations stagger
                m = 4 * wave + mi
                for k in (KT - 2, KT - 1):
                    nc.tensor.matmul(psw[mi],
                                     w_t[k][:, m * 128:(m + 1) * 128],
                                     x_t[(k, 0)], start=False,
                                     stop=(k == KT - 1),
                                     skip_group_check=True)
                qkt = qk_pool.tile([128, TC], bf, tag="qkt", name="qkt",
                                   bufs=12)
                if mi % 2 == 0:
                    nc.scalar.copy(out=qkt, in_=psw[mi])
                else:
                    nc.vector.tensor_copy(out=qkt, in_=psw[mi])
                pend.append((m, nsl0, qkt))
        for n in range(1, NT):
            nsl = slice(n * TC, (n + 1) * TC)
            for m in range(8):
                ps = ps_a.tile([128, TC], f32, tag="ps_a", name="ps_a")
                for k in range(KT):
                    nc.tensor.matmul(ps, w_t[k][:, m * 128:(m + 1) * 128],
                                     x_t[(k, n)],
                                     start=(k == 0), stop=(k == KT - 1))
                qkt = qk_pool.tile([128, TC], bf, tag="qkt", name="qkt",
                                   bufs=12)
                nc.scalar.copy(out=qkt, in_=ps)
                # rope of an earlier tile: its ACT copy finished during
                # this group's 16 matmuls, so the R-matmul never stalls PE.
                if pend:
                    emit_rope(*pend.popleft())
                pend.append((m, nsl, qkt))
        while pend:
            emit_rope(*pend.popleft())

        rtmp.release()
        ps_r.release()
        ps_a.release()
        qk_pool.release()
        tpool.release()
        wpool.release()

        # ---- stage C: v = x @ Wv ----
        v_pool = tc.alloc_tile_pool(name="v_sb", bufs=1, side="right")
        wppool = tc.alloc_tile_pool(name="wp_sb", bufs=1, side="right")
        y_pool = tc.alloc_tile_pool(name="yT_sb", bufs=1, side="right")
        for hk in range(HPC):
            wt = wppool.tile([128, C], bf, tag=f"wp{hk}", name=f"wp{hk}")
            nc.sync.dma_start(out=wt, in_=wp3[:, hk])
            wp_t.append(wt)
        v_t = [v_pool.tile([128, CL], bf, tag=f"v{mt}", name=f"v{mt}")
               for mt in range(TT)]
        ps_c = tc.alloc_tile_pool(name="ps_c", bufs=4, space="PSUM")
        for mt in range(TT):
            ps = ps_c.tile([128, CL], f32, tag="ps_c", name="ps_c")
            n, off = mt // 4, (mt % 4) * 128
            for k in range(KT):
                nc.tensor.matmul(ps, x_t[(k, n)][:, off:off + 128], wv_t[k],
                                 start=(k == 0), stop=(k == KT - 1))
            nc.scalar.copy(out=v_t[mt], in_=ps)
        ps_c.release()
        xpool.release()

        # ---- stage D attention + stage E (proj) as PE filler ----
        yT = [y_pool.tile([128, T], bf, tag=f"yT{h}", name=f"yT{h}")
              for h in range(HPC)]

        maskpool = tc.alloc_tile_pool(name="maskp", bufs=1)
        mask_sb = maskpool.tile([128, 2, 2 * TC], bf, name="mask_sb")
        nc.sync.dma_start(out=mask_sb, in_=masks3)
        pp_pool = tc.alloc_tile_pool(name="pp", bufs=1)
        acc_pool = tc.alloc_tile_pool(name="accp", bufs=1)
        sm_pool = tc.alloc_tile_pool(name="small", bufs=1)
        o_pool = tc.alloc_tile_pool(name="ostage", bufs=1)
        ps_s = tc.alloc_tile_pool(name="ps_s", bufs=2, space="PSUM")
        ps_y = tc.alloc_tile_pool(name="ps_y", bufs=2, space="PSUM")
        ps_d = tc.alloc_tile_pool(name="ps_d", bufs=1, space="PSUM")
        ps_o = tc.alloc_tile_pool(name="ps_o", bufs=1, space="PSUM")

        e_jobs = deque()
        e_count = [0]
        e_pool = [ps_o]

        def emit_e_group():
            if not e_jobs:
                return
            mt, n2 = e_jobs.popleft()
            msl = slice(mt * 128, (mt + 1) * 128)
            nsl = slice(n2 * TC, (n2 + 1) * TC)
            pso = e_pool[0].tile([128, TC], f32, tag="o", name="o_ps")
            for hk in range(HPC):
                nc.tensor.matmul(pso, yT[hk][:, msl], wp_t[hk][:, nsl],
                                 start=(hk == 0), stop=(hk == HPC - 1))
            ot = o_pool.tile([128, TC], bf, tag="ot", name="ot", bufs=8)
            # alternate evacuation engine to balance ACT vs DVE load
            if e_count[0] % 2 == 0:
                nc.scalar.copy(out=ot, in_=pso)
            else:
                nc.vector.tensor_copy(out=ot, in_=pso)
            e_count[0] += 1
            nc.sync.dma_start(out=out[msl, nsl], in_=ot)

        # Attention runs as one flat software pipeline over all
        # (chunk, head) batches: PV matmuls lag the S matmuls by LAG
        # batches, so exp latency never head-of-line blocks the PE —
        # including chunk I=0 (which has no E-filler yet) and at every
        # chunk boundary.
        def emit_norm(I, h, acc, y_ps):
            isl = slice(I * TC, (I + 1) * TC)
            d_ps = ps_d.tile([128, 4], f32, tag="d", name="d_ps")
            for qq in range(4):
                nc.tensor.matmul(d_ps[:, qq:qq + 1],
                                 acc[:, qq * 128:(qq + 1) * 128],
                                 ones_sb, start=True, stop=True,
                                 skip_group_check=True)
            recip = sm_pool.tile([128, 4], bf, tag="recip",
                                 name="recip", bufs=2)
            with nc.allow_low_precision(
                    reason="1/d in bf16: 0.4% on softmax scale, "
                           "validated 6e-3 rel err end to end"):
                nc.vector.reciprocal(out=recip, in_=d_ps)
            # 4 column transposes into one [1, 512] psum row so the
            # gpsimd broadcast reads from partition 0 in one shot;
            # shares ps_d's bank (sequential with d_ps by data deps).
            # bf16 keeps the transposes single-pass (fp32 is LOW_HIGH).
            tT_ps = ps_d.tile([1, TC], bf, tag="d", name="tT_ps")
            for qq in range(4):
                nc.tensor.transpose(tT_ps[:, qq * 128:(qq + 1) * 128],
                                    recip[:, qq:qq + 1], ident_bf)
            recipT = sm_pool.tile([1, TC], bf, tag="recipT",
                                  name="recipT", bufs=2)
            nc.vector.tensor_copy(out=recipT, in_=tT_ps)
            recipB = sm_pool.tile([128, TC], bf, tag="recipB",
                                  name="recipB", bufs=2)
            nc.gpsimd.partition_broadcast(recipB, recipT)
            nc.vector.tensor_mul(yT[h][:, isl], y_ps, recipB)

        steps = []
        for I in range(NT):
            bt = _BATCHES[I]
            for h in range(HPC):
                for bi, (j0, mp) in enumerate(bt):
                    steps.append((I, h, bi, len(bt), j0, mp))

        LAG = 2
        chunk_state = {}   # (I, h) -> [acc, y_ps]
        pv_q = deque()     # (I, h, bi, nb, j0, src)

        def pop_pv():
            I, h, bi, nb, j0, mp, src = pv_q.popleft()
            if bi == 0:
                chunk_state[(I, h)][1] = ps_y.tile([128, TC], f32, tag="y",
                                                   name="y_ps")
            y_ps = chunk_state[(I, h)][1]
            for half in range(2):
                J = j0 + half
                lo = 0 if mp is None else 128 * (2 * mp + half)
                nc.tensor.matmul(
                    y_ps[:, lo:TC], v_t[J][:, h * 128:(h + 1) * 128],
                    src[:, half * TC + lo:(half + 1) * TC],
                    start=(bi == 0 and half == 0),
                    stop=(bi == nb - 1 and half == 1),
                )
            if bi == nb - 1:
                acc, y_ps = chunk_state.pop((I, h))
                emit_norm(I, h, acc, y_ps)
                if h == HPC - 1:
                    for ml in range(4):
                        for n2 in range(NT):
                            e_jobs.append((4 * I + ml, n2))

        for (I, h, bi, nb, j0, mp) in steps:
            q_h = rope_sb[h]
            k_h = rope_sb[4 + h]
            s_ps = ps_s.tile([128, 2 * TC], f32, tag="s", name="s_ps")
            # For diagonal tiles d = 2*mp+half the first 128*d query
            # columns are fully masked: compute only the live range.
            los = [0, 0] if mp is None else [128 * (2 * mp + ha)
                                             for ha in range(2)]
            for half in range(2):
                J = j0 + half
                nc.tensor.matmul(
                    s_ps[:, half * TC + los[half]:(half + 1) * TC],
                    k_h[:, J * 128:(J + 1) * 128],
                    q_h[:, I * TC + los[half]:(I + 1) * TC],
                    start=True, stop=True, skip_group_check=True,
                )
            pp = pp_pool.tile([128, 2 * TC], bf, tag="pp", name="pp",
                              bufs=LAG + 2)
            if mp is None:
                nc.scalar.activation(out=pp, in_=s_ps, func=Exp, scale=SCALE)
            else:
                for half in range(2):
                    sl = slice(half * TC + los[half], (half + 1) * TC)
                    nc.scalar.activation(out=pp[:, sl], in_=s_ps[:, sl],
                                         func=Exp, scale=SCALE)
            emit_e_group()
            src = pp
            if mp is not None:
                ppm = pp_pool.tile([128, 2 * TC], bf, tag="ppm",
                                   name="ppm", bufs=LAG + 2)
                for half in range(2):
                    sl = slice(half * TC + los[half], (half + 1) * TC)
                    nc.vector.tensor_mul(ppm[:, sl], pp[:, sl],
                                         mask_sb[:, mp][:, sl])
                src = ppm
            if bi == 0:
                acc = acc_pool.tile([128, TC], bf, tag="acc", name="acc",
                                    bufs=3)
                chunk_state[(I, h)] = [acc, None]
                nc.vector.tensor_copy(out=acc, in_=src[:, 0:TC])
            else:
                acc = chunk_state[(I, h)][0]
                nc.vector.tensor_add(acc[:, los[0]:TC], acc[:, los[0]:TC],
                                     src[:, los[0]:TC])
            nc.vector.tensor_add(acc[:, los[1]:TC], acc[:, los[1]:TC],
                                 src[:, TC + los[1]:2 * TC])
            pv_q.append((I, h, bi, nb, j0, mp, src))
            if len(pv_q) > LAG:
                pop_pv()
        while pv_q:
            pop_pv()
        # tail: attention psum pools are done; hand the remaining E groups
        # a 4-deep psum pool so the group->evacuate->DMA chain pipelines
        ps_o.release()
        ps_d.release()
        ps_y.release()
        ps_s.release()
        ps_tail = tc.alloc_tile_pool(name="ps_tail", bufs=6, space="PSUM")
        e_pool[0] = ps_tail
        while e_jobs:
            emit_e_group()

        for p in (o_pool, sm_pool, acc_pool, pp_pool, maskpool, rope_pool,
                  mpool, y_pool, wppool, v_pool, wvpool, ps_tail):
            p.release()
    nc.compile()
    return nc


def _host_prep(x, w_qkv, w_proj, freqs_cis):
    """Build per-core input maps (slicing + layout + dtype prep only)."""
    import ml_dtypes
    BF = ml_dtypes.bfloat16

    x = np.asarray(x, dtype=np.float32)
    w_qkv = np.asarray(w_qkv, dtype=np.float32)
    w_proj = np.asarray(w_proj, dtype=np.float32)
    fc = np.asarray(freqs_cis, dtype=np.float32)

    xTb = [np.ascontiguousarray(x[b].T).astype(BF) for b in range(B)]

    cos = fc[:, :, 0].T  # [64, T]
    sin = fc[:, :, 1].T
    cosP = np.repeat(cos, 2, axis=0).astype(BF)  # [128, T]
    sinP = np.repeat(sin, 2, axis=0).astype(BF)

    rt = np.zeros((HD, HD), dtype=np.float32)
    for d in range(HD // 2):
        rt[2 * d, 2 * d + 1] = 1.0
        rt[2 * d + 1, 2 * d] = -1.0
    rt = rt.astype(BF)

    # masks[p][jj, u]: pair p covers diagonal tiles d = 2p + u//TC
    masks = np.zeros((2, 128, 2 * TC), dtype=np.float32)
    jj = np.arange(128)[:, None]
    for p in range(2):
        for tp in range(2):
            d = 2 * p + tp
            ii = np.arange(TC)[None, :]
            masks[p][:, tp * TC:(tp + 1) * TC] = (ii >= jj + 128 * d)
    masks = masks.astype(BF)

    ones = np.ones((128, 1), dtype=np.float32).astype(BF)
    ident = np.eye(128, dtype=np.float32).astype(BF)

    in_maps = []
    for core in range(8):
        b = core // 4
        g = core % 4
        qc = w_qkv[:, 512 * g: 512 * (g + 1)]
        kc = w_qkv[:, 2048 + 512 * g: 2048 + 512 * (g + 1)]
        vc = w_qkv[:, 4096 + 512 * g: 4096 + 512 * (g + 1)]
        wqk_c = np.concatenate([qc, kc], axis=1).astype(BF)
        wv_c = np.ascontiguousarray(vc).astype(BF)
        wp_c = np.ascontiguousarray(
            w_proj[512 * g: 512 * (g + 1), :]).astype(BF)
        in_maps.append({
            "xT": xTb[b],
            "wqk": wqk_c,
            "wv": wv_c,
            "wp": wp_c,
            "cosP": cosP,
            "sinP": sinP,
            "rt": rt,
            "masks": masks,
            "ones": ones,
            "ident": ident,
        })
    return in_maps


def _get_nc():
    if "nc" not in _CACHE:
        _CACHE["nc"] = _build_nc()
    return _CACHE["nc"]


def kernel(x, w_qkv, w_proj, freqs_cis, attn_mask, _trace=False):
    from concourse.bass_utils import run_bass_kernel_spmd

    in_maps = _host_prep(x, w_qkv, w_proj, freqs_cis)
    nc = _get_nc()
    res = run_bass_kernel_spmd(
        nc, in_maps, core_ids=list(range(8)), trace=_trace,
    )
    outs = [r["out"].astype(np.float64) for r in res.results]
    full = np.stack([
        outs[0] + outs[1] + outs[2] + outs[3],
        outs[4] + outs[5] + outs[6] + outs[7],
    ]).astype(np.float32)
    if _trace:
        kernel._last_results = res
    return full
